# revision 1
# baseline (speedup 1.0000x reference)
"""Multi-head attention (B=2, S=2048, D=1024, H=16) on 8 Trainium2 cores.

Sharding: (batch, head-group-of-4) -> 8 cores, Megatron-style. Core c
handles batch c//4 and heads 4*(c%4)..4*(c%4)+3 (d_local = 256 columns of
Wq/Wk/Wv, 256 rows of Wo). Each core computes a partial [2048, 1024]
output; the host sums the 4 partials per batch (row-parallel Wo).

Key-side truncation: softmax keys are masked per batch to valid_lens;
only ceil(max(valid_lens)/128) key tiles are ever computed (the rest
contribute exp(-1e6) = 0). The mask is applied as a per-partition bias
on the ScalarE exp that evacuates score PSUM tiles (scores are computed
transposed: [key, query]).

Precision: activations/weights stream in as fp16 (inputs are ~N(0,1), so
fp16's 11-bit mantissa costs ~5e-5 rel per element); all matmuls run
single-pass (fp16 or raw-fp32 "float32r", 1 cycle/row); PSUM accumulates
fp32. Partial outputs return as fp16 and are summed in fp32 on host.

The kernel program is built at call time from the actual valid_lens, so
any input values work; shapes are hardcoded to this problem.
"""
import sys
if "/opt/trn_rl_repo" not in sys.path:
    sys.path.insert(0, "/opt/trn_rl_repo")
import os
import time
import numpy as np

B, SQ, SK, D, H, HD = 2, 2048, 2048, 1024, 16, 64
NEG = -1.0e6
N_CORES = 8
DL = 256          # d_local: 4 heads * 64
KD = D // 128     # contraction tiles over D

_NC_CACHE = {}
last_results = None
last_exec_wall_s = None

# "f16": fp16 streams and fp16 attention core (fast path; FWL weight loads)
# "f32r": all-fp32 storage, single-pass raw-fp32 matmuls
# "f32": exact fp32 (4 cycles/row matmuls)
PREC = os.environ.get("BASS_MHA_PREC", "f16")


def _build(KT, prec=None):
    import concourse.bass as bass  # noqa: F401
    import concourse.tile as tile
    from concourse import bacc, mybir

    prec = PREC if prec is None else prec
    f32 = mybir.dt.float32
    f16 = mybir.dt.float16
    # matmul-operand dtype for the attention core (scores/ctx/out-proj).
    # fp16 (not float32r) so LDWEIGHTS gets the fast-weight-load path on HW.
    md = {"f32": f32, "f32r": mybir.dt.float32r, "f16": f16}[prec]
    # dtype of the streamed activations/weights (and their matmuls)
    xd = f16 if prec == "f16" else md
    # output dtype
    od = f16 if prec == "f16" else f32

    LK = KT * 128
    kchunks = [(i * 512, min(512, LK - i * 512)) for i in range((LK + 511) // 512)]

    nc = bacc.Bacc("TRN2", target_bir_lowering=False, debug=False,
                   num_devices=N_CORES)
    xqT = nc.dram_tensor("xqT", [D, SQ], xd, kind="ExternalInput")
    xkT = nc.dram_tensor("xkT", [D, LK], xd, kind="ExternalInput")
    xvT = nc.dram_tensor("xvT", [D, LK], xd, kind="ExternalInput")
    wq = nc.dram_tensor("wq", [D, DL], xd, kind="ExternalInput")
    wk = nc.dram_tensor("wk", [D, DL], xd, kind="ExternalInput")
    wv = nc.dram_tensor("wv", [D, DL], xd, kind="ExternalInput")
    wo = nc.dram_tensor("wo", [DL, D], md, kind="ExternalInput")
    mask = nc.dram_tensor("mask", [128, KT], f32, kind="ExternalInput")
    out = nc.dram_tensor("out", [SQ, D], od, kind="ExternalOutput")
    dbg = os.environ.get("BASS_MHA_DEBUG") == "1"
    if dbg:
        dbg_qt = nc.dram_tensor("dbg_qt", [128, 2, SQ], md, kind="ExternalOutput")
        dbg_kt = nc.dram_tensor("dbg_kt", [128, 2, LK], md, kind="ExternalOutput")
        dbg_v = nc.dram_tensor("dbg_v", [128, KT, 4, 128], md, kind="ExternalOutput")

    with tile.TileContext(nc) as tc:
        with tc.tile_pool(name="singles", bufs=1) as sg:
            wq_sb = sg.tile([128, KD, DL], xd)
            wk_sb = sg.tile([128, KD, DL], xd)
            wv_sb = sg.tile([128, KD, DL], xd)
            wo_sb = sg.tile([128, DL // 128, D], md)
            mask_sb = sg.tile([128, KT], f32)
            kt_sb = sg.tile([128, 2, LK], md)       # K^T  [d_local, key]
            v_sb = sg.tile([128, KT, 4, 128], md)   # V''  [key, head, d | ones]
            qt_sb = sg.tile([128, 2, SQ], md)       # Q^T  [d_local, query]
            ctxT_sb = sg.tile([128, 2, SQ], md)     # Ctx^T normalized

            # DMA issue order = arrival order: K path, Q path, V path, Wo
            nc.sync.dma_start(out=mask_sb, in_=mask[:, :])
            nc.sync.dma_start(out=wk_sb, in_=wk[:, :].rearrange("(k p) j -> p k j", p=128))
            if md == f16:
                nc.vector.memset(v_sb, 1.0)
            else:
                nc.vector.memset(v_sb.bitcast(f32), 1.0)
            # dummy exp: pulls the ~2.7us activation-table load off phase C's
            # critical path (ACT is otherwise idle until the first softmax)
            warm_sb = sg.tile([1, 1], f32)
            nc.scalar.activation(warm_sb, mask_sb[0:1, 0:1],
                                 mybir.ActivationFunctionType.Exp)

            # ---- resident input streams (DMA priority: xk, xq, xv) ----
            strm_cm = tc.tile_pool(name="streams", bufs=1)
            strm = strm_cm.__enter__()
            xk_full = strm.tile([128, KD, LK], xd)
            xq_full = strm.tile([128, KD, SQ], xd)
            xv_full = strm.tile([128, KD, LK], xd)
            for k in range(KD):
                nc.sync.dma_start(out=xk_full[:, k, :],
                                  in_=xkT[k * 128:(k + 1) * 128, :])

            # ---- Phase A1: K^T = (Wk^T blocks) @ Xk^T, [256, LK] ----
            with tc.tile_pool(name="psA", bufs=1, space="PSUM") as psA:
                accs = {}
                for m in range(2):
                    for ci, (c0, cw) in enumerate(kchunks):
                        accs[(m, ci)] = psA.tile([128, cw], f32,
                                                 tag=f"kt{m}_{ci}", name=f"kt{m}_{ci}")
                for k in range(KD):
                    for m in range(2):
                        for ci, (c0, cw) in enumerate(kchunks):
                            nc.tensor.matmul(accs[(m, ci)],
                                             wk_sb[:, k, m * 128:(m + 1) * 128],
                                             xk_full[:, k, c0:c0 + cw],
                                             start=(k == 0), stop=(k == KD - 1))
                for m in range(2):
                    for ci, (c0, cw) in enumerate(kchunks):
                        nc.vector.tensor_copy(kt_sb[:, m, c0:c0 + cw], accs[(m, ci)])

            # ---- Phases B + A2: Q^T and V projections riding the DMA ----
            # xq streams in right after xk; Q accumulates in narrow passes
            # (2 or 4 PSUM banks) so the V accumulators (1 bank per key
            # tile, bank-aligned: matmul start=True clears a whole bank)
            # fit alongside. Pass 0 rides the xq stream; V rides xv.
            nc.sync.dma_start(out=wq_sb, in_=wq[:, :].rearrange("(k p) j -> p k j", p=128))
            if KT <= 8:
                qpass, nchunk = 2, 2
            else:
                qpass, nchunk = 4, 1
            maxg = 8 - 2 * nchunk
            vgroups = [list(range(g0, min(g0 + maxg, KT)))
                       for g0 in range(0, KT, maxg)]
            with tc.tile_pool(name="psB", bufs=1, space="PSUM") as psB, \
                 tc.tile_pool(name="psV", bufs=1, space="PSUM") as psV:
                for k in range(KD):
                    nc.sync.dma_start(out=xq_full[:, k, :],
                                      in_=xqT[k * 128:(k + 1) * 128, :])
                nc.sync.dma_start(out=wv_sb, in_=wv[:, :].rearrange("(k p) j -> p k j", p=128))
                for k in range(KD):
                    nc.sync.dma_start(out=xv_full[:, k, :],
                                      in_=xvT[k * 128:(k + 1) * 128, :])
                for p in range(qpass):
                    chunks = list(range(p * nchunk, (p + 1) * nchunk))
                    qaccs = {}
                    for m in range(2):
                        for c in chunks:
                            qaccs[(m, c)] = psB.tile([128, 512], f32,
                                                     tag=f"q{m}_{c % nchunk}",
                                                     name=f"qp{p}_{m}_{c}")
                    for k in range(KD):
                        for m in range(2):
                            for c in chunks:
                                nc.tensor.matmul(
                                    qaccs[(m, c)],
                                    wq_sb[:, k, m * 128:(m + 1) * 128],
                                    xq_full[:, k, c * 512:(c + 1) * 512],
                                    start=(k == 0), stop=(k == KD - 1))
                    if p < len(vgroups):
                        g = vgroups[p]
                        vacc = psV.tile([128, len(g), 512], f32, tag="vg",
                                        name=f"vg{p}")
                        for k in range(KD):
                            for vi, t in enumerate(g):
                                nc.tensor.matmul(
                                    vacc[:, vi, 0:DL],
                                    xv_full[:, k, t * 128:(t + 1) * 128],
                                    wv_sb[:, k, :],
                                    start=(k == 0), stop=(k == KD - 1),
                                    skip_group_check=True)
                        for vi, t in enumerate(g):
                            for hh in range(4):
                                nc.vector.tensor_copy(
                                    v_sb[:, t, hh, 0:64],
                                    vacc[:, vi, hh * 64:(hh + 1) * 64])
                    for m in range(2):
                        for c in chunks:
                            nc.vector.tensor_copy(
                                qt_sb[:, m, c * 512:(c + 1) * 512], qaccs[(m, c)])

            nc.sync.dma_start(out=wo_sb, in_=wo[:, :].rearrange("(k p) j -> p k j", p=128))

            # ---- Phase C: per-head attention ----
            # scores^T tile = K^T_h.T @ Q^T_h  -> exp(bias=mask) -> P^T
            # ctx'' = V''.T @ P^T : rows 0-63 ctx, rows 64-127 denominator
            with tc.tile_pool(name="pt", bufs=8) as ptp, \
                 tc.tile_pool(name="misc", bufs=4) as mp, \
                 tc.tile_pool(name="psC", bufs=2, space="PSUM") as psC, \
                 tc.tile_pool(name="psS", bufs=2, space="PSUM") as psS:
                for half in range(2):
                    for hh in range(4):
                        mt, mo = hh // 2, 64 * (hh % 2)
                        h0 = half * 1024
                        ctx_ps = psC.tile([128, 1024], f32, tag="ctx")
                        for t in range(KT):
                            pt_t = ptp.tile([128, 1024], md, tag="pt")
                            s_ps = psS.tile([128, 1024], f32, tag="s")
                            for cq in range(2):
                                nc.tensor.matmul(
                                    s_ps[:, cq * 512:(cq + 1) * 512],
                                    kt_sb[mo:mo + 64, mt, t * 128:(t + 1) * 128],
                                    qt_sb[mo:mo + 64, mt, h0 + cq * 512:h0 + (cq + 1) * 512],
                                    start=True, stop=True)
                            nc.scalar.activation(
                                pt_t, s_ps,
                                mybir.ActivationFunctionType.Exp,
                                bias=mask_sb[:, t:t + 1], scale=0.125)
                            for c in range(2):
                                nc.tensor.matmul(ctx_ps[:, c * 512:(c + 1) * 512],
                                                 v_sb[:, t, hh, :],
                                                 pt_t[:, c * 512:(c + 1) * 512],
                                                 start=(t == 0), stop=(t == KT - 1),
                                                 skip_group_check=True)
                        # rows 64-127 of ctx_ps all hold the softmax denominator
                        rcb = mp.tile([64, 1024], f32, tag="rcb")
                        nc.vector.reciprocal(rcb, ctx_ps[64:128, :])
                        nc.vector.tensor_mul(ctxT_sb[mo:mo + 64, mt, h0:h0 + 1024],
                                             ctx_ps[0:64, :], rcb)

            # ---- Phase D: partial output projection ----
            with tc.tile_pool(name="po", bufs=6) as pop, \
                 tc.tile_pool(name="psD", bufs=4, space="PSUM") as psD:
                for qi in range(SQ // 128):
                    o_ps = psD.tile([128, D], f32, tag="o")
                    for n in range(2):
                        for kk in range(2):
                            nc.tensor.matmul(o_ps[:, n * 512:(n + 1) * 512],
                                             ctxT_sb[:, kk, qi * 128:(qi + 1) * 128],
                                             wo_sb[:, kk, n * 512:(n + 1) * 512],
                                             start=(kk == 0), stop=(kk == 1))
                    o_sb = pop.tile([128, D], od, tag="o_sb")
                    nc.scalar.copy(o_sb[:, 0:512], o_ps[:, 0:512])
                    nc.vector.tensor_copy(o_sb[:, 512:1024], o_ps[:, 512:1024])
                    nc.sync.dma_start(out=out[qi * 128:(qi + 1) * 128, :], in_=o_sb)
            strm_cm.__exit__(None, None, None)
            if dbg:
                nc.sync.dma_start(out=dbg_qt[:, :, :], in_=qt_sb)
                nc.sync.dma_start(out=dbg_kt[:, :, :], in_=kt_sb)
                nc.sync.dma_start(out=dbg_v[:, :, :, :], in_=v_sb)
    nc.compile()
    return nc


def kernel(**inputs):
    global last_results, last_exec_wall_s
    from concourse.bass_utils import run_bass_kernel_spmd

    # BASS_TRACE needs the axon NTFF hook; disable tracing when the hook
    # module is unavailable so a stray env var cannot crash the run.
    if os.environ.get("BASS_TRACE"):
        try:
            from antenv import axon_hooks  # noqa: F401
        except Exception:
            os.environ["BASS_NEVER_TRACE"] = "1"

    q = np.asarray(inputs["queries"], dtype=np.float32)
    kx = np.asarray(inputs["keys"], dtype=np.float32)
    vx = np.asarray(inputs["values"], dtype=np.float32)
    vl = np.asarray(inputs["valid_lens"], dtype=np.int64).reshape(B)
    Wq = np.asarray(inputs["Wq"], dtype=np.float32)
    Wk = np.asarray(inputs["Wk"], dtype=np.float32)
    Wv = np.asarray(inputs["Wv"], dtype=np.float32)
    Wo = np.asarray(inputs["Wo"], dtype=np.float32)
    assert q.shape == (B, SQ, D) and kx.shape == (B, SK, D) and vx.shape == (B, SK, D)

    lens = np.clip(vl, 1, SK)
    lmax = int(lens.max())
    KT = (lmax + 127) // 128
    LK = KT * 128

    key = (KT, PREC)
    if key not in _NC_CACHE:
        _NC_CACHE[key] = _build(KT)
    nc = _NC_CACHE[key]

    xdt = np.float16 if PREC == "f16" else np.float32

    in_maps = []
    for c in range(N_CORES):
        b, hg = c // 4, c % 4
        cols = slice(DL * hg, DL * (hg + 1))
        m = np.where(np.arange(LK) < lens[b], 0.0, NEG).astype(np.float32)
        in_maps.append({
            "xqT": np.ascontiguousarray(q[b].T.astype(xdt)),
            "xkT": np.ascontiguousarray(kx[b, :LK].T.astype(xdt)),
            "xvT": np.ascontiguousarray(vx[b, :LK].T.astype(xdt)),
            "wq": np.ascontiguousarray(Wq[:, cols].astype(xdt)),
            "wk": np.ascontiguousarray(Wk[:, cols].astype(xdt)),
            "wv": np.ascontiguousarray(Wv[:, cols].astype(xdt)),
            "wo": np.ascontiguousarray(Wo[cols, :].astype(xdt)),
            "mask": np.ascontiguousarray(m.reshape(KT, 128).T),
        })

    t0 = time.perf_counter()
    res = run_bass_kernel_spmd(nc, in_maps, core_ids=list(range(N_CORES)))
    last_exec_wall_s = time.perf_counter() - t0
    last_results = res

    outs = [res.results[c]["out"].astype(np.float32) for c in range(N_CORES)]
    full = np.stack([outs[0] + outs[1] + outs[2] + outs[3],
                     outs[4] + outs[5] + outs[6] + outs[7]])
    return full.astype(np.float32)



# revision 44
# speedup vs baseline: 1.0708x; 1.0708x over previous
"""Multi-head attention (B=2, S=2048, D=1024, H=16) on 8 Trainium2 cores.

Sharding: pure tensor-parallel over heads (Megatron): core c owns heads
{2c, 2c+1} (d_local = 128 columns of Wq/Wk/Wv, 128 rows of Wo) and
processes BOTH batches. Each core emits a [2, 2048, 1024] partial output
(row-parallel Wo); the host sums the 8 partials per batch.

Why: the SPMD program's attention work scales with KT0+KT1 (per 2 heads)
instead of 4*max(KT0,KT1) (per 4 heads) under the old batch x head-group
split, so key-length imbalance between the two batches no longer inflates
the program's critical path.

Key-side truncation: only ceil(valid_len/128) key tiles per batch are
computed; the per-batch mask rides the ScalarE exp as a per-partition
bias. Scores are computed transposed ([key, query]); the softmax
denominator comes free via 64 ones-columns appended to V (ones-trick).

Precision: fp16 streams/weights, fp32 PSUM accumulation (rel err ~8e-4).

The program is built at call time from the actual valid_lens (cached by
(KTA, KTB)); batch A is the one with more key tiles and is processed
first so its longer attention phase starts as early as possible.
"""
import sys
if "/opt/trn_rl_repo" not in sys.path:
    sys.path.insert(0, "/opt/trn_rl_repo")
import os
import time
import numpy as np

B, SQ, SK, D, H, HD = 2, 2048, 2048, 1024, 16, 64
NEG = -1.0e6
N_CORES = 8
DL = 128          # d_local: 2 heads * 64
KD = D // 128     # contraction tiles over D

_NC_CACHE = {}
last_results = None
last_exec_wall_s = None


def _build(KTA, KTB):
    import concourse.bass as bass  # noqa: F401
    import concourse.tile as tile
    from concourse import bacc, mybir

    f32 = mybir.dt.float32
    f16 = mybir.dt.float16
    EXP = mybir.ActivationFunctionType.Exp
    RECIP = mybir.ActivationFunctionType.Reciprocal

    LKA, LKB = KTA * 128, KTB * 128
    # [(k0, nk)] chunk groups for the k/v/weight streams (fewer, larger DMAs)
    kgrp = [(0, 4), (4, 4)]

    nc = bacc.Bacc("TRN2", target_bir_lowering=False, debug=False,
                   num_devices=N_CORES)
    xqT = nc.dram_tensor("xqT", [2, D, SQ], f16, kind="ExternalInput")
    xkTA = nc.dram_tensor("xkTA", [D, LKA], f16, kind="ExternalInput")
    xvTA = nc.dram_tensor("xvTA", [D, LKA], f16, kind="ExternalInput")
    xkTB = nc.dram_tensor("xkTB", [D, LKB], f16, kind="ExternalInput")
    xvTB = nc.dram_tensor("xvTB", [D, LKB], f16, kind="ExternalInput")
    wqkv = nc.dram_tensor("wqkv", [D, 3 * DL], f16, kind="ExternalInput")
    wo = nc.dram_tensor("wo", [DL, D], f16, kind="ExternalInput")
    maskA = nc.dram_tensor("maskA", [128, KTA], f32, kind="ExternalInput")
    maskB = nc.dram_tensor("maskB", [128, KTB], f32, kind="ExternalInput")
    out = nc.dram_tensor("out", [2, SQ, D], f16, kind="ExternalOutput")

    with tile.TileContext(nc) as tc:
        with tc.tile_pool(name="singles", bufs=1) as sg:
            wqkv_sb = sg.tile([128, KD, 3 * DL], f16)
            wo_sb = sg.tile([128, D], f16)
            maskA_sb = sg.tile([128, KTA], f32)
            maskB_sb = sg.tile([128, KTB], f32)
            kt_sb = {0: sg.tile([128, LKA], f16, name="ktA"),
                     1: sg.tile([128, LKB], f16, name="ktB")}
            qt_sb = {0: sg.tile([128, SQ], f16, name="qtA"),
                     1: sg.tile([128, SQ], f16, name="qtB")}
            v_sb = {0: sg.tile([128, KTA, 2, 128], f16, name="vA"),
                    1: sg.tile([128, KTB, 2, 128], f16, name="vB")}
            ctx_sb = {0: sg.tile([128, SQ], f16, name="ctxA"),
                      1: sg.tile([128, SQ], f16, name="ctxB")}
            warm_sb = sg.tile([128, 256], f16)

            KT = {0: KTA, 1: KTB}
            mask_sb = {0: maskA_sb, 1: maskB_sb}
            xkT = {0: xkTA, 1: xkTB}
            xvT = {0: xvTA, 1: xvTB}

            # V'' ones-columns (softmax denominator); dim columns are
            # overwritten by the V-projection evacuations below.
            nc.gpsimd.memset(v_sb[0], 1.0)
            nc.gpsimd.memset(v_sb[1], 1.0)
            nc.vector.memset(warm_sb, 0.0)

            # ---- input DMAs, arrival order = need order ----
            nc.sync.dma_start(
                out=wqkv_sb[:, 0:4, :],
                in_=wqkv[0:512, :].rearrange("(k p) j -> p k j", p=128))
            strm_cm = tc.tile_pool(name="streams", bufs=1)
            strm = strm_cm.__enter__()
            xk = {b: strm.tile([128, KD, 128 * KT[b]], f16, name=f"xk{b}")
                  for b in (0, 1)}
            xq = {b: strm.tile([128, KD, SQ], f16, name=f"xq{b}")
                  for b in (0, 1)}
            xv = {b: strm.tile([128, KD, 128 * KT[b]], f16, name=f"xv{b}")
                  for b in (0, 1)}
            def dma_kv(b, which):
                src = xkT[b] if which == "k" else xvT[b]
                dst = xk[b] if which == "k" else xv[b]
                for k0, nk in kgrp:
                    nc.sync.dma_start(
                        out=dst[:, k0:k0 + nk, :],
                        in_=src[k0 * 128:(k0 + nk) * 128, :]
                        .rearrange("(k p) j -> p k j", p=128))

            def dma_q(b):
                for k in range(KD):
                    nc.sync.dma_start(out=xq[b][:, k, :],
                                      in_=xqT[b, k * 128:(k + 1) * 128, :])

            def dma_q_cols(b):
                # column-chunk order: Q-projection pass ci becomes ready
                # as soon as chunk ci lands (contraction needs all k)
                for ci in range(4):
                    nc.sync.dma_start(
                        out=xq[b][:, :, ci * 512:(ci + 1) * 512],
                        in_=xqT[b, :, ci * 512:(ci + 1) * 512]
                        .rearrange("(k p) j -> p k j", p=128))

            dma_kv(0, "k")
            nc.sync.dma_start(
                out=wqkv_sb[:, 4:8, :],
                in_=wqkv[512:1024, :].rearrange("(k p) j -> p k j", p=128))
            nc.sync.dma_start(out=maskA_sb, in_=maskA[:, :])
            nc.sync.dma_start(out=maskB_sb, in_=maskB[:, :])
            dma_kv(0, "v")
            dma_q(0)
            dma_kv(1, "k")
            dma_kv(1, "v")
            dma_q_cols(1)
            nc.sync.dma_start(out=wo_sb, in_=wo[:, :])

            # ---- PE p-state warmup: keep the tensor engine busy during the
            # initial DMA latency so real matmuls start at full clock ----
            with tc.tile_pool(name="psW", bufs=1, space="PSUM") as psW:
                wp = psW.tile([128, 256], f32)
                for _ in range(22):
                    nc.tensor.matmul(wp, warm_sb[:, 0:128], warm_sb,
                                     start=True, stop=True)

            def copy_eng(eng, dst, src):
                if eng == "v":
                    nc.vector.tensor_copy(dst, src)
                elif eng == "a":
                    nc.scalar.copy(dst, src)
                else:
                    nc.gpsimd.tensor_copy(dst, src)

            def proj_k(b, eng):
                # K^T[b] = Wk^T @ Xk^T : [128, LK_b]
                LK = 128 * KT[b]
                chunks = [(i * 512, min(512, LK - i * 512))
                          for i in range((LK + 511) // 512)]
                with tc.tile_pool(name=f"psA{b}", bufs=1, space="PSUM") as ps:
                    accs = [ps.tile([128, cw], f32, tag=f"kt{ci}",
                                    name=f"kt{b}_{ci}")
                            for ci, (c0, cw) in enumerate(chunks)]
                    for k in range(KD):
                        for ci, (c0, cw) in enumerate(chunks):
                            nc.tensor.matmul(accs[ci],
                                             wqkv_sb[:, k, 0:128],
                                             xk[b][:, k, c0:c0 + cw],
                                             start=(k == 0), stop=(k == KD - 1))
                    for ci, (c0, cw) in enumerate(chunks):
                        copy_eng(eng[ci % len(eng)],
                                 kt_sb[b][:, c0:c0 + cw], accs[ci])

            def proj_q(b, eng):
                # Q^T[b] = Wq^T @ Xq^T : [128, 2048]
                with tc.tile_pool(name=f"psB{b}", bufs=1, space="PSUM") as ps:
                    accs = [ps.tile([128, 512], f32, tag=f"q{ci}",
                                    name=f"q{b}_{ci}") for ci in range(4)]
                    for k in range(KD):
                        for ci in range(4):
                            nc.tensor.matmul(accs[ci],
                                             wqkv_sb[:, k, 128:256],
                                             xq[b][:, k, ci * 512:(ci + 1) * 512],
                                             start=(k == 0), stop=(k == KD - 1))
                    for ci in range(4):
                        copy_eng(eng[ci % len(eng)],
                                 qt_sb[b][:, ci * 512:(ci + 1) * 512], accs[ci])

            def proj_v(b, eng):
                # V''[b] : [key, head, dim|ones], groups of <=4 PSUM banks
                for g0 in range(0, KT[b], 4):
                    g = list(range(g0, min(g0 + 4, KT[b])))
                    with tc.tile_pool(name=f"psV{b}{g0}", bufs=1,
                                      space="PSUM") as ps:
                        vacc = ps.tile([128, len(g), 512], f32, tag="vg",
                                       name=f"v{b}_{g0}")
                        for k in range(KD):
                            for vi, t in enumerate(g):
                                nc.tensor.matmul(
                                    vacc[:, vi, 0:DL],
                                    xv[b][:, k, t * 128:(t + 1) * 128],
                                    wqkv_sb[:, k, 256:384],
                                    start=(k == 0), stop=(k == KD - 1),
                                    skip_group_check=True)
                        for vi, t in enumerate(g):
                            for hh in range(2):
                                copy_eng(eng[(2 * vi + hh) % len(eng)],
                                         v_sb[b][:, t, hh, 0:64],
                                         vacc[:, vi, hh * 64:(hh + 1) * 64])

            def attend(b, pools, extra=None):
                # scores^T -> exp -> ctx'' per (head, query-1024-chunk,
                # key-tile); ctx accumulates in a [128,1024] 2-bank tile,
                # normalized per chunk. extra(i) interleaves foreign work.
                ptp, mp, psS, psC = pools
                nit = 0
                for hh, cq in ((0, 0), (1, 0), (0, 1), (1, 1)):
                    if True:
                        q0 = cq * 1024
                        ctx_ps = psC.tile([128, 1024], f32, tag="ctx",
                                          name=f"ctx{b}_{hh}_{cq}")
                        for t in range(KT[b]):
                            s_ps = psS.tile([128, 1024], f32, tag="s")
                            for c in range(2):
                                nc.tensor.matmul(
                                    s_ps[:, c * 512:(c + 1) * 512],
                                    kt_sb[b][hh * 64:hh * 64 + 64,
                                             t * 128:(t + 1) * 128],
                                    qt_sb[b][hh * 64:hh * 64 + 64,
                                             q0 + c * 512:q0 + (c + 1) * 512],
                                    start=True, stop=True)
                            pt = ptp.tile([128, 1024], f16, tag="pt")
                            nc.scalar.activation(
                                pt, s_ps, EXP,
                                bias=mask_sb[b][:, t:t + 1], scale=0.125)
                            for c in range(2):
                                nc.tensor.matmul(
                                    ctx_ps[:, c * 512:(c + 1) * 512],
                                    v_sb[b][:, t, hh, :],
                                    pt[:, c * 512:(c + 1) * 512],
                                    start=(t == 0), stop=(t == KT[b] - 1),
                                    skip_group_check=True)
                            if extra is not None:
                                extra(nit)
                            nit += 1
                        # rows 64-127 all hold the softmax denominator:
                        # one fused divide normalizes and evacuates
                        nc.vector.tensor_tensor(
                            ctx_sb[b][hh * 64:hh * 64 + 64, q0:q0 + 1024],
                            ctx_ps[0:64, :], ctx_ps[64:128, :],
                            op=mybir.AluOpType.divide)

            evac_ct = [0]
            osb_map = {0: {}, 1: {}}

            def out_proj(b, pools, qr, engines):
                # partial out[b] rows = ctx''[b]^T @ Wo_local; [128,512]
                # PSUM grain; o_sb tiles live per query-tile PAIR (the DMA
                # after the odd qi reads both halves)
                psD, op = pools
                for qi in qr:
                    pb = qi // 2
                    if pb not in osb_map[b]:
                        osb_map[b][pb] = op.tile(
                            [128, 2, D], f16, tag=f"o{pb % 2}",
                            name=f"osb{b}_{pb}")
                    o_sb = osb_map[b][pb]
                    for n in range(2):
                        o_ps = psD.tile([128, 512], f32, tag="x",
                                        name=f"o{b}_{qi}_{n}")
                        nc.tensor.matmul(o_ps,
                                         ctx_sb[b][:, qi * 128:(qi + 1) * 128],
                                         wo_sb[:, n * 512:(n + 1) * 512],
                                         start=True, stop=True)
                        eng = engines[evac_ct[0] % len(engines)]
                        evac_ct[0] += 1
                        copy_eng(eng, o_sb[:, qi % 2, n * 512:(n + 1) * 512],
                                 o_ps)
                    if qi % 2 == 1:
                        nc.sync.dma_start(
                            out=out[b, (qi - 1) * 128:(qi + 1) * 128, :]
                            .rearrange("(c p) j -> p c j", p=128),
                            in_=o_sb)

            # ---- batch A: projections ride the streams, then attention.
            # V before Q: V rides the earlier xv stream in otherwise-idle
            # PE time and frees its PSUM banks before attention opens ----
            proj_k(0, ("v",))
            proj_v(0, ("v",))
            proj_q(0, ("a", "v"))

            # batch-B projections as single-PSUM-bank steps, interleaved
            # into batch-A's ACT-bound attention cadence (all on GPSIMD so
            # nothing queues behind DVE norms)
            def bsteps(psX):
                LKB_ = 128 * KT[1]

                def a1b_step(c0, cw):
                    acc = psX.tile([128, 512], f32, tag="x", name="xa")
                    for k in range(KD):
                        nc.tensor.matmul(acc[:, 0:cw], wqkv_sb[:, k, 0:128],
                                         xk[1][:, k, c0:c0 + cw],
                                         start=(k == 0), stop=(k == KD - 1))
                    copy_eng("v", kt_sb[1][:, c0:c0 + cw], acc[:, 0:cw])

                def vb_step(t):
                    acc = psX.tile([128, 512], f32, tag="x", name="xv")
                    for k in range(KD):
                        nc.tensor.matmul(acc[:, 0:DL],
                                         xv[1][:, k, t * 128:(t + 1) * 128],
                                         wqkv_sb[:, k, 256:384],
                                         start=(k == 0), stop=(k == KD - 1))
                    for hh in range(2):
                        copy_eng("v", v_sb[1][:, t, hh, 0:64],
                                 acc[:, hh * 64:(hh + 1) * 64])

                def qb_step(ci):
                    acc = psX.tile([128, 512], f32, tag="x", name="xq")
                    for k in range(KD):
                        nc.tensor.matmul(acc, wqkv_sb[:, k, 128:256],
                                         xq[1][:, k, ci * 512:(ci + 1) * 512],
                                         start=(k == 0), stop=(k == KD - 1))
                    copy_eng("v", qt_sb[1][:, ci * 512:(ci + 1) * 512], acc)

                steps = []
                for i in range((LKB_ + 511) // 512):
                    c0 = i * 512
                    steps.append(lambda c0=c0, cw=min(512, LKB_ - c0):
                                 a1b_step(c0, cw))
                steps.extend(lambda t=t: vb_step(t) for t in range(KT[1]))
                steps.extend(lambda ci=ci: qb_step(ci) for ci in range(4))
                return steps

            # One continuous PSUM configuration from first attention to last
            # output tile: psS (2 banks, score rotation) + psC (4 banks, ctx
            # accumulator) + aux (2 banks, shared rotation for batch-B
            # projection steps, then both batches' out-projection PSUM).
            with tc.tile_pool(name="pt", bufs=6) as ptp, \
                 tc.tile_pool(name="misc", bufs=2) as mp, \
                 tc.tile_pool(name="ob", bufs=4) as op:
                with tc.tile_pool(name="psS", bufs=2, space="PSUM") as psS, \
                     tc.tile_pool(name="psC", bufs=1, space="PSUM") as psC, \
                     tc.tile_pool(name="aux", bufs=2, space="PSUM") as aux:
                    steps = bsteps(aux)
                    # batch-B projection steps ride attend(0)'s first half
                    # (group order (h0,cq0),(h1,cq0) normalizes query half 0
                    # of both heads) ... then batch A's out-projection qi 0-7
                    # rides the second half, with out-DMAs filling the DMA
                    # lull between the input and output streams.
                    nit_A = 4 * KT[0]
                    half_A = 2 * KT[0]
                    smap = {}
                    if half_A > 2:
                        for j in range(len(steps)):
                            it = 1 + j * (half_A - 1) // len(steps)
                            smap.setdefault(it, []).append(j)
                    qmapA = {}
                    if nit_A - half_A > 1:
                        for qi in range(8):
                            it = half_A + qi * (nit_A - half_A - 1) // 8
                            qmapA.setdefault(it, []).append(qi)

                    def extraA(i):
                        for j in smap.get(i, ()):
                            steps[j]()
                        if i in qmapA:
                            out_proj(0, (aux, op), qmapA[i], ("v", "v", "a"))

                    attend(0, (ptp, mp, psS, psC), extra=extraA)
                    if half_A <= 2:
                        for s in steps:
                            s()
                    done_A = sorted(q for qs in qmapA.values() for q in qs)
                    rest_A = [q for q in range(16) if q not in done_A]
                    # batch B attention carries batch A's remaining
                    # out-projection tiles
                    nb = 4 * KT[1]
                    qsched = {}
                    if nb >= 2:
                        ns = nb - 1
                        nr = len(rest_A)
                        for i in range(ns):
                            qsched[i] = (rest_A[nr * i // ns:
                                                nr * (i + 1) // ns],
                                         ("v", "a"))
                    else:
                        qsched[0] = (rest_A, ("v", "a"))

                    def extra(i):
                        if i in qsched:
                            qr, eng = qsched[i]
                            out_proj(0, (aux, op), qr, eng)

                    attend(1, (ptp, mp, psS, psC), extra=extra)
                # final out-projection in its own deep PSUM rotation so the
                # tail runs at the out-DMA rate, not the evacuation rate
                with tc.tile_pool(name="psD2", bufs=6, space="PSUM") as psD2:
                    out_proj(1, (psD2, op), range(16), ("a", "v"))
            strm_cm.__exit__(None, None, None)
    nc.compile()
    return nc


def kernel(**inputs):
    global last_results, last_exec_wall_s
    from concourse.bass_utils import run_bass_kernel_spmd

    # BASS_TRACE needs the axon NTFF hook; disable tracing when the hook
    # module is unavailable so a stray env var cannot crash the run.
    if os.environ.get("BASS_TRACE"):
        try:
            from antenv import axon_hooks  # noqa: F401
        except Exception:
            os.environ["BASS_NEVER_TRACE"] = "1"

    q = np.asarray(inputs["queries"], dtype=np.float32)
    kx = np.asarray(inputs["keys"], dtype=np.float32)
    vx = np.asarray(inputs["values"], dtype=np.float32)
    vl = np.asarray(inputs["valid_lens"], dtype=np.int64).reshape(B)
    Wq = np.asarray(inputs["Wq"], dtype=np.float32)
    Wk = np.asarray(inputs["Wk"], dtype=np.float32)
    Wv = np.asarray(inputs["Wv"], dtype=np.float32)
    Wo = np.asarray(inputs["Wo"], dtype=np.float32)
    assert q.shape == (B, SQ, D) and kx.shape == (B, SK, D) and vx.shape == (B, SK, D)

    lens = np.clip(vl, 1, SK)
    KTs = [(int(l) + 127) // 128 for l in lens]
    # batch A = more key tiles, processed first
    bA = 0 if KTs[0] >= KTs[1] else 1
    bB = 1 - bA
    KTA, KTB = KTs[bA], KTs[bB]
    LKA, LKB = KTA * 128, KTB * 128

    key = (KTA, KTB)
    if key not in _NC_CACHE:
        _NC_CACHE[key] = _build(KTA, KTB)
    nc = _NC_CACHE[key]

    def m128(b, KT):
        m = np.where(np.arange(KT * 128) < lens[b], 0.0, NEG).astype(np.float32)
        return np.ascontiguousarray(m.reshape(KT, 128).T)

    xqT_full = np.ascontiguousarray(
        np.stack([q[bA].T, q[bB].T]).astype(np.float16))
    in_maps = []
    for c in range(N_CORES):
        cols = slice(DL * c, DL * (c + 1))
        in_maps.append({
            "xqT": xqT_full,
            "xkTA": np.ascontiguousarray(kx[bA, :LKA].T.astype(np.float16)),
            "xvTA": np.ascontiguousarray(vx[bA, :LKA].T.astype(np.float16)),
            "xkTB": np.ascontiguousarray(kx[bB, :LKB].T.astype(np.float16)),
            "xvTB": np.ascontiguousarray(vx[bB, :LKB].T.astype(np.float16)),
            "wqkv": np.ascontiguousarray(np.concatenate(
                [Wk[:, cols], Wq[:, cols], Wv[:, cols]],
                axis=1).astype(np.float16)),
            "wo": np.ascontiguousarray(Wo[cols, :].astype(np.float16)),
            "maskA": m128(bA, KTA),
            "maskB": m128(bB, KTB),
        })

    t0 = time.perf_counter()
    res = run_bass_kernel_spmd(nc, in_maps, core_ids=list(range(N_CORES)))
    last_exec_wall_s = time.perf_counter() - t0
    last_results = res

    outs = [res.results[c]["out"].astype(np.float32) for c in range(N_CORES)]
    acc = outs[0]
    for c in range(1, N_CORES):
        acc = acc + outs[c]
    full = np.empty((B, SQ, D), dtype=np.float32)
    full[bA] = acc[0]
    full[bB] = acc[1]
    return full


# revision 66
# speedup vs baseline: 1.1181x; 1.0441x over previous
"""Multi-head attention (B=2, S=2048, D=1024, H=16) on 8 Trainium2 cores.

Sharding: pure tensor-parallel over heads (Megatron): core c owns heads
{2c, 2c+1} (d_local = 128 columns of Wq/Wk/Wv, 128 rows of Wo) and
processes BOTH batches. Each core emits a [2, 2048, 1024] partial output
(row-parallel Wo); the host sums the 8 partials per batch.

Why: the SPMD program's attention work scales with KT0+KT1 (per 2 heads)
instead of 4*max(KT0,KT1) (per 4 heads) under the old batch x head-group
split, so key-length imbalance between the two batches no longer inflates
the program's critical path.

Key-side truncation: only ceil(valid_len/128) key tiles per batch are
computed; the per-batch mask rides the ScalarE exp as a per-partition
bias. Scores are computed transposed ([key, query]); the softmax
denominator comes free via 64 ones-columns appended to V (ones-trick).

Precision: fp16 streams/weights, fp32 PSUM accumulation (rel err ~8e-4).

The program is built at call time from the actual valid_lens (cached by
(KTA, KTB)); batch A is the one with more key tiles and is processed
first so its longer attention phase starts as early as possible.
"""
import sys
if "/opt/trn_rl_repo" not in sys.path:
    sys.path.insert(0, "/opt/trn_rl_repo")
import os
import time
import numpy as np

B, SQ, SK, D, H, HD = 2, 2048, 2048, 1024, 16, 64
NEG = -1.0e6
N_CORES = 8
DL = 128          # d_local: 2 heads * 64
KD = D // 128     # contraction tiles over D

_NC_CACHE = {}
last_results = None
last_exec_wall_s = None


def _build(KTA, KTB):
    import concourse.bass as bass  # noqa: F401
    import concourse.tile as tile
    from concourse import bacc, mybir

    f32 = mybir.dt.float32
    f16 = mybir.dt.float16
    EXP = mybir.ActivationFunctionType.Exp
    RECIP = mybir.ActivationFunctionType.Reciprocal

    LKA, LKB = KTA * 128, KTB * 128
    # [(k0, nk)] chunk groups for the k/v/weight streams (fewer, larger DMAs)
    kgrp = [(0, 4), (4, 4)]

    nc = bacc.Bacc("TRN2", target_bir_lowering=False, debug=False,
                   num_devices=N_CORES)
    xqT = nc.dram_tensor("xqT", [2, D, SQ], f16, kind="ExternalInput")
    xkTA = nc.dram_tensor("xkTA", [D, LKA], f16, kind="ExternalInput")
    xvTA = nc.dram_tensor("xvTA", [D, LKA], f16, kind="ExternalInput")
    xkTB = nc.dram_tensor("xkTB", [D, LKB], f16, kind="ExternalInput")
    xvTB = nc.dram_tensor("xvTB", [D, LKB], f16, kind="ExternalInput")
    wqkv = nc.dram_tensor("wqkv", [D, 3 * DL], f16, kind="ExternalInput")
    wo = nc.dram_tensor("wo", [DL, D], f16, kind="ExternalInput")
    maskA = nc.dram_tensor("maskA", [128, KTA], f32, kind="ExternalInput")
    maskB = nc.dram_tensor("maskB", [128, KTB], f32, kind="ExternalInput")
    out = nc.dram_tensor("out", [2, SQ, D], f16, kind="ExternalOutput")

    with tile.TileContext(nc) as tc:
        with tc.tile_pool(name="singles", bufs=1) as sg:
            wqkv_sb = sg.tile([128, KD, 3 * DL], f16)
            wo_sb = sg.tile([128, D], f16)
            maskA_sb = sg.tile([128, KTA], f32)
            maskB_sb = sg.tile([128, KTB], f32)
            kt_sb = {0: sg.tile([128, LKA], f16, name="ktA"),
                     1: sg.tile([128, LKB], f16, name="ktB")}
            qt_sb = {0: sg.tile([128, SQ], f16, name="qtA"),
                     1: sg.tile([128, SQ], f16, name="qtB")}
            v_sb = {0: sg.tile([128, KTA, 2, 128], f16, name="vA"),
                    1: sg.tile([128, KTB, 2, 128], f16, name="vB")}
            ctx_sb = {0: sg.tile([128, SQ], f16, name="ctxA"),
                      1: sg.tile([128, SQ], f16, name="ctxB")}
            warm_sb = sg.tile([128, 256], f16)

            KT = {0: KTA, 1: KTB}
            mask_sb = {0: maskA_sb, 1: maskB_sb}
            xkT = {0: xkTA, 1: xkTB}
            xvT = {0: xvTA, 1: xvTB}

            # V'' ones-columns (softmax denominator); dim columns are
            # overwritten by the V-projection evacuations below.
            nc.gpsimd.memset(v_sb[0], 1.0)
            nc.gpsimd.memset(v_sb[1], 1.0)
            nc.vector.memset(warm_sb, 0.0)

            # ---- input DMAs, arrival order = need order ----
            nc.sync.dma_start(
                out=wqkv_sb[:, 0:4, :],
                in_=wqkv[0:512, :].rearrange("(k p) j -> p k j", p=128))
            strm_cm = tc.tile_pool(name="streams", bufs=1)
            strm = strm_cm.__enter__()
            xk = {b: strm.tile([128, KD, 128 * KT[b]], f16, name=f"xk{b}")
                  for b in (0, 1)}
            xq = {b: strm.tile([128, KD, SQ], f16, name=f"xq{b}")
                  for b in (0, 1)}
            xv = {b: strm.tile([128, KD, 128 * KT[b]], f16, name=f"xv{b}")
                  for b in (0, 1)}
            def dma_kv(b, which):
                src = xkT[b] if which == "k" else xvT[b]
                dst = xk[b] if which == "k" else xv[b]
                for k0, nk in kgrp:
                    nc.sync.dma_start(
                        out=dst[:, k0:k0 + nk, :],
                        in_=src[k0 * 128:(k0 + nk) * 128, :]
                        .rearrange("(k p) j -> p k j", p=128))

            def dma_q(b):
                for k in range(KD):
                    nc.sync.dma_start(out=xq[b][:, k, :],
                                      in_=xqT[b, k * 128:(k + 1) * 128, :])

            def dma_q_cols(b):
                # column-chunk order: Q-projection pass ci becomes ready
                # as soon as chunk ci lands (contraction needs all k)
                for ci in range(4):
                    nc.sync.dma_start(
                        out=xq[b][:, :, ci * 512:(ci + 1) * 512],
                        in_=xqT[b, :, ci * 512:(ci + 1) * 512]
                        .rearrange("(k p) j -> p k j", p=128))

            dma_kv(0, "k")
            nc.sync.dma_start(
                out=wqkv_sb[:, 4:8, :],
                in_=wqkv[512:1024, :].rearrange("(k p) j -> p k j", p=128))
            nc.sync.dma_start(out=maskA_sb, in_=maskA[:, :])
            nc.sync.dma_start(out=maskB_sb, in_=maskB[:, :])
            dma_kv(0, "v")   # xv before xq: ctx never stalls on V''
            dma_q_cols(0)
            dma_kv(1, "k")
            dma_kv(1, "v")
            dma_q_cols(1)
            nc.sync.dma_start(out=wo_sb, in_=wo[:, :])

            # ---- PE p-state warmup: keep the tensor engine busy during the
            # initial DMA latency so real matmuls start at full clock; the
            # pool stays open so stream-gated phases can emit filler too ----
            psW_cm = tc.tile_pool(name="psW", bufs=1, space="PSUM")
            psW = psW_cm.__enter__()
            wp = psW.tile([128, 256], f32)

            def wfill(n):
                for _ in range(n):
                    nc.tensor.matmul(wp, warm_sb[:, 0:128], warm_sb,
                                     start=True, stop=True)

            wfill(22)

            def copy_eng(eng, dst, src):
                if eng == "v":
                    nc.vector.tensor_copy(dst, src)
                elif eng == "a":
                    nc.scalar.copy(dst, src)
                else:
                    nc.gpsimd.tensor_copy(dst, src)

            def proj_k(b, eng):
                # K^T[b] = Wk^T @ Xk^T : [128, LK_b]
                LK = 128 * KT[b]
                chunks = [(i * 512, min(512, LK - i * 512))
                          for i in range((LK + 511) // 512)]
                with tc.tile_pool(name=f"psA{b}", bufs=1, space="PSUM") as ps:
                    accs = [ps.tile([128, cw], f32, tag=f"kt{ci}",
                                    name=f"kt{b}_{ci}")
                            for ci, (c0, cw) in enumerate(chunks)]
                    for k in range(KD):
                        for ci, (c0, cw) in enumerate(chunks):
                            nc.tensor.matmul(accs[ci],
                                             wqkv_sb[:, k, 0:128],
                                             xk[b][:, k, c0:c0 + cw],
                                             start=(k == 0), stop=(k == KD - 1))
                    for ci, (c0, cw) in enumerate(chunks):
                        copy_eng(eng[ci % len(eng)],
                                 kt_sb[b][:, c0:c0 + cw], accs[ci])

            def proj_q(b, eng, wfill=None):
                # Q^T[b] = Wq^T @ Xq^T : [128, 2048]. wfill emits idle
                # matmuls between DMA-gated k-chunks to hold the PE p-state.
                with tc.tile_pool(name=f"psB{b}", bufs=1, space="PSUM") as ps:
                    accs = [ps.tile([128, 512], f32, tag=f"q{ci}",
                                    name=f"q{b}_{ci}") for ci in range(4)]
                    for k in range(KD):
                        for ci in range(4):
                            nc.tensor.matmul(accs[ci],
                                             wqkv_sb[:, k, 128:256],
                                             xq[b][:, k, ci * 512:(ci + 1) * 512],
                                             start=(k == 0), stop=(k == KD - 1))
                        if wfill is not None and k < KD - 1:
                            wfill(2)
                    for ci in range(4):
                        copy_eng(eng[ci % len(eng)],
                                 qt_sb[b][:, ci * 512:(ci + 1) * 512], accs[ci])

            def proj_v(b, eng):
                # V''[b] : [key, head, dim|ones]; one pool, per-group tags,
                # so a later group never waits an earlier group's evacs
                gs = [list(range(g0, min(g0 + 4, KT[b])))
                      for g0 in range(0, KT[b], 4)]
                if KT[b] > 7:   # bank budget: fall back to serial groups
                    gs = [[t] for t in range(KT[b])]
                with tc.tile_pool(name=f"psV{b}", bufs=1, space="PSUM") as ps:
                    for gi, g in enumerate(gs):
                        tag = f"vg{gi % 4}" if KT[b] > 7 else f"vg{gi}"
                        vacc = ps.tile([128, len(g), 512], f32, tag=tag,
                                       name=f"v{b}_{gi}")
                        for k in range(KD):
                            for vi, t in enumerate(g):
                                nc.tensor.matmul(
                                    vacc[:, vi, 0:DL],
                                    xv[b][:, k, t * 128:(t + 1) * 128],
                                    wqkv_sb[:, k, 256:384],
                                    start=(k == 0), stop=(k == KD - 1),
                                    skip_group_check=True)
                        for vi, t in enumerate(g):
                            for hh in range(2):
                                copy_eng(eng[(2 * vi + hh) % len(eng)],
                                         v_sb[b][:, t, hh, 0:64],
                                         vacc[:, vi, hh * 64:(hh + 1) * 64])

            def attend(b, pools, extra=None):
                # scores^T -> exp -> ctx'' per (head, 512-query-chunk,
                # key-tile), chunk-major so chunk c needs only Q column
                # pass c; ctx accumulates in a [128,512] 1-bank tile,
                # normalized per chunk. extra(i) interleaves foreign work.
                ptp, mp, psS, psC = pools
                nit = 0
                for cq in range(4):
                    for hh in range(2):
                        q0 = cq * 512
                        ctx_ps = psC.tile([128, 512], f32, tag="ctx",
                                          name=f"ctx{b}_{hh}_{cq}")
                        for t in range(KT[b]):
                            s_ps = psS.tile([128, 512], f32, tag="s")
                            nc.tensor.matmul(
                                s_ps,
                                kt_sb[b][hh * 64:hh * 64 + 64,
                                         t * 128:(t + 1) * 128],
                                qt_sb[b][hh * 64:hh * 64 + 64,
                                         q0:q0 + 512],
                                start=True, stop=True)
                            pt = ptp.tile([128, 512], f16, tag="pt")
                            nc.scalar.activation(
                                pt, s_ps, EXP,
                                bias=mask_sb[b][:, t:t + 1], scale=0.125)
                            nc.tensor.matmul(
                                ctx_ps, v_sb[b][:, t, hh, :], pt,
                                start=(t == 0), stop=(t == KT[b] - 1),
                                skip_group_check=True)
                            if extra is not None:
                                extra(nit)
                            nit += 1
                        # rows 64-127 all hold the softmax denominator
                        # (only one TensorTensor input may come from PSUM,
                        # so reciprocal lands in SBUF first)
                        rcb = mp.tile([64, 512], f32, tag="rcb")
                        nc.vector.reciprocal(rcb, ctx_ps[64:128, :])
                        nc.vector.tensor_mul(
                            ctx_sb[b][hh * 64:hh * 64 + 64, q0:q0 + 512],
                            ctx_ps[0:64, :], rcb)

            evac_ct = [0]
            osb_map = {0: {}, 1: {}}

            def out_proj(b, pools, qr, engines):
                # partial out[b] rows = ctx''[b]^T @ Wo_local; [128,512]
                # PSUM grain; o_sb tiles live per query-tile PAIR (the DMA
                # after the odd qi reads both halves)
                psD, op = pools
                for qi in qr:
                    pb = qi // 2
                    if pb not in osb_map[b]:
                        osb_map[b][pb] = op.tile(
                            [128, 2, D], f16, tag=f"o{pb % 2}",
                            name=f"osb{b}_{pb}")
                    o_sb = osb_map[b][pb]
                    for n in range(2):
                        o_ps = psD.tile([128, 512], f32, tag="x",
                                        name=f"o{b}_{qi}_{n}")
                        nc.tensor.matmul(o_ps,
                                         ctx_sb[b][:, qi * 128:(qi + 1) * 128],
                                         wo_sb[:, n * 512:(n + 1) * 512],
                                         start=True, stop=True)
                        eng = engines[evac_ct[0] % len(engines)]
                        evac_ct[0] += 1
                        copy_eng(eng, o_sb[:, qi % 2, n * 512:(n + 1) * 512],
                                 o_ps)
                    if qi % 2 == 1:
                        nc.sync.dma_start(
                            out=out[b, (qi - 1) * 128:(qi + 1) * 128, :]
                            .rearrange("(c p) j -> p c j", p=128),
                            in_=o_sb)

            # ---- batch A K/V projections ride the early streams; Q runs
            # as column passes so attention starts while Q still streams ----
            proj_k(0, ("v",))
            proj_v(0, ("v",))
            psW_cm.__exit__(None, None, None)

            # batch-B projections as single-PSUM-bank steps, interleaved
            # into batch-A's ACT-bound attention cadence (all on GPSIMD so
            # nothing queues behind DVE norms)
            def bsteps(psX):
                LKB_ = 128 * KT[1]

                def a1b_step(c0, cw):
                    acc = psX.tile([128, 512], f32, tag="x", name="xa")
                    for k in range(KD):
                        nc.tensor.matmul(acc[:, 0:cw], wqkv_sb[:, k, 0:128],
                                         xk[1][:, k, c0:c0 + cw],
                                         start=(k == 0), stop=(k == KD - 1))
                    copy_eng("v", kt_sb[1][:, c0:c0 + cw], acc[:, 0:cw])

                def vb_step(t):
                    acc = psX.tile([128, 512], f32, tag="x", name="xv")
                    for k in range(KD):
                        nc.tensor.matmul(acc[:, 0:DL],
                                         xv[1][:, k, t * 128:(t + 1) * 128],
                                         wqkv_sb[:, k, 256:384],
                                         start=(k == 0), stop=(k == KD - 1))
                    for hh in range(2):
                        copy_eng("v", v_sb[1][:, t, hh, 0:64],
                                 acc[:, hh * 64:(hh + 1) * 64])

                def qb_step(ci, b=1):
                    acc = psX.tile([128, 512], f32, tag="x", name="xq")
                    for k in range(KD):
                        nc.tensor.matmul(acc, wqkv_sb[:, k, 128:256],
                                         xq[b][:, k, ci * 512:(ci + 1) * 512],
                                         start=(k == 0), stop=(k == KD - 1))
                    copy_eng("v", qt_sb[b][:, ci * 512:(ci + 1) * 512], acc)

                steps = []
                for i in range((LKB_ + 511) // 512):
                    c0 = i * 512
                    steps.append(lambda c0=c0, cw=min(512, LKB_ - c0):
                                 a1b_step(c0, cw))
                steps.extend(lambda t=t: vb_step(t) for t in range(KT[1]))
                steps.extend(lambda ci=ci: qb_step(ci) for ci in range(4))
                return steps, qb_step

            # One continuous PSUM configuration from first attention to last
            # output tile: psS (2 banks, score rotation) + psC (4 banks, ctx
            # accumulator) + aux (2 banks, shared rotation for batch-B
            # projection steps, then both batches' out-projection PSUM).
            with tc.tile_pool(name="pt", bufs=6) as ptp, \
                 tc.tile_pool(name="misc", bufs=2) as mp, \
                 tc.tile_pool(name="ob", bufs=6) as op:
                with tc.tile_pool(name="psS", bufs=3, space="PSUM") as psS, \
                     tc.tile_pool(name="psC", bufs=2, space="PSUM") as psC, \
                     tc.tile_pool(name="aux", bufs=2, space="PSUM") as aux:
                    steps, qa_step = bsteps(aux)
                    # Batch A's Q column pass 0 gates the first group;
                    # passes 1-3 interleave ahead of the chunks that need
                    # them, tracking the xq column-DMA arrivals. Batch-B
                    # projection steps ride attend(0)'s second half.
                    qa_step(0, b=0)
                    nit_A = 8 * KT[0]
                    smap = {}
                    for i, frac in ((1, 0.15), (2, 0.375), (3, 0.55)):
                        it = max(i, int(nit_A * frac))
                        smap.setdefault(it, []).append(
                            lambda ci=i: qa_step(ci, b=0))
                    for j, s in enumerate(steps):
                        it = max(4, int(nit_A * (0.62 + 0.33 * j / len(steps))))
                        smap.setdefault(it, []).append(s)

                    # a small slice of batch A's out-projection (qi 0-3,
                    # query chunk 0, normalized after group 4) rides the tail
                    # of attend(0) so its out-DMAs start during the DMA lull
                    qmapA = {}

                    def extraA(i):
                        for s in smap.get(i, ()):
                            s()
                        if i in qmapA:
                            out_proj(0, (aux, op), qmapA[i], ("a", "v"))

                    attend(0, (ptp, mp, psS, psC), extra=extraA)
                    for it, fns in sorted(smap.items()):
                        if it >= nit_A:
                            for s in fns:
                                s()
                    done_A = sorted(q for qs in qmapA.values() for q in qs)
                    rest_A = [q for q in range(16) if q not in done_A]
                    # batch B attention carries the rest of batch A's
                    # out-projection, spread across its cadence
                    nb = 8 * KT[1]
                    qsched = {}
                    ns = max(1, nb - 1)
                    nr = len(rest_A)
                    for i in range(ns):
                        qsched[i] = (rest_A[nr * i // ns: nr * (i + 1) // ns],
                                     ("v", "v", "a"))

                    def extra(i):
                        if i in qsched:
                            qr, eng = qsched[i]
                            out_proj(0, (aux, op), qr, eng)

                    attend(1, (ptp, mp, psS, psC), extra=extra)
                # final out-projection in its own deep PSUM rotation so the
                # tail runs at the out-DMA rate, not the evacuation rate
                with tc.tile_pool(name="psD2", bufs=6, space="PSUM") as psD2:
                    out_proj(1, (psD2, op), range(16), ("a", "v"))
            strm_cm.__exit__(None, None, None)
    nc.compile()
    return nc


def kernel(**inputs):
    global last_results, last_exec_wall_s
    from concourse.bass_utils import run_bass_kernel_spmd

    # BASS_TRACE needs the axon NTFF hook; disable tracing when the hook
    # module is unavailable so a stray env var cannot crash the run.
    if os.environ.get("BASS_TRACE"):
        try:
            from antenv import axon_hooks  # noqa: F401
        except Exception:
            os.environ["BASS_NEVER_TRACE"] = "1"

    q = np.asarray(inputs["queries"], dtype=np.float32)
    kx = np.asarray(inputs["keys"], dtype=np.float32)
    vx = np.asarray(inputs["values"], dtype=np.float32)
    vl = np.asarray(inputs["valid_lens"], dtype=np.int64).reshape(B)
    Wq = np.asarray(inputs["Wq"], dtype=np.float32)
    Wk = np.asarray(inputs["Wk"], dtype=np.float32)
    Wv = np.asarray(inputs["Wv"], dtype=np.float32)
    Wo = np.asarray(inputs["Wo"], dtype=np.float32)
    assert q.shape == (B, SQ, D) and kx.shape == (B, SK, D) and vx.shape == (B, SK, D)

    lens = np.clip(vl, 1, SK)
    KTs = [(int(l) + 127) // 128 for l in lens]
    # batch A = more key tiles, processed first
    bA = 0 if KTs[0] >= KTs[1] else 1
    bB = 1 - bA
    KTA, KTB = KTs[bA], KTs[bB]
    LKA, LKB = KTA * 128, KTB * 128

    key = (KTA, KTB)
    if key not in _NC_CACHE:
        _NC_CACHE[key] = _build(KTA, KTB)
    nc = _NC_CACHE[key]

    def m128(b, KT):
        m = np.where(np.arange(KT * 128) < lens[b], 0.0, NEG).astype(np.float32)
        return np.ascontiguousarray(m.reshape(KT, 128).T)

    xqT_full = np.ascontiguousarray(
        np.stack([q[bA].T, q[bB].T]).astype(np.float16))
    in_maps = []
    for c in range(N_CORES):
        cols = slice(DL * c, DL * (c + 1))
        in_maps.append({
            "xqT": xqT_full,
            "xkTA": np.ascontiguousarray(kx[bA, :LKA].T.astype(np.float16)),
            "xvTA": np.ascontiguousarray(vx[bA, :LKA].T.astype(np.float16)),
            "xkTB": np.ascontiguousarray(kx[bB, :LKB].T.astype(np.float16)),
            "xvTB": np.ascontiguousarray(vx[bB, :LKB].T.astype(np.float16)),
            "wqkv": np.ascontiguousarray(np.concatenate(
                [Wk[:, cols], Wq[:, cols], Wv[:, cols]],
                axis=1).astype(np.float16)),
            "wo": np.ascontiguousarray(Wo[cols, :].astype(np.float16)),
            "maskA": m128(bA, KTA),
            "maskB": m128(bB, KTB),
        })

    t0 = time.perf_counter()
    res = run_bass_kernel_spmd(nc, in_maps, core_ids=list(range(N_CORES)))
    last_exec_wall_s = time.perf_counter() - t0
    last_results = res

    outs = [res.results[c]["out"].astype(np.float32) for c in range(N_CORES)]
    acc = outs[0]
    for c in range(1, N_CORES):
        acc = acc + outs[c]
    full = np.empty((B, SQ, D), dtype=np.float32)
    full[bA] = acc[0]
    full[bB] = acc[1]
    return full


# revision 72
# speedup vs baseline: 1.1308x; 1.0114x over previous
"""Multi-head attention (B=2, S=2048, D=1024, H=16) on 8 Trainium2 cores.

Sharding: pure tensor-parallel over heads (Megatron): core c owns heads
{2c, 2c+1} (d_local = 128 columns of Wq/Wk/Wv, 128 rows of Wo) and
processes BOTH batches. Each core emits a [2, 2048, 1024] partial output
(row-parallel Wo); the host sums the 8 partials per batch.

Why: the SPMD program's attention work scales with KT0+KT1 (per 2 heads)
instead of 4*max(KT0,KT1) (per 4 heads) under the old batch x head-group
split, so key-length imbalance between the two batches no longer inflates
the program's critical path.

Key-side truncation: only ceil(valid_len/128) key tiles per batch are
computed; the per-batch mask rides the ScalarE exp as a per-partition
bias. Scores are computed transposed ([key, query]); the softmax
denominator comes free via 64 ones-columns appended to V (ones-trick).

Precision: fp16 streams/weights, fp32 PSUM accumulation (rel err ~8e-4).

Schedule (single instruction stream, deeply interleaved):
- Q streams in column chunks; attention runs per (512-query-chunk, head)
  so the first scores start after only the first Q column pass.
- Batch B's K/V/Q projections run as single-PSUM-bank steps interleaved
  into batch A's attention cadence; batch A's output projection rides
  batch B's attention; batch B's output projection is the only tail.
- PSUM: scores rotation 4x[128,512] + ctx accumulators 2 + shared aux 2
  (projection steps / out-proj) = 8 banks, one configuration end to end.
- PSUM evacuation is split across DVE and ACT (GPSIMD cannot touch PSUM);
  out-DMAs fire per query-tile pair; PE p-state is kept warm by filler
  matmuls during the initial DMA latency.

The program is built at call time from the actual valid_lens (cached by
(KTA, KTB)); batch A is the one with more key tiles and is processed
first so its longer attention phase starts as early as possible.
"""
import sys
if "/opt/trn_rl_repo" not in sys.path:
    sys.path.insert(0, "/opt/trn_rl_repo")
import os
import time
import numpy as np

B, SQ, SK, D, H, HD = 2, 2048, 2048, 1024, 16, 64
NEG = -1.0e6
N_CORES = 8
DL = 128          # d_local: 2 heads * 64
KD = D // 128     # contraction tiles over D

_NC_CACHE = {}
last_results = None
last_exec_wall_s = None


def _build(KTA, KTB):
    import concourse.bass as bass  # noqa: F401
    import concourse.tile as tile
    from concourse import bacc, mybir

    f32 = mybir.dt.float32
    f16 = mybir.dt.float16
    EXP = mybir.ActivationFunctionType.Exp

    LKA, LKB = KTA * 128, KTB * 128
    # [(k0, nk)] chunk groups for the k/v/weight streams (fewer, larger DMAs)
    kgrp = [(0, 4), (4, 4)]

    nc = bacc.Bacc("TRN2", target_bir_lowering=False, debug=False,
                   num_devices=N_CORES)
    xqT = nc.dram_tensor("xqT", [2, D, SQ], f16, kind="ExternalInput")
    xkTA = nc.dram_tensor("xkTA", [D, LKA], f16, kind="ExternalInput")
    xvTA = nc.dram_tensor("xvTA", [D, LKA], f16, kind="ExternalInput")
    xkTB = nc.dram_tensor("xkTB", [D, LKB], f16, kind="ExternalInput")
    xvTB = nc.dram_tensor("xvTB", [D, LKB], f16, kind="ExternalInput")
    wqkv = nc.dram_tensor("wqkv", [D, 3 * DL], f16, kind="ExternalInput")
    wo = nc.dram_tensor("wo", [DL, D], f16, kind="ExternalInput")
    maskA = nc.dram_tensor("maskA", [128, KTA], f32, kind="ExternalInput")
    maskB = nc.dram_tensor("maskB", [128, KTB], f32, kind="ExternalInput")
    out = nc.dram_tensor("out", [2, SQ, D], f16, kind="ExternalOutput")

    with tile.TileContext(nc) as tc:
        with tc.tile_pool(name="singles", bufs=1) as sg:
            wqkv_sb = sg.tile([128, KD, 3 * DL], f16)
            wo_sb = sg.tile([128, D], f16)
            maskA_sb = sg.tile([128, KTA], f32)
            maskB_sb = sg.tile([128, KTB], f32)
            kt_sb = {0: sg.tile([128, LKA], f16, name="ktA"),
                     1: sg.tile([128, LKB], f16, name="ktB")}
            qt_sb = {0: sg.tile([128, SQ], f16, name="qtA"),
                     1: sg.tile([128, SQ], f16, name="qtB")}
            v_sb = {0: sg.tile([128, KTA, 2, 128], f16, name="vA"),
                    1: sg.tile([128, KTB, 2, 128], f16, name="vB")}
            ctx_sb = {0: sg.tile([128, SQ], f16, name="ctxA"),
                      1: sg.tile([128, SQ], f16, name="ctxB")}
            warm_sb = sg.tile([128, 256], f16)

            KT = {0: KTA, 1: KTB}
            mask_sb = {0: maskA_sb, 1: maskB_sb}
            xkT = {0: xkTA, 1: xkTB}
            xvT = {0: xvTA, 1: xvTB}

            # V'' ones-columns (softmax denominator); dim columns are
            # overwritten by the V-projection evacuations below.
            nc.gpsimd.memset(v_sb[0], 1.0)
            nc.gpsimd.memset(v_sb[1], 1.0)
            nc.vector.memset(warm_sb, 0.0)

            # ---- input DMAs, arrival order = need order ----
            nc.sync.dma_start(
                out=wqkv_sb[:, 0:4, :],
                in_=wqkv[0:512, :].rearrange("(k p) j -> p k j", p=128))
            strm_cm = tc.tile_pool(name="streams", bufs=1)
            strm = strm_cm.__enter__()
            xk = {b: strm.tile([128, KD, 128 * KT[b]], f16, name=f"xk{b}")
                  for b in (0, 1)}
            xq = {b: strm.tile([128, KD, SQ], f16, name=f"xq{b}")
                  for b in (0, 1)}
            xv = {b: strm.tile([128, KD, 128 * KT[b]], f16, name=f"xv{b}")
                  for b in (0, 1)}
            def dma_kv(b, which):
                src = xkT[b] if which == "k" else xvT[b]
                dst = xk[b] if which == "k" else xv[b]
                for k0, nk in kgrp:
                    nc.sync.dma_start(
                        out=dst[:, k0:k0 + nk, :],
                        in_=src[k0 * 128:(k0 + nk) * 128, :]
                        .rearrange("(k p) j -> p k j", p=128))

            def dma_q(b):
                for k in range(KD):
                    nc.sync.dma_start(out=xq[b][:, k, :],
                                      in_=xqT[b, k * 128:(k + 1) * 128, :])

            def dma_q_cols(b):
                # column-chunk order: Q-projection pass ci becomes ready
                # as soon as chunk ci lands (contraction needs all k)
                for ci in range(4):
                    nc.sync.dma_start(
                        out=xq[b][:, :, ci * 512:(ci + 1) * 512],
                        in_=xqT[b, :, ci * 512:(ci + 1) * 512]
                        .rearrange("(k p) j -> p k j", p=128))

            dma_kv(0, "k")
            nc.sync.dma_start(
                out=wqkv_sb[:, 4:8, :],
                in_=wqkv[512:1024, :].rearrange("(k p) j -> p k j", p=128))
            nc.sync.dma_start(out=maskA_sb, in_=maskA[:, :])
            nc.sync.dma_start(out=maskB_sb, in_=maskB[:, :])
            dma_kv(0, "v")   # xv before xq: ctx never stalls on V''
            dma_q_cols(0)
            dma_kv(1, "k")
            dma_kv(1, "v")
            dma_q_cols(1)
            nc.sync.dma_start(out=wo_sb, in_=wo[:, :])

            # ---- PE p-state warmup: keep the tensor engine busy during the
            # initial DMA latency so real matmuls start at full clock; the
            # pool stays open so stream-gated phases can emit filler too ----
            psW_cm = tc.tile_pool(name="psW", bufs=1, space="PSUM")
            psW = psW_cm.__enter__()
            wp = psW.tile([128, 256], f32)

            def wfill(n):
                for _ in range(n):
                    nc.tensor.matmul(wp, warm_sb[:, 0:128], warm_sb,
                                     start=True, stop=True)

            wfill(22)

            def copy_eng(eng, dst, src):
                if eng == "v":
                    nc.vector.tensor_copy(dst, src)
                elif eng == "a":
                    nc.scalar.copy(dst, src)
                else:
                    nc.gpsimd.tensor_copy(dst, src)

            def proj_k(b, eng):
                # K^T[b] = Wk^T @ Xk^T : [128, LK_b]
                LK = 128 * KT[b]
                chunks = [(i * 512, min(512, LK - i * 512))
                          for i in range((LK + 511) // 512)]
                with tc.tile_pool(name=f"psA{b}", bufs=1, space="PSUM") as ps:
                    accs = [ps.tile([128, cw], f32, tag=f"kt{ci}",
                                    name=f"kt{b}_{ci}")
                            for ci, (c0, cw) in enumerate(chunks)]
                    for k in range(KD):
                        for ci, (c0, cw) in enumerate(chunks):
                            nc.tensor.matmul(accs[ci],
                                             wqkv_sb[:, k, 0:128],
                                             xk[b][:, k, c0:c0 + cw],
                                             start=(k == 0), stop=(k == KD - 1))
                    for ci, (c0, cw) in enumerate(chunks):
                        copy_eng(eng[ci % len(eng)],
                                 kt_sb[b][:, c0:c0 + cw], accs[ci])

            def proj_q(b, eng, wfill=None):
                # Q^T[b] = Wq^T @ Xq^T : [128, 2048]. wfill emits idle
                # matmuls between DMA-gated k-chunks to hold the PE p-state.
                with tc.tile_pool(name=f"psB{b}", bufs=1, space="PSUM") as ps:
                    accs = [ps.tile([128, 512], f32, tag=f"q{ci}",
                                    name=f"q{b}_{ci}") for ci in range(4)]
                    for k in range(KD):
                        for ci in range(4):
                            nc.tensor.matmul(accs[ci],
                                             wqkv_sb[:, k, 128:256],
                                             xq[b][:, k, ci * 512:(ci + 1) * 512],
                                             start=(k == 0), stop=(k == KD - 1))
                        if wfill is not None and k < KD - 1:
                            wfill(2)
                    for ci in range(4):
                        copy_eng(eng[ci % len(eng)],
                                 qt_sb[b][:, ci * 512:(ci + 1) * 512], accs[ci])

            def proj_v(b, eng):
                # V''[b] : [key, head, dim|ones]; one pool, per-group tags,
                # so a later group never waits an earlier group's evacs
                gs = [list(range(g0, min(g0 + 4, KT[b])))
                      for g0 in range(0, KT[b], 4)]
                if KT[b] > 7:   # bank budget: fall back to serial groups
                    gs = [[t] for t in range(KT[b])]
                with tc.tile_pool(name=f"psV{b}", bufs=1, space="PSUM") as ps:
                    for gi, g in enumerate(gs):
                        tag = f"vg{gi % 4}" if KT[b] > 7 else f"vg{gi}"
                        vacc = ps.tile([128, len(g), 512], f32, tag=tag,
                                       name=f"v{b}_{gi}")
                        for k in range(KD):
                            for vi, t in enumerate(g):
                                nc.tensor.matmul(
                                    vacc[:, vi, 0:DL],
                                    xv[b][:, k, t * 128:(t + 1) * 128],
                                    wqkv_sb[:, k, 256:384],
                                    start=(k == 0), stop=(k == KD - 1),
                                    skip_group_check=True)
                        for vi, t in enumerate(g):
                            for hh in range(2):
                                copy_eng(eng[(2 * vi + hh) % len(eng)],
                                         v_sb[b][:, t, hh, 0:64],
                                         vacc[:, vi, hh * 64:(hh + 1) * 64])

            def attend(b, pools, extra=None):
                # scores^T -> exp -> ctx'' per (head, 512-query-chunk,
                # key-tile), chunk-major so chunk c needs only Q column
                # pass c; ctx accumulates in a [128,512] 1-bank tile,
                # normalized per chunk. extra(i) interleaves foreign work.
                ptp, mp, psS, psC = pools
                nit = 0
                for cq in range(4):
                    for hh in range(2):
                        q0 = cq * 512
                        ctx_ps = psC.tile([128, 512], f32, tag="ctx",
                                          name=f"ctx{b}_{hh}_{cq}")
                        for t in range(KT[b]):
                            s_ps = psS.tile([128, 512], f32, tag="s")
                            nc.tensor.matmul(
                                s_ps,
                                kt_sb[b][hh * 64:hh * 64 + 64,
                                         t * 128:(t + 1) * 128],
                                qt_sb[b][hh * 64:hh * 64 + 64,
                                         q0:q0 + 512],
                                start=True, stop=True)
                            pt = ptp.tile([128, 512], f16, tag="pt")
                            nc.scalar.activation(
                                pt, s_ps, EXP,
                                bias=mask_sb[b][:, t:t + 1], scale=0.125)
                            nc.tensor.matmul(
                                ctx_ps, v_sb[b][:, t, hh, :], pt,
                                start=(t == 0), stop=(t == KT[b] - 1),
                                skip_group_check=True)
                            if extra is not None:
                                extra(nit)
                            nit += 1
                        # rows 64-127 all hold the softmax denominator
                        # (only one TensorTensor input may come from PSUM,
                        # so reciprocal lands in SBUF first)
                        rcb = mp.tile([64, 512], f32, tag="rcb")
                        nc.vector.reciprocal(rcb, ctx_ps[64:128, :])
                        nc.vector.tensor_mul(
                            ctx_sb[b][hh * 64:hh * 64 + 64, q0:q0 + 512],
                            ctx_ps[0:64, :], rcb)

            evac_ct = [0]
            osb_map = {0: {}, 1: {}}

            def out_proj(b, pools, qr, engines):
                # partial out[b] rows = ctx''[b]^T @ Wo_local; [128,512]
                # PSUM grain; o_sb tiles live per query-tile PAIR (the DMA
                # after the odd qi reads both halves)
                psD, op = pools
                for qi in qr:
                    pb = qi // 2
                    if pb not in osb_map[b]:
                        osb_map[b][pb] = op.tile(
                            [128, 2, D], f16, tag=f"o{pb % 2}",
                            name=f"osb{b}_{pb}")
                    o_sb = osb_map[b][pb]
                    for n in range(2):
                        o_ps = psD.tile([128, 512], f32, tag="x",
                                        name=f"o{b}_{qi}_{n}")
                        nc.tensor.matmul(o_ps,
                                         ctx_sb[b][:, qi * 128:(qi + 1) * 128],
                                         wo_sb[:, n * 512:(n + 1) * 512],
                                         start=True, stop=True)
                        eng = engines[evac_ct[0] % len(engines)]
                        evac_ct[0] += 1
                        copy_eng(eng, o_sb[:, qi % 2, n * 512:(n + 1) * 512],
                                 o_ps)
                    if qi % 2 == 1:
                        nc.sync.dma_start(
                            out=out[b, (qi - 1) * 128:(qi + 1) * 128, :]
                            .rearrange("(c p) j -> p c j", p=128),
                            in_=o_sb)

            # ---- batch A K/V projections ride the early streams; Q runs
            # as column passes so attention starts while Q still streams ----
            proj_k(0, ("v",))
            proj_v(0, ("v",))
            psW_cm.__exit__(None, None, None)

            # batch-B projections as single-PSUM-bank steps, interleaved
            # into batch-A's ACT-bound attention cadence (all on GPSIMD so
            # nothing queues behind DVE norms)
            def bsteps(psX):
                LKB_ = 128 * KT[1]

                def a1b_step(c0, cw):
                    acc = psX.tile([128, 512], f32, tag="x", name="xa")
                    for k in range(KD):
                        nc.tensor.matmul(acc[:, 0:cw], wqkv_sb[:, k, 0:128],
                                         xk[1][:, k, c0:c0 + cw],
                                         start=(k == 0), stop=(k == KD - 1))
                    copy_eng("v", kt_sb[1][:, c0:c0 + cw], acc[:, 0:cw])

                def vb_step(t):
                    acc = psX.tile([128, 512], f32, tag="x", name="xv")
                    for k in range(KD):
                        nc.tensor.matmul(acc[:, 0:DL],
                                         xv[1][:, k, t * 128:(t + 1) * 128],
                                         wqkv_sb[:, k, 256:384],
                                         start=(k == 0), stop=(k == KD - 1))
                    for hh in range(2):
                        copy_eng("v", v_sb[1][:, t, hh, 0:64],
                                 acc[:, hh * 64:(hh + 1) * 64])

                def qb_step(ci, b=1):
                    acc = psX.tile([128, 512], f32, tag="x", name="xq")
                    for k in range(KD):
                        nc.tensor.matmul(acc, wqkv_sb[:, k, 128:256],
                                         xq[b][:, k, ci * 512:(ci + 1) * 512],
                                         start=(k == 0), stop=(k == KD - 1))
                    copy_eng("v", qt_sb[b][:, ci * 512:(ci + 1) * 512], acc)

                steps = []
                for i in range((LKB_ + 511) // 512):
                    c0 = i * 512
                    steps.append(lambda c0=c0, cw=min(512, LKB_ - c0):
                                 a1b_step(c0, cw))
                steps.extend(lambda t=t: vb_step(t) for t in range(KT[1]))
                steps.extend(lambda ci=ci: qb_step(ci) for ci in range(4))
                return steps, qb_step

            # One continuous PSUM configuration from first attention to last
            # output tile: psS (2 banks, score rotation) + psC (4 banks, ctx
            # accumulator) + aux (2 banks, shared rotation for batch-B
            # projection steps, then both batches' out-projection PSUM).
            with tc.tile_pool(name="pt", bufs=6) as ptp, \
                 tc.tile_pool(name="misc", bufs=2) as mp, \
                 tc.tile_pool(name="ob", bufs=6) as op:
                with tc.tile_pool(name="psS", bufs=4, space="PSUM") as psS, \
                     tc.tile_pool(name="psC", bufs=2, space="PSUM") as psC, \
                     tc.tile_pool(name="aux", bufs=2, space="PSUM") as aux:
                    steps, qa_step = bsteps(aux)
                    # Batch A's Q column pass 0 gates the first group;
                    # passes 1-3 interleave ahead of the chunks that need
                    # them, tracking the xq column-DMA arrivals. Batch-B
                    # projection steps ride attend(0)'s second half.
                    qa_step(0, b=0)
                    nit_A = 8 * KT[0]
                    smap = {}
                    for i, frac in ((1, 0.15), (2, 0.375), (3, 0.55)):
                        it = max(i, int(nit_A * frac))
                        smap.setdefault(it, []).append(
                            lambda ci=i: qa_step(ci, b=0))
                    for j, s in enumerate(steps):
                        it = max(4, int(nit_A * (0.62 + 0.33 * j / len(steps))))
                        smap.setdefault(it, []).append(s)

                    # a small slice of batch A's out-projection (qi 0-3,
                    # query chunk 0, normalized after group 4) rides the tail
                    # of attend(0) so its out-DMAs start during the DMA lull
                    qmapA = {}

                    def extraA(i):
                        for s in smap.get(i, ()):
                            s()
                        if i in qmapA:
                            out_proj(0, (aux, op), qmapA[i], ("a", "v"))

                    attend(0, (ptp, mp, psS, psC), extra=extraA)
                    for it, fns in sorted(smap.items()):
                        if it >= nit_A:
                            for s in fns:
                                s()
                    done_A = sorted(q for qs in qmapA.values() for q in qs)
                    rest_A = [q for q in range(16) if q not in done_A]
                    # batch B attention carries the rest of batch A's
                    # out-projection, spread across its cadence
                    nb = 8 * KT[1]
                    qsched = {}
                    ns = max(1, nb - 1)
                    nr = len(rest_A)
                    for i in range(ns):
                        qsched[i] = (rest_A[nr * i // ns: nr * (i + 1) // ns],
                                     ("v", "v", "a"))

                    def extra(i):
                        if i in qsched:
                            qr, eng = qsched[i]
                            out_proj(0, (aux, op), qr, eng)

                    attend(1, (ptp, mp, psS, psC), extra=extra)
                # final out-projection in its own deep PSUM rotation so the
                # tail runs at the out-DMA rate, not the evacuation rate
                with tc.tile_pool(name="psD2", bufs=6, space="PSUM") as psD2:
                    out_proj(1, (psD2, op), range(16), ("a", "v"))
            strm_cm.__exit__(None, None, None)
    nc.compile()
    return nc


def kernel(**inputs):
    global last_results, last_exec_wall_s
    from concourse.bass_utils import run_bass_kernel_spmd

    # BASS_TRACE needs the axon NTFF hook; disable tracing when the hook
    # module is unavailable so a stray env var cannot crash the run.
    if os.environ.get("BASS_TRACE"):
        try:
            from antenv import axon_hooks  # noqa: F401
        except Exception:
            os.environ["BASS_NEVER_TRACE"] = "1"

    q = np.asarray(inputs["queries"], dtype=np.float32)
    kx = np.asarray(inputs["keys"], dtype=np.float32)
    vx = np.asarray(inputs["values"], dtype=np.float32)
    vl = np.asarray(inputs["valid_lens"], dtype=np.int64).reshape(B)
    Wq = np.asarray(inputs["Wq"], dtype=np.float32)
    Wk = np.asarray(inputs["Wk"], dtype=np.float32)
    Wv = np.asarray(inputs["Wv"], dtype=np.float32)
    Wo = np.asarray(inputs["Wo"], dtype=np.float32)
    assert q.shape == (B, SQ, D) and kx.shape == (B, SK, D) and vx.shape == (B, SK, D)

    lens = np.clip(vl, 1, SK)
    KTs = [(int(l) + 127) // 128 for l in lens]
    # batch A = more key tiles, processed first
    bA = 0 if KTs[0] >= KTs[1] else 1
    bB = 1 - bA
    KTA, KTB = KTs[bA], KTs[bB]
    LKA, LKB = KTA * 128, KTB * 128

    key = (KTA, KTB)
    if key not in _NC_CACHE:
        _NC_CACHE[key] = _build(KTA, KTB)
    nc = _NC_CACHE[key]

    def m128(b, KT):
        m = np.where(np.arange(KT * 128) < lens[b], 0.0, NEG).astype(np.float32)
        return np.ascontiguousarray(m.reshape(KT, 128).T)

    xqT_full = np.ascontiguousarray(
        np.stack([q[bA].T, q[bB].T]).astype(np.float16))
    in_maps = []
    for c in range(N_CORES):
        cols = slice(DL * c, DL * (c + 1))
        in_maps.append({
            "xqT": xqT_full,
            "xkTA": np.ascontiguousarray(kx[bA, :LKA].T.astype(np.float16)),
            "xvTA": np.ascontiguousarray(vx[bA, :LKA].T.astype(np.float16)),
            "xkTB": np.ascontiguousarray(kx[bB, :LKB].T.astype(np.float16)),
            "xvTB": np.ascontiguousarray(vx[bB, :LKB].T.astype(np.float16)),
            "wqkv": np.ascontiguousarray(np.concatenate(
                [Wk[:, cols], Wq[:, cols], Wv[:, cols]],
                axis=1).astype(np.float16)),
            "wo": np.ascontiguousarray(Wo[cols, :].astype(np.float16)),
            "maskA": m128(bA, KTA),
            "maskB": m128(bB, KTB),
        })

    t0 = time.perf_counter()
    res = run_bass_kernel_spmd(nc, in_maps, core_ids=list(range(N_CORES)))
    last_exec_wall_s = time.perf_counter() - t0
    last_results = res

    outs = [res.results[c]["out"].astype(np.float32) for c in range(N_CORES)]
    acc = outs[0]
    for c in range(1, N_CORES):
        acc = acc + outs[c]
    full = np.empty((B, SQ, D), dtype=np.float32)
    full[bA] = acc[0]
    full[bB] = acc[1]
    return full


# revision 77
# speedup vs baseline: 1.1423x; 1.0101x over previous
"""Multi-head attention (B=2, S=2048, D=1024, H=16) on 8 Trainium2 cores.

Sharding: pure tensor-parallel over heads (Megatron): core c owns heads
{2c, 2c+1} (d_local = 128 columns of Wq/Wk/Wv, 128 rows of Wo) and
processes BOTH batches. Each core emits a [2, 2048, 1024] partial output
(row-parallel Wo); the host sums the 8 partials per batch.

Why: the SPMD program's attention work scales with KT0+KT1 (per 2 heads)
instead of 4*max(KT0,KT1) (per 4 heads) under the old batch x head-group
split, so key-length imbalance between the two batches no longer inflates
the program's critical path.

Key-side truncation: only ceil(valid_len/128) key tiles per batch are
computed; the per-batch mask rides the ScalarE exp as a per-partition
bias. Scores are computed transposed ([key, query]); the softmax
denominator comes free via 64 ones-columns appended to V (ones-trick).

Precision: fp16 streams/weights, fp32 PSUM accumulation (rel err ~8e-4).

Schedule (single instruction stream, deeply interleaved):
- Q streams in column chunks; attention runs per (512-query-chunk, head)
  so the first scores start after only the first Q column pass.
- Batch B's K/V/Q projections run as single-PSUM-bank steps interleaved
  into batch A's attention cadence; batch A's output projection rides
  batch B's attention; batch B's output projection is the only tail.
- PSUM: scores rotation 4x[128,512] + ctx accumulators 2 + shared aux 2
  (projection steps / out-proj) = 8 banks, one configuration end to end.
- PSUM evacuation is split across DVE and ACT (GPSIMD cannot touch PSUM);
  out-DMAs fire per query-tile pair; PE p-state is kept warm by filler
  matmuls during the initial DMA latency.

The program is built at call time from the actual valid_lens (cached by
(KTA, KTB)); batch A is the one with more key tiles and is processed
first so its longer attention phase starts as early as possible.
"""
import sys
if "/opt/trn_rl_repo" not in sys.path:
    sys.path.insert(0, "/opt/trn_rl_repo")
import os
import time
import numpy as np

B, SQ, SK, D, H, HD = 2, 2048, 2048, 1024, 16, 64
NEG = -1.0e6
N_CORES = 8
DL = 128          # d_local: 2 heads * 64
KD = D // 128     # contraction tiles over D

_NC_CACHE = {}
last_results = None
last_exec_wall_s = None


def _build(KTA, KTB):
    import concourse.bass as bass  # noqa: F401
    import concourse.tile as tile
    from concourse import bacc, mybir

    f32 = mybir.dt.float32
    f16 = mybir.dt.float16
    EXP = mybir.ActivationFunctionType.Exp

    LKA, LKB = KTA * 128, KTB * 128
    # [(k0, nk)] chunk groups for the k/v/weight streams (fewer, larger DMAs)
    kgrp = [(0, 4), (4, 4)]

    nc = bacc.Bacc("TRN2", target_bir_lowering=False, debug=False,
                   num_devices=N_CORES)
    xqT = nc.dram_tensor("xqT", [2, D, SQ], f16, kind="ExternalInput")
    xkTA = nc.dram_tensor("xkTA", [D, LKA], f16, kind="ExternalInput")
    xvTA = nc.dram_tensor("xvTA", [D, LKA], f16, kind="ExternalInput")
    xkTB = nc.dram_tensor("xkTB", [D, LKB], f16, kind="ExternalInput")
    xvTB = nc.dram_tensor("xvTB", [D, LKB], f16, kind="ExternalInput")
    wqkv = nc.dram_tensor("wqkv", [D, 3 * DL], f16, kind="ExternalInput")
    wo = nc.dram_tensor("wo", [DL, D], f16, kind="ExternalInput")
    maskA = nc.dram_tensor("maskA", [128, KTA], f32, kind="ExternalInput")
    maskB = nc.dram_tensor("maskB", [128, KTB], f32, kind="ExternalInput")
    out = nc.dram_tensor("out", [2, SQ, D], f16, kind="ExternalOutput")

    with tile.TileContext(nc) as tc:
        with tc.tile_pool(name="singles", bufs=1) as sg:
            wqkv_sb = sg.tile([128, KD, 3 * DL], f16)
            wo_sb = sg.tile([128, D], f16)
            maskA_sb = sg.tile([128, KTA], f32)
            maskB_sb = sg.tile([128, KTB], f32)
            kt_sb = {0: sg.tile([128, LKA], f16, name="ktA"),
                     1: sg.tile([128, LKB], f16, name="ktB")}
            qt_sb = {0: sg.tile([128, SQ], f16, name="qtA"),
                     1: sg.tile([128, SQ], f16, name="qtB")}
            v_sb = {0: sg.tile([128, KTA, 2, 128], f16, name="vA"),
                    1: sg.tile([128, KTB, 2, 128], f16, name="vB")}
            ctx_sb = {0: sg.tile([128, SQ], f16, name="ctxA"),
                      1: sg.tile([128, SQ], f16, name="ctxB")}
            warm_sb = sg.tile([128, 256], f16)

            KT = {0: KTA, 1: KTB}
            mask_sb = {0: maskA_sb, 1: maskB_sb}
            xkT = {0: xkTA, 1: xkTB}
            xvT = {0: xvTA, 1: xvTB}

            # V'' ones-columns (softmax denominator); dim columns are
            # overwritten by the V-projection evacuations below.
            nc.gpsimd.memset(v_sb[0], 1.0)
            nc.gpsimd.memset(v_sb[1], 1.0)
            nc.vector.memset(warm_sb, 0.0)

            # ---- input DMAs, arrival order = need order ----
            nc.sync.dma_start(
                out=wqkv_sb[:, 0:4, :],
                in_=wqkv[0:512, :].rearrange("(k p) j -> p k j", p=128))
            strm_cm = tc.tile_pool(name="streams", bufs=1)
            strm = strm_cm.__enter__()
            xk = {b: strm.tile([128, KD, 128 * KT[b]], f16, name=f"xk{b}")
                  for b in (0, 1)}
            xq = {b: strm.tile([128, KD, SQ], f16, name=f"xq{b}")
                  for b in (0, 1)}
            xv = {b: strm.tile([128, KD, 128 * KT[b]], f16, name=f"xv{b}")
                  for b in (0, 1)}
            def dma_kv(b, which):
                src = xkT[b] if which == "k" else xvT[b]
                dst = xk[b] if which == "k" else xv[b]
                for k0, nk in kgrp:
                    nc.sync.dma_start(
                        out=dst[:, k0:k0 + nk, :],
                        in_=src[k0 * 128:(k0 + nk) * 128, :]
                        .rearrange("(k p) j -> p k j", p=128))

            def dma_q(b):
                for k in range(KD):
                    nc.sync.dma_start(out=xq[b][:, k, :],
                                      in_=xqT[b, k * 128:(k + 1) * 128, :])

            def dma_q_cols(b):
                # column-chunk order: Q-projection pass ci becomes ready
                # as soon as chunk ci lands (contraction needs all k)
                for ci in range(4):
                    nc.sync.dma_start(
                        out=xq[b][:, :, ci * 512:(ci + 1) * 512],
                        in_=xqT[b, :, ci * 512:(ci + 1) * 512]
                        .rearrange("(k p) j -> p k j", p=128))

            dma_kv(0, "k")
            nc.sync.dma_start(
                out=wqkv_sb[:, 4:8, :],
                in_=wqkv[512:1024, :].rearrange("(k p) j -> p k j", p=128))
            nc.sync.dma_start(out=maskA_sb, in_=maskA[:, :])
            nc.sync.dma_start(out=maskB_sb, in_=maskB[:, :])
            dma_kv(0, "v")   # xv before xq: ctx never stalls on V''
            dma_q_cols(0)
            dma_kv(1, "k")
            dma_kv(1, "v")
            dma_q_cols(1)
            nc.sync.dma_start(out=wo_sb, in_=wo[:, :])

            # ---- PE p-state warmup: keep the tensor engine busy during the
            # initial DMA latency so real matmuls start at full clock; the
            # pool stays open so stream-gated phases can emit filler too ----
            psW_cm = tc.tile_pool(name="psW", bufs=1, space="PSUM")
            psW = psW_cm.__enter__()
            wp = psW.tile([128, 256], f32)

            def wfill(n):
                for _ in range(n):
                    nc.tensor.matmul(wp, warm_sb[:, 0:128], warm_sb,
                                     start=True, stop=True)

            wfill(22)

            def copy_eng(eng, dst, src):
                if eng == "v":
                    nc.vector.tensor_copy(dst, src)
                elif eng == "a":
                    nc.scalar.copy(dst, src)
                else:
                    nc.gpsimd.tensor_copy(dst, src)

            def proj_k(b, eng):
                # K^T[b] = Wk^T @ Xk^T : [128, LK_b]
                LK = 128 * KT[b]
                chunks = [(i * 512, min(512, LK - i * 512))
                          for i in range((LK + 511) // 512)]
                with tc.tile_pool(name=f"psA{b}", bufs=1, space="PSUM") as ps:
                    accs = [ps.tile([128, cw], f32, tag=f"kt{ci}",
                                    name=f"kt{b}_{ci}")
                            for ci, (c0, cw) in enumerate(chunks)]
                    for k in range(KD):
                        for ci, (c0, cw) in enumerate(chunks):
                            nc.tensor.matmul(accs[ci],
                                             wqkv_sb[:, k, 0:128],
                                             xk[b][:, k, c0:c0 + cw],
                                             start=(k == 0), stop=(k == KD - 1))
                    for ci, (c0, cw) in enumerate(chunks):
                        copy_eng(eng[ci % len(eng)],
                                 kt_sb[b][:, c0:c0 + cw], accs[ci])

            def proj_q(b, eng, wfill=None):
                # Q^T[b] = Wq^T @ Xq^T : [128, 2048]. wfill emits idle
                # matmuls between DMA-gated k-chunks to hold the PE p-state.
                with tc.tile_pool(name=f"psB{b}", bufs=1, space="PSUM") as ps:
                    accs = [ps.tile([128, 512], f32, tag=f"q{ci}",
                                    name=f"q{b}_{ci}") for ci in range(4)]
                    for k in range(KD):
                        for ci in range(4):
                            nc.tensor.matmul(accs[ci],
                                             wqkv_sb[:, k, 128:256],
                                             xq[b][:, k, ci * 512:(ci + 1) * 512],
                                             start=(k == 0), stop=(k == KD - 1))
                        if wfill is not None and k < KD - 1:
                            wfill(2)
                    for ci in range(4):
                        copy_eng(eng[ci % len(eng)],
                                 qt_sb[b][:, ci * 512:(ci + 1) * 512], accs[ci])

            def proj_v(b, eng):
                # V''[b] : [key, head, dim|ones]; one pool, per-group tags,
                # so a later group never waits an earlier group's evacs
                gs = [list(range(g0, min(g0 + 4, KT[b])))
                      for g0 in range(0, KT[b], 4)]
                if KT[b] > 7:   # bank budget: fall back to serial groups
                    gs = [[t] for t in range(KT[b])]
                with tc.tile_pool(name=f"psV{b}", bufs=1, space="PSUM") as ps:
                    for gi, g in enumerate(gs):
                        tag = f"vg{gi % 4}" if KT[b] > 7 else f"vg{gi}"
                        vacc = ps.tile([128, len(g), 512], f32, tag=tag,
                                       name=f"v{b}_{gi}")
                        for k in range(KD):
                            for vi, t in enumerate(g):
                                nc.tensor.matmul(
                                    vacc[:, vi, 0:DL],
                                    xv[b][:, k, t * 128:(t + 1) * 128],
                                    wqkv_sb[:, k, 256:384],
                                    start=(k == 0), stop=(k == KD - 1),
                                    skip_group_check=True)
                        for vi, t in enumerate(g):
                            for hh in range(2):
                                copy_eng(eng[(2 * vi + hh) % len(eng)],
                                         v_sb[b][:, t, hh, 0:64],
                                         vacc[:, vi, hh * 64:(hh + 1) * 64])

            def attend(b, pools, extra=None):
                # scores^T -> exp -> ctx'' per (head, 512-query-chunk,
                # key-tile), chunk-major so chunk c needs only Q column
                # pass c; ctx accumulates in a [128,512] 1-bank tile,
                # normalized per chunk. extra(i) interleaves foreign work.
                ptp, mp, psS, psC = pools
                nit = 0
                for cq in range(4):
                    for hh in range(2):
                        q0 = cq * 512
                        ctx_ps = psC.tile([128, 512], f32, tag="ctx",
                                          name=f"ctx{b}_{hh}_{cq}")
                        for t in range(KT[b]):
                            s_ps = psS.tile([128, 512], f32, tag="s")
                            nc.tensor.matmul(
                                s_ps,
                                kt_sb[b][hh * 64:hh * 64 + 64,
                                         t * 128:(t + 1) * 128],
                                qt_sb[b][hh * 64:hh * 64 + 64,
                                         q0:q0 + 512],
                                start=True, stop=True)
                            pt = ptp.tile([128, 512], f16, tag="pt")
                            nc.scalar.activation(
                                pt, s_ps, EXP,
                                bias=mask_sb[b][:, t:t + 1], scale=0.125)
                            nc.tensor.matmul(
                                ctx_ps, v_sb[b][:, t, hh, :], pt,
                                start=(t == 0), stop=(t == KT[b] - 1),
                                skip_group_check=True)
                            if extra is not None:
                                extra(nit)
                            nit += 1
                        # rows 64-127 all hold the softmax denominator
                        # (only one TensorTensor input may come from PSUM,
                        # so reciprocal lands in SBUF first)
                        rcb = mp.tile([64, 512], f32, tag="rcb")
                        nc.vector.reciprocal(rcb, ctx_ps[64:128, :])
                        nc.vector.tensor_mul(
                            ctx_sb[b][hh * 64:hh * 64 + 64, q0:q0 + 512],
                            ctx_ps[0:64, :], rcb)

            evac_ct = [0]
            osb_map = {0: {}, 1: {}}

            def out_proj(b, pools, qr, engines):
                # partial out[b] rows = ctx''[b]^T @ Wo_local; [128,512]
                # PSUM grain; o_sb tiles live per query-tile PAIR (the DMA
                # after the odd qi reads both halves)
                psD, op = pools
                for qi in qr:
                    pb = qi // 2
                    if pb not in osb_map[b]:
                        osb_map[b][pb] = op.tile(
                            [128, 2, D], f16, tag=f"o{pb % 2}",
                            name=f"osb{b}_{pb}")
                    o_sb = osb_map[b][pb]
                    for n in range(2):
                        o_ps = psD.tile([128, 512], f32, tag="x",
                                        name=f"o{b}_{qi}_{n}")
                        nc.tensor.matmul(o_ps,
                                         ctx_sb[b][:, qi * 128:(qi + 1) * 128],
                                         wo_sb[:, n * 512:(n + 1) * 512],
                                         start=True, stop=True)
                        eng = engines[evac_ct[0] % len(engines)]
                        evac_ct[0] += 1
                        copy_eng(eng, o_sb[:, qi % 2, n * 512:(n + 1) * 512],
                                 o_ps)
                    if qi % 2 == 1:
                        nc.sync.dma_start(
                            out=out[b, (qi - 1) * 128:(qi + 1) * 128, :]
                            .rearrange("(c p) j -> p c j", p=128),
                            in_=o_sb)

            # ---- batch A K/V projections ride the early streams; Q runs
            # as column passes so attention starts while Q still streams ----
            proj_k(0, ("v",))
            proj_v(0, ("v",))
            psW_cm.__exit__(None, None, None)

            # batch-B projections as single-PSUM-bank steps, interleaved
            # into batch-A's ACT-bound attention cadence (all on GPSIMD so
            # nothing queues behind DVE norms)
            def bsteps(psX):
                LKB_ = 128 * KT[1]

                def a1b_step(c0, cw):
                    acc = psX.tile([128, 512], f32, tag="x", name="xa")
                    for k in range(KD):
                        nc.tensor.matmul(acc[:, 0:cw], wqkv_sb[:, k, 0:128],
                                         xk[1][:, k, c0:c0 + cw],
                                         start=(k == 0), stop=(k == KD - 1))
                    copy_eng("v", kt_sb[1][:, c0:c0 + cw], acc[:, 0:cw])

                def vb_step(t):
                    acc = psX.tile([128, 512], f32, tag="x", name="xv")
                    for k in range(KD):
                        nc.tensor.matmul(acc[:, 0:DL],
                                         xv[1][:, k, t * 128:(t + 1) * 128],
                                         wqkv_sb[:, k, 256:384],
                                         start=(k == 0), stop=(k == KD - 1))
                    for hh in range(2):
                        copy_eng("v", v_sb[1][:, t, hh, 0:64],
                                 acc[:, hh * 64:(hh + 1) * 64])

                def qb_step(ci, b=1):
                    acc = psX.tile([128, 512], f32, tag="x", name="xq")
                    for k in range(KD):
                        nc.tensor.matmul(acc, wqkv_sb[:, k, 128:256],
                                         xq[b][:, k, ci * 512:(ci + 1) * 512],
                                         start=(k == 0), stop=(k == KD - 1))
                    copy_eng("v", qt_sb[b][:, ci * 512:(ci + 1) * 512], acc)

                steps = []
                for i in range((LKB_ + 511) // 512):
                    c0 = i * 512
                    steps.append(lambda c0=c0, cw=min(512, LKB_ - c0):
                                 a1b_step(c0, cw))
                steps.extend(lambda t=t: vb_step(t) for t in range(KT[1]))
                steps.extend(lambda ci=ci: qb_step(ci) for ci in range(4))
                return steps, qb_step

            # One continuous PSUM configuration from first attention to last
            # output tile: psS (2 banks, score rotation) + psC (4 banks, ctx
            # accumulator) + aux (2 banks, shared rotation for batch-B
            # projection steps, then both batches' out-projection PSUM).
            with tc.tile_pool(name="pt", bufs=6) as ptp, \
                 tc.tile_pool(name="misc", bufs=2) as mp, \
                 tc.tile_pool(name="ob", bufs=6) as op:
                with tc.tile_pool(name="psS", bufs=4, space="PSUM") as psS, \
                     tc.tile_pool(name="psC", bufs=2, space="PSUM") as psC, \
                     tc.tile_pool(name="aux", bufs=2, space="PSUM") as aux:
                    steps, qa_step = bsteps(aux)
                    # Batch A's Q column pass 0 gates the first group;
                    # passes 1-3 interleave ahead of the chunks that need
                    # them, tracking the xq column-DMA arrivals. Batch-B
                    # projection steps ride attend(0)'s second half.
                    qa_step(0, b=0)
                    nit_A = 8 * KT[0]
                    smap = {}
                    for i, frac in ((1, 0.15), (2, 0.375), (3, 0.55)):
                        it = max(i, int(nit_A * frac))
                        smap.setdefault(it, []).append(
                            lambda ci=i: qa_step(ci, b=0))
                    for j, s in enumerate(steps):
                        it = max(4, int(nit_A * (0.62 + 0.33 * j / len(steps))))
                        smap.setdefault(it, []).append(s)

                    # a small slice of batch A's out-projection (qi 0-3,
                    # query chunk 0, normalized after group 4) rides the tail
                    # of attend(0) so its out-DMAs start during the DMA lull
                    qmapA = {}

                    def extraA(i):
                        for s in smap.get(i, ()):
                            s()
                        if i in qmapA:
                            out_proj(0, (aux, op), qmapA[i], ("a", "v"))

                    attend(0, (ptp, mp, psS, psC), extra=extraA)
                    for it, fns in sorted(smap.items()):
                        if it >= nit_A:
                            for s in fns:
                                s()
                    done_A = sorted(q for qs in qmapA.values() for q in qs)
                    rest_A = [q for q in range(16) if q not in done_A]
                    # batch B attention carries the rest of batch A's
                    # out-projection, spread across its cadence
                    nb = 8 * KT[1]
                    qsched = {}
                    ns = max(1, nb - 1)
                    nr = len(rest_A)
                    for i in range(ns):
                        # later slots lean on ACT: its exps wind down while
                        # DVE still carries the trailing norms
                        eng = ("v", "v", "a") if i < ns // 2 else ("a", "v")
                        qsched[i] = (rest_A[nr * i // ns: nr * (i + 1) // ns],
                                     eng)

                    def extra(i):
                        if i in qsched:
                            qr, eng = qsched[i]
                            out_proj(0, (aux, op), qr, eng)

                    attend(1, (ptp, mp, psS, psC), extra=extra)
                # final out-projection in its own deep PSUM rotation so the
                # tail runs at the out-DMA rate, not the evacuation rate
                with tc.tile_pool(name="psD2", bufs=6, space="PSUM") as psD2:
                    out_proj(1, (psD2, op), range(16), ("a", "v"))
            strm_cm.__exit__(None, None, None)
    nc.compile()
    return nc


def kernel(**inputs):
    global last_results, last_exec_wall_s
    from concourse.bass_utils import run_bass_kernel_spmd

    # BASS_TRACE needs the axon NTFF hook; disable tracing when the hook
    # module is unavailable so a stray env var cannot crash the run.
    if os.environ.get("BASS_TRACE"):
        try:
            from antenv import axon_hooks  # noqa: F401
        except Exception:
            os.environ["BASS_NEVER_TRACE"] = "1"

    q = np.asarray(inputs["queries"], dtype=np.float32)
    kx = np.asarray(inputs["keys"], dtype=np.float32)
    vx = np.asarray(inputs["values"], dtype=np.float32)
    vl = np.asarray(inputs["valid_lens"], dtype=np.int64).reshape(B)
    Wq = np.asarray(inputs["Wq"], dtype=np.float32)
    Wk = np.asarray(inputs["Wk"], dtype=np.float32)
    Wv = np.asarray(inputs["Wv"], dtype=np.float32)
    Wo = np.asarray(inputs["Wo"], dtype=np.float32)
    assert q.shape == (B, SQ, D) and kx.shape == (B, SK, D) and vx.shape == (B, SK, D)

    lens = np.clip(vl, 1, SK)
    KTs = [(int(l) + 127) // 128 for l in lens]
    # batch A = more key tiles, processed first
    bA = 0 if KTs[0] >= KTs[1] else 1
    bB = 1 - bA
    KTA, KTB = KTs[bA], KTs[bB]
    LKA, LKB = KTA * 128, KTB * 128

    key = (KTA, KTB)
    if key not in _NC_CACHE:
        _NC_CACHE[key] = _build(KTA, KTB)
    nc = _NC_CACHE[key]

    def m128(b, KT):
        m = np.where(np.arange(KT * 128) < lens[b], 0.0, NEG).astype(np.float32)
        return np.ascontiguousarray(m.reshape(KT, 128).T)

    xqT_full = np.ascontiguousarray(
        np.stack([q[bA].T, q[bB].T]).astype(np.float16))
    in_maps = []
    for c in range(N_CORES):
        cols = slice(DL * c, DL * (c + 1))
        in_maps.append({
            "xqT": xqT_full,
            "xkTA": np.ascontiguousarray(kx[bA, :LKA].T.astype(np.float16)),
            "xvTA": np.ascontiguousarray(vx[bA, :LKA].T.astype(np.float16)),
            "xkTB": np.ascontiguousarray(kx[bB, :LKB].T.astype(np.float16)),
            "xvTB": np.ascontiguousarray(vx[bB, :LKB].T.astype(np.float16)),
            "wqkv": np.ascontiguousarray(np.concatenate(
                [Wk[:, cols], Wq[:, cols], Wv[:, cols]],
                axis=1).astype(np.float16)),
            "wo": np.ascontiguousarray(Wo[cols, :].astype(np.float16)),
            "maskA": m128(bA, KTA),
            "maskB": m128(bB, KTB),
        })

    t0 = time.perf_counter()
    res = run_bass_kernel_spmd(nc, in_maps, core_ids=list(range(N_CORES)))
    last_exec_wall_s = time.perf_counter() - t0
    last_results = res

    outs = [res.results[c]["out"].astype(np.float32) for c in range(N_CORES)]
    acc = outs[0]
    for c in range(1, N_CORES):
        acc = acc + outs[c]
    full = np.empty((B, SQ, D), dtype=np.float32)
    full[bA] = acc[0]
    full[bB] = acc[1]
    return full


# revision 86
# speedup vs baseline: 1.1536x; 1.0099x over previous
"""Multi-head attention (B=2, S=2048, D=1024, H=16) on 8 Trainium2 cores.

Sharding: pure tensor-parallel over heads (Megatron): core c owns heads
{2c, 2c+1} (d_local = 128 columns of Wq/Wk/Wv, 128 rows of Wo) and
processes BOTH batches. Each core emits a [2, 2048, 1024] partial output
(row-parallel Wo); the host sums the 8 partials per batch.

Why: the SPMD program's attention work scales with KT0+KT1 (per 2 heads)
instead of 4*max(KT0,KT1) (per 4 heads) under the old batch x head-group
split, so key-length imbalance between the two batches no longer inflates
the program's critical path.

Key-side truncation: only ceil(valid_len/128) key tiles per batch are
computed; the per-batch mask rides the ScalarE exp as a per-partition
bias. Scores are computed transposed ([key, query]); the softmax
denominator comes free via 64 ones-columns appended to V (ones-trick).

Precision: fp16 streams/weights, fp32 PSUM accumulation (rel err ~8e-4).

Schedule (single instruction stream, deeply interleaved):
- Q streams in column chunks; attention runs per (512-query-chunk, head)
  so the first scores start after only the first Q column pass.
- Batch B's K/V/Q projections run as single-PSUM-bank steps interleaved
  into batch A's attention cadence; batch A's output projection rides
  batch B's attention; batch B's output projection is the only tail.
- PSUM: scores rotation 4x[128,512] + ctx accumulators 2 + shared aux 2
  (projection steps / out-proj) = 8 banks, one configuration end to end.
- PSUM evacuation is split across DVE and ACT (GPSIMD cannot touch PSUM);
  out-DMAs fire per query-tile pair; PE p-state is kept warm by filler
  matmuls during the initial DMA latency.

The program is built at call time from the actual valid_lens (cached by
(KTA, KTB)); batch A is the one with more key tiles and is processed
first so its longer attention phase starts as early as possible.
"""
import sys
if "/opt/trn_rl_repo" not in sys.path:
    sys.path.insert(0, "/opt/trn_rl_repo")
import os
import time
import numpy as np

B, SQ, SK, D, H, HD = 2, 2048, 2048, 1024, 16, 64
NEG = -1.0e6
N_CORES = 8
DL = 128          # d_local: 2 heads * 64
KD = D // 128     # contraction tiles over D

_NC_CACHE = {}
last_results = None
last_exec_wall_s = None


def _build(KTA, KTB):
    import concourse.bass as bass  # noqa: F401
    import concourse.tile as tile
    from concourse import bacc, mybir

    f32 = mybir.dt.float32
    f16 = mybir.dt.float16
    EXP = mybir.ActivationFunctionType.Exp

    LKA, LKB = KTA * 128, KTB * 128
    # [(k0, nk)] chunk groups for the k/v/weight streams (fewer, larger DMAs)
    kgrp = [(0, 4), (4, 4)]

    nc = bacc.Bacc("TRN2", target_bir_lowering=False, debug=False,
                   num_devices=N_CORES)
    xqT = nc.dram_tensor("xqT", [2, D, SQ], f16, kind="ExternalInput")
    xkTA = nc.dram_tensor("xkTA", [D, LKA], f16, kind="ExternalInput")
    xvTA = nc.dram_tensor("xvTA", [D, LKA], f16, kind="ExternalInput")
    xkTB = nc.dram_tensor("xkTB", [D, LKB], f16, kind="ExternalInput")
    xvTB = nc.dram_tensor("xvTB", [D, LKB], f16, kind="ExternalInput")
    wqkv = nc.dram_tensor("wqkv", [D, 3 * DL], f16, kind="ExternalInput")
    wo = nc.dram_tensor("wo", [DL, D], f16, kind="ExternalInput")
    maskA = nc.dram_tensor("maskA", [128, KTA], f32, kind="ExternalInput")
    maskB = nc.dram_tensor("maskB", [128, KTB], f32, kind="ExternalInput")
    out = nc.dram_tensor("out", [2, SQ, D], f16, kind="ExternalOutput")

    with tile.TileContext(nc) as tc:
        with tc.tile_pool(name="singles", bufs=1) as sg:
            wqkv_sb = sg.tile([128, KD, 3 * DL], f16)
            wo_sb = sg.tile([128, D], f16)
            maskA_sb = sg.tile([128, KTA], f32)
            maskB_sb = sg.tile([128, KTB], f32)
            kt_sb = {0: sg.tile([128, LKA], f16, name="ktA"),
                     1: sg.tile([128, LKB], f16, name="ktB")}
            qt_sb = {0: sg.tile([128, SQ], f16, name="qtA"),
                     1: sg.tile([128, SQ], f16, name="qtB")}
            v_sb = {0: sg.tile([128, KTA, 2, 128], f16, name="vA"),
                    1: sg.tile([128, KTB, 2, 128], f16, name="vB")}
            ctx_sb = {0: sg.tile([128, SQ], f16, name="ctxA"),
                      1: sg.tile([128, SQ], f16, name="ctxB")}
            warm_sb = sg.tile([128, 256], f16)

            KT = {0: KTA, 1: KTB}
            mask_sb = {0: maskA_sb, 1: maskB_sb}
            xkT = {0: xkTA, 1: xkTB}
            xvT = {0: xvTA, 1: xvTB}

            # V'' ones-columns (softmax denominator); dim columns are
            # overwritten by the V-projection evacuations below.
            nc.gpsimd.memset(v_sb[0], 1.0)
            nc.gpsimd.memset(v_sb[1], 1.0)
            nc.vector.memset(warm_sb, 0.0)

            # ---- input DMAs, arrival order = need order ----
            nc.sync.dma_start(
                out=wqkv_sb[:, 0:4, :],
                in_=wqkv[0:512, :].rearrange("(k p) j -> p k j", p=128))
            strm_cm = tc.tile_pool(name="streams", bufs=1)
            strm = strm_cm.__enter__()
            xk = {b: strm.tile([128, KD, 128 * KT[b]], f16, name=f"xk{b}")
                  for b in (0, 1)}
            xq = {b: strm.tile([128, KD, SQ], f16, name=f"xq{b}")
                  for b in (0, 1)}
            xv = {b: strm.tile([128, KD, 128 * KT[b]], f16, name=f"xv{b}")
                  for b in (0, 1)}
            def dma_kv(b, which):
                src = xkT[b] if which == "k" else xvT[b]
                dst = xk[b] if which == "k" else xv[b]
                for k0, nk in kgrp:
                    nc.sync.dma_start(
                        out=dst[:, k0:k0 + nk, :],
                        in_=src[k0 * 128:(k0 + nk) * 128, :]
                        .rearrange("(k p) j -> p k j", p=128))

            def dma_q(b):
                for k in range(KD):
                    nc.sync.dma_start(out=xq[b][:, k, :],
                                      in_=xqT[b, k * 128:(k + 1) * 128, :])

            def dma_q_cols(b):
                # column-chunk order: Q-projection pass ci becomes ready
                # as soon as chunk ci lands (contraction needs all k)
                for ci in range(4):
                    nc.sync.dma_start(
                        out=xq[b][:, :, ci * 512:(ci + 1) * 512],
                        in_=xqT[b, :, ci * 512:(ci + 1) * 512]
                        .rearrange("(k p) j -> p k j", p=128))

            dma_kv(0, "k")
            nc.sync.dma_start(
                out=wqkv_sb[:, 4:8, :],
                in_=wqkv[512:1024, :].rearrange("(k p) j -> p k j", p=128))
            nc.sync.dma_start(out=maskA_sb, in_=maskA[:, :])
            nc.sync.dma_start(out=maskB_sb, in_=maskB[:, :])
            dma_kv(0, "v")   # xv before xq: ctx never stalls on V''
            dma_q_cols(0)
            dma_kv(1, "k")
            dma_kv(1, "v")
            dma_q_cols(1)
            nc.sync.dma_start(out=wo_sb, in_=wo[:, :])

            # ---- PE p-state warmup: keep the tensor engine busy during the
            # initial DMA latency so real matmuls start at full clock; the
            # pool stays open so stream-gated phases can emit filler too ----
            psW_cm = tc.tile_pool(name="psW", bufs=1, space="PSUM")
            psW = psW_cm.__enter__()
            wp = psW.tile([128, 256], f32)

            def wfill(n):
                for _ in range(n):
                    nc.tensor.matmul(wp, warm_sb[:, 0:128], warm_sb,
                                     start=True, stop=True)

            wfill(22)

            def copy_eng(eng, dst, src):
                if eng == "v":
                    nc.vector.tensor_copy(dst, src)
                elif eng == "a":
                    nc.scalar.copy(dst, src)
                else:
                    nc.gpsimd.tensor_copy(dst, src)

            def proj_k(b, eng):
                # K^T[b] = Wk^T @ Xk^T : [128, LK_b]
                LK = 128 * KT[b]
                chunks = [(i * 512, min(512, LK - i * 512))
                          for i in range((LK + 511) // 512)]
                with tc.tile_pool(name=f"psA{b}", bufs=1, space="PSUM") as ps:
                    accs = [ps.tile([128, cw], f32, tag=f"kt{ci}",
                                    name=f"kt{b}_{ci}")
                            for ci, (c0, cw) in enumerate(chunks)]
                    for k in range(KD):
                        for ci, (c0, cw) in enumerate(chunks):
                            nc.tensor.matmul(accs[ci],
                                             wqkv_sb[:, k, 0:128],
                                             xk[b][:, k, c0:c0 + cw],
                                             start=(k == 0), stop=(k == KD - 1))
                    for ci, (c0, cw) in enumerate(chunks):
                        copy_eng(eng[ci % len(eng)],
                                 kt_sb[b][:, c0:c0 + cw], accs[ci])

            def proj_q(b, eng, wfill=None):
                # Q^T[b] = Wq^T @ Xq^T : [128, 2048]. wfill emits idle
                # matmuls between DMA-gated k-chunks to hold the PE p-state.
                with tc.tile_pool(name=f"psB{b}", bufs=1, space="PSUM") as ps:
                    accs = [ps.tile([128, 512], f32, tag=f"q{ci}",
                                    name=f"q{b}_{ci}") for ci in range(4)]
                    for k in range(KD):
                        for ci in range(4):
                            nc.tensor.matmul(accs[ci],
                                             wqkv_sb[:, k, 128:256],
                                             xq[b][:, k, ci * 512:(ci + 1) * 512],
                                             start=(k == 0), stop=(k == KD - 1))
                        if wfill is not None and k < KD - 1:
                            wfill(2)
                    for ci in range(4):
                        copy_eng(eng[ci % len(eng)],
                                 qt_sb[b][:, ci * 512:(ci + 1) * 512], accs[ci])

            def proj_v(b, eng):
                # V''[b] : [key, head, dim|ones]; one pool, per-group tags,
                # so a later group never waits an earlier group's evacs
                gs = [list(range(g0, min(g0 + 4, KT[b])))
                      for g0 in range(0, KT[b], 4)]
                if KT[b] > 7:   # bank budget: fall back to serial groups
                    gs = [[t] for t in range(KT[b])]
                with tc.tile_pool(name=f"psV{b}", bufs=1, space="PSUM") as ps:
                    for gi, g in enumerate(gs):
                        tag = f"vg{gi % 4}" if KT[b] > 7 else f"vg{gi}"
                        vacc = ps.tile([128, len(g), 512], f32, tag=tag,
                                       name=f"v{b}_{gi}")
                        for k in range(KD):
                            for vi, t in enumerate(g):
                                nc.tensor.matmul(
                                    vacc[:, vi, 0:DL],
                                    xv[b][:, k, t * 128:(t + 1) * 128],
                                    wqkv_sb[:, k, 256:384],
                                    start=(k == 0), stop=(k == KD - 1),
                                    skip_group_check=True)
                        for vi, t in enumerate(g):
                            for hh in range(2):
                                copy_eng(eng[(2 * vi + hh) % len(eng)],
                                         v_sb[b][:, t, hh, 0:64],
                                         vacc[:, vi, hh * 64:(hh + 1) * 64])

            def attend(b, pools, extra=None):
                # scores^T -> exp -> ctx'' per (head, 512-query-chunk,
                # key-tile), chunk-major so chunk c needs only Q column
                # pass c; ctx accumulates in a [128,512] 1-bank tile,
                # normalized per chunk. extra(i) interleaves foreign work.
                ptp, mp, psS, psC = pools
                nit = 0
                for cq in range(4):
                    for hh in range(2):
                        q0 = cq * 512
                        ctx_ps = psC.tile([128, 512], f32, tag="ctx",
                                          name=f"ctx{b}_{hh}_{cq}")
                        for t in range(KT[b]):
                            s_ps = psS.tile([128, 512], f32, tag="s")
                            nc.tensor.matmul(
                                s_ps,
                                kt_sb[b][hh * 64:hh * 64 + 64,
                                         t * 128:(t + 1) * 128],
                                qt_sb[b][hh * 64:hh * 64 + 64,
                                         q0:q0 + 512],
                                start=True, stop=True)
                            pt = ptp.tile([128, 512], f16, tag="pt")
                            nc.scalar.activation(
                                pt, s_ps, EXP,
                                bias=mask_sb[b][:, t:t + 1], scale=0.125)
                            nc.tensor.matmul(
                                ctx_ps, v_sb[b][:, t, hh, :], pt,
                                start=(t == 0), stop=(t == KT[b] - 1),
                                skip_group_check=True)
                            if extra is not None:
                                extra(nit)
                            nit += 1
                        # rows 64-127 all hold the softmax denominator
                        # (only one TensorTensor input may come from PSUM,
                        # so reciprocal lands in SBUF first)
                        rcb = mp.tile([64, 512], f32, tag="rcb")
                        nc.vector.reciprocal(rcb, ctx_ps[64:128, :])
                        nc.vector.tensor_mul(
                            ctx_sb[b][hh * 64:hh * 64 + 64, q0:q0 + 512],
                            ctx_ps[0:64, :], rcb)

            evac_ct = [0]
            osb_map = {0: {}, 1: {}}

            def out_proj(b, pools, qr, engines):
                # partial out[b] rows = ctx''[b]^T @ Wo_local; [128,512]
                # PSUM grain; o_sb tiles live per query-tile PAIR (the DMA
                # after the odd qi reads both halves)
                psD, op = pools
                for qi in qr:
                    pb = qi // 2
                    if pb not in osb_map[b]:
                        osb_map[b][pb] = op.tile(
                            [128, 2, D], f16, tag=f"o{pb % 2}",
                            name=f"osb{b}_{pb}")
                    o_sb = osb_map[b][pb]
                    for n in range(2):
                        o_ps = psD.tile([128, 512], f32, tag="x",
                                        name=f"o{b}_{qi}_{n}")
                        nc.tensor.matmul(o_ps,
                                         ctx_sb[b][:, qi * 128:(qi + 1) * 128],
                                         wo_sb[:, n * 512:(n + 1) * 512],
                                         start=True, stop=True)
                        eng = engines[evac_ct[0] % len(engines)]
                        evac_ct[0] += 1
                        copy_eng(eng, o_sb[:, qi % 2, n * 512:(n + 1) * 512],
                                 o_ps)
                    if qi % 2 == 1:
                        nc.sync.dma_start(
                            out=out[b, (qi - 1) * 128:(qi + 1) * 128, :]
                            .rearrange("(c p) j -> p c j", p=128),
                            in_=o_sb)

            # ---- batch A K/V projections ride the early streams; Q runs
            # as column passes so attention starts while Q still streams ----
            proj_k(0, ("v",))
            proj_v(0, ("v",))
            psW_cm.__exit__(None, None, None)

            # batch-B projections as single-PSUM-bank steps, interleaved
            # into batch-A's ACT-bound attention cadence (all on GPSIMD so
            # nothing queues behind DVE norms)
            def bsteps(psX):
                LKB_ = 128 * KT[1]

                def a1b_step(c0, cw):
                    acc = psX.tile([128, 512], f32, tag="x", name="xa")
                    for k in range(KD):
                        nc.tensor.matmul(acc[:, 0:cw], wqkv_sb[:, k, 0:128],
                                         xk[1][:, k, c0:c0 + cw],
                                         start=(k == 0), stop=(k == KD - 1))
                    copy_eng("v", kt_sb[1][:, c0:c0 + cw], acc[:, 0:cw])

                def vb_step(t):
                    acc = psX.tile([128, 512], f32, tag="x", name="xv")
                    for k in range(KD):
                        nc.tensor.matmul(acc[:, 0:DL],
                                         xv[1][:, k, t * 128:(t + 1) * 128],
                                         wqkv_sb[:, k, 256:384],
                                         start=(k == 0), stop=(k == KD - 1))
                    for hh in range(2):
                        copy_eng("v", v_sb[1][:, t, hh, 0:64],
                                 acc[:, hh * 64:(hh + 1) * 64])

                def qb_step(ci, b=1):
                    acc = psX.tile([128, 512], f32, tag="x", name="xq")
                    for k in range(KD):
                        nc.tensor.matmul(acc, wqkv_sb[:, k, 128:256],
                                         xq[b][:, k, ci * 512:(ci + 1) * 512],
                                         start=(k == 0), stop=(k == KD - 1))
                    copy_eng("v", qt_sb[b][:, ci * 512:(ci + 1) * 512], acc)

                steps = []
                for i in range((LKB_ + 511) // 512):
                    c0 = i * 512
                    steps.append(lambda c0=c0, cw=min(512, LKB_ - c0):
                                 a1b_step(c0, cw))
                steps.extend(lambda t=t: vb_step(t) for t in range(KT[1]))
                steps.extend(lambda ci=ci: qb_step(ci) for ci in range(4))
                return steps, qb_step

            # One continuous PSUM configuration from first attention to last
            # output tile: psS (2 banks, score rotation) + psC (4 banks, ctx
            # accumulator) + aux (2 banks, shared rotation for batch-B
            # projection steps, then both batches' out-projection PSUM).
            with tc.tile_pool(name="pt", bufs=6) as ptp, \
                 tc.tile_pool(name="misc", bufs=2) as mp, \
                 tc.tile_pool(name="ob", bufs=6) as op:
                with tc.tile_pool(name="psS", bufs=4, space="PSUM") as psS, \
                     tc.tile_pool(name="psC", bufs=2, space="PSUM") as psC, \
                     tc.tile_pool(name="aux", bufs=2, space="PSUM") as aux:
                    steps, qa_step = bsteps(aux)
                    # Batch A's Q column pass 0 gates the first group;
                    # passes 1-3 interleave ahead of the chunks that need
                    # them, tracking the xq column-DMA arrivals. Batch-B
                    # projection steps ride attend(0)'s second half.
                    qa_step(0, b=0)
                    nit_A = 8 * KT[0]
                    smap = {}
                    for i, frac in ((1, 0.15), (2, 0.375), (3, 0.55)):
                        it = max(i, int(nit_A * frac))
                        smap.setdefault(it, []).append(
                            lambda ci=i: qa_step(ci, b=0))
                    for j, s in enumerate(steps):
                        it = max(4, int(nit_A * (0.62 + 0.33 * j / len(steps))))
                        smap.setdefault(it, []).append(s)

                    # a small slice of batch A's out-projection (qi 0-3,
                    # query chunk 0, normalized after group 4) rides the tail
                    # of attend(0) so its out-DMAs start during the DMA lull
                    qmapA = {}

                    def extraA(i):
                        for s in smap.get(i, ()):
                            s()
                        if i in qmapA:
                            out_proj(0, (aux, op), qmapA[i], ("a", "v"))

                    attend(0, (ptp, mp, psS, psC), extra=extraA)
                    for it, fns in sorted(smap.items()):
                        if it >= nit_A:
                            for s in fns:
                                s()
                    done_A = sorted(q for qs in qmapA.values() for q in qs)
                    rest_A = [q for q in range(16) if q not in done_A]
                    # batch B attention carries the rest of batch A's
                    # out-projection, spread across its cadence
                    nb = 8 * KT[1]
                    qsched = {}
                    ns = max(1, nb - 1)
                    nr = len(rest_A)
                    for i in range(ns):
                        # later slots lean on ACT: its exps wind down while
                        # DVE still carries the trailing norms
                        eng = ("v", "v", "a") if i < ns // 2 else ("a", "v")
                        qsched[i] = (rest_A[nr * i // ns: nr * (i + 1) // ns],
                                     eng)

                    def extra(i):
                        if i in qsched:
                            qr, eng = qsched[i]
                            out_proj(0, (aux, op), qr, eng)

                    attend(1, (ptp, mp, psS, psC), extra=extra)
                # final out-projection in its own deep PSUM rotation so the
                # tail runs at the out-DMA rate, not the evacuation rate
                    # first tail tiles run from the still-open aux pool so
                    # PE flows into the tail while psD2's banks hand over
                    out_proj(1, (aux, op), range(0, 2), ("a", "v"))
                with tc.tile_pool(name="psD2", bufs=6, space="PSUM") as psD2:
                    out_proj(1, (psD2, op), range(2, 16), ("a", "v"))
            strm_cm.__exit__(None, None, None)
    nc.compile()
    return nc


def kernel(**inputs):
    global last_results, last_exec_wall_s
    from concourse.bass_utils import run_bass_kernel_spmd

    # BASS_TRACE needs the axon NTFF hook; disable tracing when the hook
    # module is unavailable so a stray env var cannot crash the run.
    if os.environ.get("BASS_TRACE"):
        try:
            from antenv import axon_hooks  # noqa: F401
        except Exception:
            os.environ["BASS_NEVER_TRACE"] = "1"

    q = np.asarray(inputs["queries"], dtype=np.float32)
    kx = np.asarray(inputs["keys"], dtype=np.float32)
    vx = np.asarray(inputs["values"], dtype=np.float32)
    vl = np.asarray(inputs["valid_lens"], dtype=np.int64).reshape(B)
    Wq = np.asarray(inputs["Wq"], dtype=np.float32)
    Wk = np.asarray(inputs["Wk"], dtype=np.float32)
    Wv = np.asarray(inputs["Wv"], dtype=np.float32)
    Wo = np.asarray(inputs["Wo"], dtype=np.float32)
    assert q.shape == (B, SQ, D) and kx.shape == (B, SK, D) and vx.shape == (B, SK, D)

    lens = np.clip(vl, 1, SK)
    KTs = [(int(l) + 127) // 128 for l in lens]
    # batch A = more key tiles, processed first
    bA = 0 if KTs[0] >= KTs[1] else 1
    bB = 1 - bA
    KTA, KTB = KTs[bA], KTs[bB]
    LKA, LKB = KTA * 128, KTB * 128

    key = (KTA, KTB)
    if key not in _NC_CACHE:
        _NC_CACHE[key] = _build(KTA, KTB)
    nc = _NC_CACHE[key]

    def m128(b, KT):
        m = np.where(np.arange(KT * 128) < lens[b], 0.0, NEG).astype(np.float32)
        return np.ascontiguousarray(m.reshape(KT, 128).T)

    xqT_full = np.ascontiguousarray(
        np.stack([q[bA].T, q[bB].T]).astype(np.float16))
    in_maps = []
    for c in range(N_CORES):
        cols = slice(DL * c, DL * (c + 1))
        in_maps.append({
            "xqT": xqT_full,
            "xkTA": np.ascontiguousarray(kx[bA, :LKA].T.astype(np.float16)),
            "xvTA": np.ascontiguousarray(vx[bA, :LKA].T.astype(np.float16)),
            "xkTB": np.ascontiguousarray(kx[bB, :LKB].T.astype(np.float16)),
            "xvTB": np.ascontiguousarray(vx[bB, :LKB].T.astype(np.float16)),
            "wqkv": np.ascontiguousarray(np.concatenate(
                [Wk[:, cols], Wq[:, cols], Wv[:, cols]],
                axis=1).astype(np.float16)),
            "wo": np.ascontiguousarray(Wo[cols, :].astype(np.float16)),
            "maskA": m128(bA, KTA),
            "maskB": m128(bB, KTB),
        })

    t0 = time.perf_counter()
    res = run_bass_kernel_spmd(nc, in_maps, core_ids=list(range(N_CORES)))
    last_exec_wall_s = time.perf_counter() - t0
    last_results = res

    outs = [res.results[c]["out"].astype(np.float32) for c in range(N_CORES)]
    acc = outs[0]
    for c in range(1, N_CORES):
        acc = acc + outs[c]
    full = np.empty((B, SQ, D), dtype=np.float32)
    full[bA] = acc[0]
    full[bB] = acc[1]
    return full


# revision 88
# speedup vs baseline: 1.1626x; 1.0078x over previous
"""Multi-head attention (B=2, S=2048, D=1024, H=16) on 8 Trainium2 cores.

Sharding: pure tensor-parallel over heads (Megatron): core c owns heads
{2c, 2c+1} (d_local = 128 columns of Wq/Wk/Wv, 128 rows of Wo) and
processes BOTH batches. Each core emits a [2, 2048, 1024] partial output
(row-parallel Wo); the host sums the 8 partials per batch.

Why: the SPMD program's attention work scales with KT0+KT1 (per 2 heads)
instead of 4*max(KT0,KT1) (per 4 heads) under the old batch x head-group
split, so key-length imbalance between the two batches no longer inflates
the program's critical path.

Key-side truncation: only ceil(valid_len/128) key tiles per batch are
computed; the per-batch mask rides the ScalarE exp as a per-partition
bias. Scores are computed transposed ([key, query]); the softmax
denominator comes free via 64 ones-columns appended to V (ones-trick).

Precision: fp16 streams/weights, fp32 PSUM accumulation (rel err ~8e-4).

Schedule (single instruction stream, deeply interleaved):
- Q streams in column chunks; attention runs per (512-query-chunk, head)
  so the first scores start after only the first Q column pass.
- Batch B's K/V/Q projections run as single-PSUM-bank steps interleaved
  into batch A's attention cadence; batch A's output projection rides
  batch B's attention; batch B's output projection is the only tail.
- PSUM: scores rotation 4x[128,512] + ctx accumulators 2 + shared aux 2
  (projection steps / out-proj) = 8 banks, one configuration end to end.
- PSUM evacuation is split across DVE and ACT (GPSIMD cannot touch PSUM);
  out-DMAs fire per query-tile pair; PE p-state is kept warm by filler
  matmuls during the initial DMA latency.

The program is built at call time from the actual valid_lens (cached by
(KTA, KTB)); batch A is the one with more key tiles and is processed
first so its longer attention phase starts as early as possible.
"""
import sys
if "/opt/trn_rl_repo" not in sys.path:
    sys.path.insert(0, "/opt/trn_rl_repo")
import os
import time
import numpy as np

B, SQ, SK, D, H, HD = 2, 2048, 2048, 1024, 16, 64
NEG = -1.0e6
N_CORES = 8
DL = 128          # d_local: 2 heads * 64
KD = D // 128     # contraction tiles over D

_NC_CACHE = {}
last_results = None
last_exec_wall_s = None


def _build(KTA, KTB):
    import concourse.bass as bass  # noqa: F401
    import concourse.tile as tile
    from concourse import bacc, mybir

    f32 = mybir.dt.float32
    f16 = mybir.dt.float16
    EXP = mybir.ActivationFunctionType.Exp

    LKA, LKB = KTA * 128, KTB * 128
    # [(k0, nk)] chunk groups for the k/v/weight streams (fewer, larger DMAs)
    kgrp = [(0, 4), (4, 4)]

    nc = bacc.Bacc("TRN2", target_bir_lowering=False, debug=False,
                   num_devices=N_CORES)
    xqT = nc.dram_tensor("xqT", [2, D, SQ], f16, kind="ExternalInput")
    xkTA = nc.dram_tensor("xkTA", [D, LKA], f16, kind="ExternalInput")
    xvTA = nc.dram_tensor("xvTA", [D, LKA], f16, kind="ExternalInput")
    xkTB = nc.dram_tensor("xkTB", [D, LKB], f16, kind="ExternalInput")
    xvTB = nc.dram_tensor("xvTB", [D, LKB], f16, kind="ExternalInput")
    wqkv = nc.dram_tensor("wqkv", [D, 3 * DL], f16, kind="ExternalInput")
    wo = nc.dram_tensor("wo", [DL, D], f16, kind="ExternalInput")
    maskA = nc.dram_tensor("maskA", [128, KTA], f32, kind="ExternalInput")
    maskB = nc.dram_tensor("maskB", [128, KTB], f32, kind="ExternalInput")
    out = nc.dram_tensor("out", [2, SQ, D], f16, kind="ExternalOutput")

    with tile.TileContext(nc) as tc:
        with tc.tile_pool(name="singles", bufs=1) as sg:
            wqkv_sb = sg.tile([128, KD, 3 * DL], f16)
            wo_sb = sg.tile([128, D], f16)
            maskA_sb = sg.tile([128, KTA], f32)
            maskB_sb = sg.tile([128, KTB], f32)
            kt_sb = {0: sg.tile([128, LKA], f16, name="ktA"),
                     1: sg.tile([128, LKB], f16, name="ktB")}
            qt_sb = {0: sg.tile([128, SQ], f16, name="qtA"),
                     1: sg.tile([128, SQ], f16, name="qtB")}
            v_sb = {0: sg.tile([128, KTA, 2, 128], f16, name="vA"),
                    1: sg.tile([128, KTB, 2, 128], f16, name="vB")}
            ctx_sb = {0: sg.tile([128, SQ], f16, name="ctxA"),
                      1: sg.tile([128, SQ], f16, name="ctxB")}
            warm_sb = sg.tile([128, 256], f16)

            KT = {0: KTA, 1: KTB}
            mask_sb = {0: maskA_sb, 1: maskB_sb}
            xkT = {0: xkTA, 1: xkTB}
            xvT = {0: xvTA, 1: xvTB}

            # V'' ones-columns (softmax denominator); dim columns are
            # overwritten by the V-projection evacuations below.
            nc.gpsimd.memset(v_sb[0], 1.0)
            nc.gpsimd.memset(v_sb[1], 1.0)
            nc.vector.memset(warm_sb, 0.0)

            # ---- input DMAs, arrival order = need order ----
            nc.sync.dma_start(
                out=wqkv_sb[:, 0:4, :],
                in_=wqkv[0:512, :].rearrange("(k p) j -> p k j", p=128))
            strm_cm = tc.tile_pool(name="streams", bufs=1)
            strm = strm_cm.__enter__()
            xk = {b: strm.tile([128, KD, 128 * KT[b]], f16, name=f"xk{b}")
                  for b in (0, 1)}
            xq = {b: strm.tile([128, KD, SQ], f16, name=f"xq{b}")
                  for b in (0, 1)}
            xv = {b: strm.tile([128, KD, 128 * KT[b]], f16, name=f"xv{b}")
                  for b in (0, 1)}
            def dma_kv(b, which):
                src = xkT[b] if which == "k" else xvT[b]
                dst = xk[b] if which == "k" else xv[b]
                for k0, nk in kgrp:
                    nc.sync.dma_start(
                        out=dst[:, k0:k0 + nk, :],
                        in_=src[k0 * 128:(k0 + nk) * 128, :]
                        .rearrange("(k p) j -> p k j", p=128))

            def dma_q(b):
                for k in range(KD):
                    nc.sync.dma_start(out=xq[b][:, k, :],
                                      in_=xqT[b, k * 128:(k + 1) * 128, :])

            def dma_q_cols(b):
                # column-chunk order: Q-projection pass ci becomes ready
                # as soon as chunk ci lands (contraction needs all k)
                for ci in range(4):
                    nc.sync.dma_start(
                        out=xq[b][:, :, ci * 512:(ci + 1) * 512],
                        in_=xqT[b, :, ci * 512:(ci + 1) * 512]
                        .rearrange("(k p) j -> p k j", p=128))

            dma_kv(0, "k")
            nc.sync.dma_start(
                out=wqkv_sb[:, 4:8, :],
                in_=wqkv[512:1024, :].rearrange("(k p) j -> p k j", p=128))
            nc.sync.dma_start(out=maskA_sb, in_=maskA[:, :])
            nc.sync.dma_start(out=maskB_sb, in_=maskB[:, :])
            dma_kv(0, "v")   # xv before xq: ctx never stalls on V''
            dma_q_cols(0)
            dma_kv(1, "k")
            dma_kv(1, "v")
            dma_q_cols(1)
            nc.sync.dma_start(out=wo_sb, in_=wo[:, :])

            # ---- PE p-state warmup: keep the tensor engine busy during the
            # initial DMA latency so real matmuls start at full clock; the
            # pool stays open so stream-gated phases can emit filler too ----
            psW_cm = tc.tile_pool(name="psW", bufs=1, space="PSUM")
            psW = psW_cm.__enter__()
            wp = psW.tile([128, 256], f32)

            def wfill(n):
                for _ in range(n):
                    nc.tensor.matmul(wp, warm_sb[:, 0:128], warm_sb,
                                     start=True, stop=True)

            wfill(22)

            def copy_eng(eng, dst, src):
                if eng == "v":
                    nc.vector.tensor_copy(dst, src)
                elif eng == "a":
                    nc.scalar.copy(dst, src)
                else:
                    nc.gpsimd.tensor_copy(dst, src)

            def proj_k(b, eng):
                # K^T[b] = Wk^T @ Xk^T : [128, LK_b]
                LK = 128 * KT[b]
                chunks = [(i * 512, min(512, LK - i * 512))
                          for i in range((LK + 511) // 512)]
                with tc.tile_pool(name=f"psA{b}", bufs=1, space="PSUM") as ps:
                    accs = [ps.tile([128, cw], f32, tag=f"kt{ci}",
                                    name=f"kt{b}_{ci}")
                            for ci, (c0, cw) in enumerate(chunks)]
                    for k in range(KD):
                        for ci, (c0, cw) in enumerate(chunks):
                            nc.tensor.matmul(accs[ci],
                                             wqkv_sb[:, k, 0:128],
                                             xk[b][:, k, c0:c0 + cw],
                                             start=(k == 0), stop=(k == KD - 1))
                    for ci, (c0, cw) in enumerate(chunks):
                        copy_eng(eng[ci % len(eng)],
                                 kt_sb[b][:, c0:c0 + cw], accs[ci])

            def proj_q(b, eng, wfill=None):
                # Q^T[b] = Wq^T @ Xq^T : [128, 2048]. wfill emits idle
                # matmuls between DMA-gated k-chunks to hold the PE p-state.
                with tc.tile_pool(name=f"psB{b}", bufs=1, space="PSUM") as ps:
                    accs = [ps.tile([128, 512], f32, tag=f"q{ci}",
                                    name=f"q{b}_{ci}") for ci in range(4)]
                    for k in range(KD):
                        for ci in range(4):
                            nc.tensor.matmul(accs[ci],
                                             wqkv_sb[:, k, 128:256],
                                             xq[b][:, k, ci * 512:(ci + 1) * 512],
                                             start=(k == 0), stop=(k == KD - 1))
                        if wfill is not None and k < KD - 1:
                            wfill(2)
                    for ci in range(4):
                        copy_eng(eng[ci % len(eng)],
                                 qt_sb[b][:, ci * 512:(ci + 1) * 512], accs[ci])

            def proj_v(b, eng):
                # V''[b] : [key, head, dim|ones]; one pool, per-group tags,
                # so a later group never waits an earlier group's evacs
                gs = [list(range(g0, min(g0 + 4, KT[b])))
                      for g0 in range(0, KT[b], 4)]
                if KT[b] > 7:   # bank budget: fall back to serial groups
                    gs = [[t] for t in range(KT[b])]
                with tc.tile_pool(name=f"psV{b}", bufs=1, space="PSUM") as ps:
                    for gi, g in enumerate(gs):
                        tag = f"vg{gi % 4}" if KT[b] > 7 else f"vg{gi}"
                        vacc = ps.tile([128, len(g), 512], f32, tag=tag,
                                       name=f"v{b}_{gi}")
                        for k in range(KD):
                            for vi, t in enumerate(g):
                                nc.tensor.matmul(
                                    vacc[:, vi, 0:DL],
                                    xv[b][:, k, t * 128:(t + 1) * 128],
                                    wqkv_sb[:, k, 256:384],
                                    start=(k == 0), stop=(k == KD - 1),
                                    skip_group_check=True)
                        for vi, t in enumerate(g):
                            for hh in range(2):
                                copy_eng(eng[(2 * vi + hh) % len(eng)],
                                         v_sb[b][:, t, hh, 0:64],
                                         vacc[:, vi, hh * 64:(hh + 1) * 64])

            def attend(b, pools, extra=None):
                # scores^T -> exp -> ctx'' per (head, 512-query-chunk,
                # key-tile), chunk-major so chunk c needs only Q column
                # pass c; ctx accumulates in a [128,512] 1-bank tile,
                # normalized per chunk. extra(i) interleaves foreign work.
                ptp, mp, psS, psC = pools
                nit = 0
                for cq in range(4):
                    for hh in range(2):
                        q0 = cq * 512
                        ctx_ps = psC.tile([128, 512], f32, tag="ctx",
                                          name=f"ctx{b}_{hh}_{cq}")
                        for t in range(KT[b]):
                            s_ps = psS.tile([128, 512], f32, tag="s")
                            nc.tensor.matmul(
                                s_ps,
                                kt_sb[b][hh * 64:hh * 64 + 64,
                                         t * 128:(t + 1) * 128],
                                qt_sb[b][hh * 64:hh * 64 + 64,
                                         q0:q0 + 512],
                                start=True, stop=True)
                            pt = ptp.tile([128, 512], f16, tag="pt")
                            nc.scalar.activation(
                                pt, s_ps, EXP,
                                bias=mask_sb[b][:, t:t + 1], scale=0.125)
                            nc.tensor.matmul(
                                ctx_ps, v_sb[b][:, t, hh, :], pt,
                                start=(t == 0), stop=(t == KT[b] - 1),
                                skip_group_check=True)
                            if extra is not None:
                                extra(nit)
                            nit += 1
                        # rows 64-127 all hold the softmax denominator
                        # (only one TensorTensor input may come from PSUM,
                        # so reciprocal lands in SBUF first)
                        rcb = mp.tile([64, 512], f32, tag="rcb")
                        nc.vector.reciprocal(rcb, ctx_ps[64:128, :])
                        nc.vector.tensor_mul(
                            ctx_sb[b][hh * 64:hh * 64 + 64, q0:q0 + 512],
                            ctx_ps[0:64, :], rcb)

            evac_ct = [0]

            def out_proj(b, pools, qr, engines):
                # partial out[b] rows = ctx''[b]^T @ Wo_local; [128,512]
                # PSUM grain; each query tile DMAs out as soon as its own
                # two evacuations land (finer DMA pipelining than pairs)
                psD, op = pools
                for qi in qr:
                    o_sb = op.tile([128, D], f16, tag=f"o{qi % 3}",
                                   name=f"osb{b}_{qi}")
                    for n in range(2):
                        o_ps = psD.tile([128, 512], f32, tag="x",
                                        name=f"o{b}_{qi}_{n}")
                        nc.tensor.matmul(o_ps,
                                         ctx_sb[b][:, qi * 128:(qi + 1) * 128],
                                         wo_sb[:, n * 512:(n + 1) * 512],
                                         start=True, stop=True)
                        eng = engines[evac_ct[0] % len(engines)]
                        evac_ct[0] += 1
                        copy_eng(eng, o_sb[:, n * 512:(n + 1) * 512], o_ps)
                    nc.sync.dma_start(
                        out=out[b, qi * 128:(qi + 1) * 128, :], in_=o_sb)

            # ---- batch A K/V projections ride the early streams; Q runs
            # as column passes so attention starts while Q still streams ----
            proj_k(0, ("v",))
            proj_v(0, ("v",))
            psW_cm.__exit__(None, None, None)

            # batch-B projections as single-PSUM-bank steps, interleaved
            # into batch-A's ACT-bound attention cadence (all on GPSIMD so
            # nothing queues behind DVE norms)
            def bsteps(psX):
                LKB_ = 128 * KT[1]

                def a1b_step(c0, cw):
                    acc = psX.tile([128, 512], f32, tag="x", name="xa")
                    for k in range(KD):
                        nc.tensor.matmul(acc[:, 0:cw], wqkv_sb[:, k, 0:128],
                                         xk[1][:, k, c0:c0 + cw],
                                         start=(k == 0), stop=(k == KD - 1))
                    copy_eng("v", kt_sb[1][:, c0:c0 + cw], acc[:, 0:cw])

                def vb_step(t):
                    acc = psX.tile([128, 512], f32, tag="x", name="xv")
                    for k in range(KD):
                        nc.tensor.matmul(acc[:, 0:DL],
                                         xv[1][:, k, t * 128:(t + 1) * 128],
                                         wqkv_sb[:, k, 256:384],
                                         start=(k == 0), stop=(k == KD - 1))
                    for hh in range(2):
                        copy_eng("v", v_sb[1][:, t, hh, 0:64],
                                 acc[:, hh * 64:(hh + 1) * 64])

                def qb_step(ci, b=1):
                    acc = psX.tile([128, 512], f32, tag="x", name="xq")
                    for k in range(KD):
                        nc.tensor.matmul(acc, wqkv_sb[:, k, 128:256],
                                         xq[b][:, k, ci * 512:(ci + 1) * 512],
                                         start=(k == 0), stop=(k == KD - 1))
                    copy_eng("v", qt_sb[b][:, ci * 512:(ci + 1) * 512], acc)

                steps = []
                for i in range((LKB_ + 511) // 512):
                    c0 = i * 512
                    steps.append(lambda c0=c0, cw=min(512, LKB_ - c0):
                                 a1b_step(c0, cw))
                steps.extend(lambda t=t: vb_step(t) for t in range(KT[1]))
                steps.extend(lambda ci=ci: qb_step(ci) for ci in range(4))
                return steps, qb_step

            # One continuous PSUM configuration from first attention to last
            # output tile: psS (2 banks, score rotation) + psC (4 banks, ctx
            # accumulator) + aux (2 banks, shared rotation for batch-B
            # projection steps, then both batches' out-projection PSUM).
            with tc.tile_pool(name="pt", bufs=6) as ptp, \
                 tc.tile_pool(name="misc", bufs=2) as mp, \
                 tc.tile_pool(name="ob", bufs=6) as op:
                with tc.tile_pool(name="psS", bufs=4, space="PSUM") as psS, \
                     tc.tile_pool(name="psC", bufs=2, space="PSUM") as psC, \
                     tc.tile_pool(name="aux", bufs=2, space="PSUM") as aux:
                    steps, qa_step = bsteps(aux)
                    # Batch A's Q column pass 0 gates the first group;
                    # passes 1-3 interleave ahead of the chunks that need
                    # them, tracking the xq column-DMA arrivals. Batch-B
                    # projection steps ride attend(0)'s second half.
                    qa_step(0, b=0)
                    nit_A = 8 * KT[0]
                    smap = {}
                    for i, frac in ((1, 0.15), (2, 0.375), (3, 0.55)):
                        it = max(i, int(nit_A * frac))
                        smap.setdefault(it, []).append(
                            lambda ci=i: qa_step(ci, b=0))
                    for j, s in enumerate(steps):
                        it = max(4, int(nit_A * (0.62 + 0.33 * j / len(steps))))
                        smap.setdefault(it, []).append(s)

                    # a small slice of batch A's out-projection (qi 0-3,
                    # query chunk 0, normalized after group 4) rides the tail
                    # of attend(0) so its out-DMAs start during the DMA lull
                    qmapA = {}

                    def extraA(i):
                        for s in smap.get(i, ()):
                            s()
                        if i in qmapA:
                            out_proj(0, (aux, op), qmapA[i], ("a", "v"))

                    attend(0, (ptp, mp, psS, psC), extra=extraA)
                    for it, fns in sorted(smap.items()):
                        if it >= nit_A:
                            for s in fns:
                                s()
                    done_A = sorted(q for qs in qmapA.values() for q in qs)
                    rest_A = [q for q in range(16) if q not in done_A]
                    # batch B attention carries the rest of batch A's
                    # out-projection, spread across its cadence
                    nb = 8 * KT[1]
                    qsched = {}
                    ns = max(1, nb - 1)
                    nr = len(rest_A)
                    for i in range(ns):
                        # later slots lean on ACT: its exps wind down while
                        # DVE still carries the trailing norms
                        eng = ("v", "v", "a") if i < ns // 2 else ("a", "v")
                        qsched[i] = (rest_A[nr * i // ns: nr * (i + 1) // ns],
                                     eng)

                    def extra(i):
                        if i in qsched:
                            qr, eng = qsched[i]
                            out_proj(0, (aux, op), qr, eng)

                    attend(1, (ptp, mp, psS, psC), extra=extra)
                # final out-projection in its own deep PSUM rotation so the
                # tail runs at the out-DMA rate, not the evacuation rate
                    # first tail tiles run from the still-open aux pool so
                    # PE flows into the tail while psD2's banks hand over
                    out_proj(1, (aux, op), range(0, 2), ("a", "v"))
                with tc.tile_pool(name="psD2", bufs=6, space="PSUM") as psD2:
                    out_proj(1, (psD2, op), range(2, 16), ("a", "v"))
            strm_cm.__exit__(None, None, None)
    nc.compile()
    return nc


def kernel(**inputs):
    global last_results, last_exec_wall_s
    from concourse.bass_utils import run_bass_kernel_spmd

    # BASS_TRACE needs the axon NTFF hook; disable tracing when the hook
    # module is unavailable so a stray env var cannot crash the run.
    if os.environ.get("BASS_TRACE"):
        try:
            from antenv import axon_hooks  # noqa: F401
        except Exception:
            os.environ["BASS_NEVER_TRACE"] = "1"

    q = np.asarray(inputs["queries"], dtype=np.float32)
    kx = np.asarray(inputs["keys"], dtype=np.float32)
    vx = np.asarray(inputs["values"], dtype=np.float32)
    vl = np.asarray(inputs["valid_lens"], dtype=np.int64).reshape(B)
    Wq = np.asarray(inputs["Wq"], dtype=np.float32)
    Wk = np.asarray(inputs["Wk"], dtype=np.float32)
    Wv = np.asarray(inputs["Wv"], dtype=np.float32)
    Wo = np.asarray(inputs["Wo"], dtype=np.float32)
    assert q.shape == (B, SQ, D) and kx.shape == (B, SK, D) and vx.shape == (B, SK, D)

    lens = np.clip(vl, 1, SK)
    KTs = [(int(l) + 127) // 128 for l in lens]
    # batch A = more key tiles, processed first
    bA = 0 if KTs[0] >= KTs[1] else 1
    bB = 1 - bA
    KTA, KTB = KTs[bA], KTs[bB]
    LKA, LKB = KTA * 128, KTB * 128

    key = (KTA, KTB)
    if key not in _NC_CACHE:
        _NC_CACHE[key] = _build(KTA, KTB)
    nc = _NC_CACHE[key]

    def m128(b, KT):
        m = np.where(np.arange(KT * 128) < lens[b], 0.0, NEG).astype(np.float32)
        return np.ascontiguousarray(m.reshape(KT, 128).T)

    xqT_full = np.ascontiguousarray(
        np.stack([q[bA].T, q[bB].T]).astype(np.float16))
    in_maps = []
    for c in range(N_CORES):
        cols = slice(DL * c, DL * (c + 1))
        in_maps.append({
            "xqT": xqT_full,
            "xkTA": np.ascontiguousarray(kx[bA, :LKA].T.astype(np.float16)),
            "xvTA": np.ascontiguousarray(vx[bA, :LKA].T.astype(np.float16)),
            "xkTB": np.ascontiguousarray(kx[bB, :LKB].T.astype(np.float16)),
            "xvTB": np.ascontiguousarray(vx[bB, :LKB].T.astype(np.float16)),
            "wqkv": np.ascontiguousarray(np.concatenate(
                [Wk[:, cols], Wq[:, cols], Wv[:, cols]],
                axis=1).astype(np.float16)),
            "wo": np.ascontiguousarray(Wo[cols, :].astype(np.float16)),
            "maskA": m128(bA, KTA),
            "maskB": m128(bB, KTB),
        })

    t0 = time.perf_counter()
    res = run_bass_kernel_spmd(nc, in_maps, core_ids=list(range(N_CORES)))
    last_exec_wall_s = time.perf_counter() - t0
    last_results = res

    outs = [res.results[c]["out"].astype(np.float32) for c in range(N_CORES)]
    acc = outs[0]
    for c in range(1, N_CORES):
        acc = acc + outs[c]
    full = np.empty((B, SQ, D), dtype=np.float32)
    full[bA] = acc[0]
    full[bB] = acc[1]
    return full


# revision 91
# speedup vs baseline: 1.1720x; 1.0081x over previous
"""Multi-head attention (B=2, S=2048, D=1024, H=16) on 8 Trainium2 cores.

Sharding: pure tensor-parallel over heads (Megatron): core c owns heads
{2c, 2c+1} (d_local = 128 columns of Wq/Wk/Wv, 128 rows of Wo) and
processes BOTH batches. Each core emits a [2, 2048, 1024] partial output
(row-parallel Wo); the host sums the 8 partials per batch.

Why: the SPMD program's attention work scales with KT0+KT1 (per 2 heads)
instead of 4*max(KT0,KT1) (per 4 heads) under the old batch x head-group
split, so key-length imbalance between the two batches no longer inflates
the program's critical path.

Key-side truncation: only ceil(valid_len/128) key tiles per batch are
computed; the per-batch mask rides the ScalarE exp as a per-partition
bias. Scores are computed transposed ([key, query]); the softmax
denominator comes free via 64 ones-columns appended to V (ones-trick).

Precision: fp16 streams/weights, fp32 PSUM accumulation (rel err ~8e-4).

Schedule (single instruction stream, deeply interleaved):
- Q streams in column chunks; attention runs per (512-query-chunk, head)
  so the first scores start after only the first Q column pass.
- Batch B's K/V/Q projections run as single-PSUM-bank steps interleaved
  into batch A's attention cadence; batch A's output projection rides
  batch B's attention; batch B's output projection is the only tail.
- PSUM: scores rotation 4x[128,512] + ctx accumulators 2 + shared aux 2
  (projection steps / out-proj) = 8 banks, one configuration end to end.
- PSUM evacuation is split across DVE and ACT (GPSIMD cannot touch PSUM);
  out-DMAs fire per query tile; PE p-state is kept warm by filler
  matmuls during the initial DMA latency.

The program is built at call time from the actual valid_lens (cached by
(KTA, KTB)); batch A is the one with more key tiles and is processed
first so its longer attention phase starts as early as possible.
"""
import sys
if "/opt/trn_rl_repo" not in sys.path:
    sys.path.insert(0, "/opt/trn_rl_repo")
import os
import time
import numpy as np

B, SQ, SK, D, H, HD = 2, 2048, 2048, 1024, 16, 64
NEG = -1.0e6
N_CORES = 8
DL = 128          # d_local: 2 heads * 64
KD = D // 128     # contraction tiles over D

_NC_CACHE = {}
last_results = None
last_exec_wall_s = None


def _build(KTA, KTB, LCA=None, LCB=None):
    import concourse.bass as bass  # noqa: F401
    import concourse.tile as tile
    from concourse import bacc, mybir

    f32 = mybir.dt.float32
    f16 = mybir.dt.float16
    EXP = mybir.ActivationFunctionType.Exp

    LKA, LKB = KTA * 128, KTB * 128
    # K/V stream DMAs only carry the valid columns (rounded up to 8);
    # the SBUF tails are zero-filled so masked tail scores stay exact
    LCA = LKA if LCA is None else LCA
    LCB = LKB if LCB is None else LCB
    # [(k0, nk)] chunk groups for the k/v/weight streams (fewer, larger DMAs)
    kgrp = [(0, 4), (4, 4)]

    nc = bacc.Bacc("TRN2", target_bir_lowering=False, debug=False,
                   num_devices=N_CORES)
    xqT = nc.dram_tensor("xqT", [2, D, SQ], f16, kind="ExternalInput")
    xkTA = nc.dram_tensor("xkTA", [D, LKA], f16, kind="ExternalInput")
    xvTA = nc.dram_tensor("xvTA", [D, LKA], f16, kind="ExternalInput")
    xkTB = nc.dram_tensor("xkTB", [D, LKB], f16, kind="ExternalInput")
    xvTB = nc.dram_tensor("xvTB", [D, LKB], f16, kind="ExternalInput")
    wqkv = nc.dram_tensor("wqkv", [D, 3 * DL], f16, kind="ExternalInput")
    wo = nc.dram_tensor("wo", [DL, D], f16, kind="ExternalInput")
    maskA = nc.dram_tensor("maskA", [128, KTA], f32, kind="ExternalInput")
    maskB = nc.dram_tensor("maskB", [128, KTB], f32, kind="ExternalInput")
    out = nc.dram_tensor("out", [2, SQ, D], f16, kind="ExternalOutput")

    with tile.TileContext(nc) as tc:
        with tc.tile_pool(name="singles", bufs=1) as sg:
            wqkv_sb = sg.tile([128, KD, 3 * DL], f16)
            wo_sb = sg.tile([128, D], f16)
            maskA_sb = sg.tile([128, KTA], f32)
            maskB_sb = sg.tile([128, KTB], f32)
            kt_sb = {0: sg.tile([128, LKA], f16, name="ktA"),
                     1: sg.tile([128, LKB], f16, name="ktB")}
            qt_sb = {0: sg.tile([128, SQ], f16, name="qtA"),
                     1: sg.tile([128, SQ], f16, name="qtB")}
            v_sb = {0: sg.tile([128, KTA, 2, 128], f16, name="vA"),
                    1: sg.tile([128, KTB, 2, 128], f16, name="vB")}
            ctx_sb = {0: sg.tile([128, SQ], f16, name="ctxA"),
                      1: sg.tile([128, SQ], f16, name="ctxB")}
            warm_sb = sg.tile([128, 256], f16)

            KT = {0: KTA, 1: KTB}
            mask_sb = {0: maskA_sb, 1: maskB_sb}
            xkT = {0: xkTA, 1: xkTB}
            xvT = {0: xvTA, 1: xvTB}

            # V'' ones-columns (softmax denominator); dim columns are
            # overwritten by the V-projection evacuations below.
            nc.gpsimd.memset(v_sb[0], 1.0)
            nc.gpsimd.memset(v_sb[1], 1.0)
            nc.vector.memset(warm_sb, 0.0)

            # ---- input DMAs, arrival order = need order ----
            nc.sync.dma_start(
                out=wqkv_sb[:, 0:4, :],
                in_=wqkv[0:512, :].rearrange("(k p) j -> p k j", p=128))
            strm_cm = tc.tile_pool(name="streams", bufs=1)
            strm = strm_cm.__enter__()
            xk = {b: strm.tile([128, KD, 128 * KT[b]], f16, name=f"xk{b}")
                  for b in (0, 1)}
            xq = {b: strm.tile([128, KD, SQ], f16, name=f"xq{b}")
                  for b in (0, 1)}
            xv = {b: strm.tile([128, KD, 128 * KT[b]], f16, name=f"xv{b}")
                  for b in (0, 1)}
            LC = {0: LCA, 1: LCB}

            def dma_kv(b, which):
                src = xkT[b] if which == "k" else xvT[b]
                dst = xk[b] if which == "k" else xv[b]
                if LC[b] < 128 * KT[b]:
                    nc.gpsimd.memset(dst[:, :, LC[b]:], 0.0)
                for k0, nk in kgrp:
                    nc.sync.dma_start(
                        out=dst[:, k0:k0 + nk, 0:LC[b]],
                        in_=src[k0 * 128:(k0 + nk) * 128, 0:LC[b]]
                        .rearrange("(k p) j -> p k j", p=128))

            def dma_q(b):
                for k in range(KD):
                    nc.sync.dma_start(out=xq[b][:, k, :],
                                      in_=xqT[b, k * 128:(k + 1) * 128, :])

            def dma_q_cols(b):
                # column-chunk order: Q-projection pass ci becomes ready
                # as soon as chunk ci lands (contraction needs all k)
                for ci in range(4):
                    nc.sync.dma_start(
                        out=xq[b][:, :, ci * 512:(ci + 1) * 512],
                        in_=xqT[b, :, ci * 512:(ci + 1) * 512]
                        .rearrange("(k p) j -> p k j", p=128))

            dma_kv(0, "k")
            nc.sync.dma_start(
                out=wqkv_sb[:, 4:8, :],
                in_=wqkv[512:1024, :].rearrange("(k p) j -> p k j", p=128))
            nc.sync.dma_start(out=maskA_sb, in_=maskA[:, :])
            nc.sync.dma_start(out=maskB_sb, in_=maskB[:, :])
            dma_kv(0, "v")   # xv before xq: ctx never stalls on V''
            dma_q_cols(0)
            dma_kv(1, "k")
            dma_kv(1, "v")
            dma_q_cols(1)
            nc.sync.dma_start(out=wo_sb, in_=wo[:, :])

            # ---- PE p-state warmup: keep the tensor engine busy during the
            # initial DMA latency so real matmuls start at full clock; the
            # pool stays open so stream-gated phases can emit filler too ----
            psW_cm = tc.tile_pool(name="psW", bufs=1, space="PSUM")
            psW = psW_cm.__enter__()
            wp = psW.tile([128, 256], f32)

            def wfill(n):
                for _ in range(n):
                    nc.tensor.matmul(wp, warm_sb[:, 0:128], warm_sb,
                                     start=True, stop=True)

            wfill(22)

            def copy_eng(eng, dst, src):
                if eng == "v":
                    nc.vector.tensor_copy(dst, src)
                elif eng == "a":
                    nc.scalar.copy(dst, src)
                else:
                    nc.gpsimd.tensor_copy(dst, src)

            def proj_k(b, eng):
                # K^T[b] = Wk^T @ Xk^T : [128, LK_b]
                LK = 128 * KT[b]
                chunks = [(i * 512, min(512, LK - i * 512))
                          for i in range((LK + 511) // 512)]
                with tc.tile_pool(name=f"psA{b}", bufs=1, space="PSUM") as ps:
                    accs = [ps.tile([128, cw], f32, tag=f"kt{ci}",
                                    name=f"kt{b}_{ci}")
                            for ci, (c0, cw) in enumerate(chunks)]
                    for k in range(KD):
                        for ci, (c0, cw) in enumerate(chunks):
                            nc.tensor.matmul(accs[ci],
                                             wqkv_sb[:, k, 0:128],
                                             xk[b][:, k, c0:c0 + cw],
                                             start=(k == 0), stop=(k == KD - 1))
                    for ci, (c0, cw) in enumerate(chunks):
                        copy_eng(eng[ci % len(eng)],
                                 kt_sb[b][:, c0:c0 + cw], accs[ci])

            def proj_q(b, eng, wfill=None):
                # Q^T[b] = Wq^T @ Xq^T : [128, 2048]. wfill emits idle
                # matmuls between DMA-gated k-chunks to hold the PE p-state.
                with tc.tile_pool(name=f"psB{b}", bufs=1, space="PSUM") as ps:
                    accs = [ps.tile([128, 512], f32, tag=f"q{ci}",
                                    name=f"q{b}_{ci}") for ci in range(4)]
                    for k in range(KD):
                        for ci in range(4):
                            nc.tensor.matmul(accs[ci],
                                             wqkv_sb[:, k, 128:256],
                                             xq[b][:, k, ci * 512:(ci + 1) * 512],
                                             start=(k == 0), stop=(k == KD - 1))
                        if wfill is not None and k < KD - 1:
                            wfill(2)
                    for ci in range(4):
                        copy_eng(eng[ci % len(eng)],
                                 qt_sb[b][:, ci * 512:(ci + 1) * 512], accs[ci])

            def proj_v(b, eng):
                # V''[b] : [key, head, dim|ones]; one pool, per-group tags,
                # so a later group never waits an earlier group's evacs
                gs = [list(range(g0, min(g0 + 4, KT[b])))
                      for g0 in range(0, KT[b], 4)]
                if KT[b] > 7:   # bank budget: fall back to serial groups
                    gs = [[t] for t in range(KT[b])]
                with tc.tile_pool(name=f"psV{b}", bufs=1, space="PSUM") as ps:
                    for gi, g in enumerate(gs):
                        tag = f"vg{gi % 4}" if KT[b] > 7 else f"vg{gi}"
                        vacc = ps.tile([128, len(g), 512], f32, tag=tag,
                                       name=f"v{b}_{gi}")
                        for k in range(KD):
                            for vi, t in enumerate(g):
                                nc.tensor.matmul(
                                    vacc[:, vi, 0:DL],
                                    xv[b][:, k, t * 128:(t + 1) * 128],
                                    wqkv_sb[:, k, 256:384],
                                    start=(k == 0), stop=(k == KD - 1),
                                    skip_group_check=True)
                        for vi, t in enumerate(g):
                            for hh in range(2):
                                copy_eng(eng[(2 * vi + hh) % len(eng)],
                                         v_sb[b][:, t, hh, 0:64],
                                         vacc[:, vi, hh * 64:(hh + 1) * 64])

            def attend(b, pools, extra=None):
                # scores^T -> exp -> ctx'' per (head, 512-query-chunk,
                # key-tile), chunk-major so chunk c needs only Q column
                # pass c; ctx accumulates in a [128,512] 1-bank tile,
                # normalized per chunk. extra(i) interleaves foreign work.
                ptp, mp, psS, psC = pools
                nit = 0
                for cq in range(4):
                    for hh in range(2):
                        q0 = cq * 512
                        ctx_ps = psC.tile([128, 512], f32, tag="ctx",
                                          name=f"ctx{b}_{hh}_{cq}")
                        for t in range(KT[b]):
                            s_ps = psS.tile([128, 512], f32, tag="s")
                            nc.tensor.matmul(
                                s_ps,
                                kt_sb[b][hh * 64:hh * 64 + 64,
                                         t * 128:(t + 1) * 128],
                                qt_sb[b][hh * 64:hh * 64 + 64,
                                         q0:q0 + 512],
                                start=True, stop=True)
                            pt = ptp.tile([128, 512], f16, tag="pt")
                            nc.scalar.activation(
                                pt, s_ps, EXP,
                                bias=mask_sb[b][:, t:t + 1], scale=0.125)
                            nc.tensor.matmul(
                                ctx_ps, v_sb[b][:, t, hh, :], pt,
                                start=(t == 0), stop=(t == KT[b] - 1),
                                skip_group_check=True)
                            if extra is not None:
                                extra(nit)
                            nit += 1
                        # rows 64-127 all hold the softmax denominator
                        # (only one TensorTensor input may come from PSUM,
                        # so reciprocal lands in SBUF first)
                        rcb = mp.tile([64, 512], f32, tag="rcb")
                        nc.vector.reciprocal(rcb, ctx_ps[64:128, :])
                        nc.vector.tensor_mul(
                            ctx_sb[b][hh * 64:hh * 64 + 64, q0:q0 + 512],
                            ctx_ps[0:64, :], rcb)

            evac_ct = [0]

            def out_proj(b, pools, qr, engines):
                # partial out[b] rows = ctx''[b]^T @ Wo_local; [128,512]
                # PSUM grain; each query tile DMAs out as soon as its own
                # two evacuations land (finer DMA pipelining than pairs)
                psD, op = pools
                for qi in qr:
                    o_sb = op.tile([128, D], f16, tag=f"o{qi % 3}",
                                   name=f"osb{b}_{qi}")
                    for n in range(2):
                        o_ps = psD.tile([128, 512], f32, tag="x",
                                        name=f"o{b}_{qi}_{n}")
                        nc.tensor.matmul(o_ps,
                                         ctx_sb[b][:, qi * 128:(qi + 1) * 128],
                                         wo_sb[:, n * 512:(n + 1) * 512],
                                         start=True, stop=True)
                        eng = engines[evac_ct[0] % len(engines)]
                        evac_ct[0] += 1
                        copy_eng(eng, o_sb[:, n * 512:(n + 1) * 512], o_ps)
                    nc.sync.dma_start(
                        out=out[b, qi * 128:(qi + 1) * 128, :], in_=o_sb)

            # ---- batch A K/V projections ride the early streams; Q runs
            # as column passes so attention starts while Q still streams ----
            proj_k(0, ("v",))
            proj_v(0, ("v",))
            psW_cm.__exit__(None, None, None)

            # batch-B projections as single-PSUM-bank steps, interleaved
            # into batch-A's ACT-bound attention cadence (all on GPSIMD so
            # nothing queues behind DVE norms)
            def bsteps(psX):
                LKB_ = 128 * KT[1]

                def a1b_step(c0, cw):
                    acc = psX.tile([128, 512], f32, tag="x", name="xa")
                    for k in range(KD):
                        nc.tensor.matmul(acc[:, 0:cw], wqkv_sb[:, k, 0:128],
                                         xk[1][:, k, c0:c0 + cw],
                                         start=(k == 0), stop=(k == KD - 1))
                    copy_eng("v", kt_sb[1][:, c0:c0 + cw], acc[:, 0:cw])

                def vb_step(t):
                    acc = psX.tile([128, 512], f32, tag="x", name="xv")
                    for k in range(KD):
                        nc.tensor.matmul(acc[:, 0:DL],
                                         xv[1][:, k, t * 128:(t + 1) * 128],
                                         wqkv_sb[:, k, 256:384],
                                         start=(k == 0), stop=(k == KD - 1))
                    for hh in range(2):
                        copy_eng("v", v_sb[1][:, t, hh, 0:64],
                                 acc[:, hh * 64:(hh + 1) * 64])

                def qb_step(ci, b=1):
                    acc = psX.tile([128, 512], f32, tag="x", name="xq")
                    for k in range(KD):
                        nc.tensor.matmul(acc, wqkv_sb[:, k, 128:256],
                                         xq[b][:, k, ci * 512:(ci + 1) * 512],
                                         start=(k == 0), stop=(k == KD - 1))
                    copy_eng("v", qt_sb[b][:, ci * 512:(ci + 1) * 512], acc)

                steps = []
                for i in range((LKB_ + 511) // 512):
                    c0 = i * 512
                    steps.append(lambda c0=c0, cw=min(512, LKB_ - c0):
                                 a1b_step(c0, cw))
                steps.extend(lambda t=t: vb_step(t) for t in range(KT[1]))
                steps.extend(lambda ci=ci: qb_step(ci) for ci in range(4))
                return steps, qb_step

            # One continuous PSUM configuration from first attention to last
            # output tile: psS (2 banks, score rotation) + psC (4 banks, ctx
            # accumulator) + aux (2 banks, shared rotation for batch-B
            # projection steps, then both batches' out-projection PSUM).
            with tc.tile_pool(name="pt", bufs=6) as ptp, \
                 tc.tile_pool(name="misc", bufs=2) as mp, \
                 tc.tile_pool(name="ob", bufs=6) as op:
                with tc.tile_pool(name="psS", bufs=4, space="PSUM") as psS, \
                     tc.tile_pool(name="psC", bufs=2, space="PSUM") as psC, \
                     tc.tile_pool(name="aux", bufs=2, space="PSUM") as aux:
                    steps, qa_step = bsteps(aux)
                    # Batch A's Q column pass 0 gates the first group;
                    # passes 1-3 interleave ahead of the chunks that need
                    # them, tracking the xq column-DMA arrivals. Batch-B
                    # projection steps ride attend(0)'s second half.
                    qa_step(0, b=0)
                    nit_A = 8 * KT[0]
                    smap = {}
                    for i, frac in ((1, 0.15), (2, 0.375), (3, 0.55)):
                        it = max(i, int(nit_A * frac))
                        smap.setdefault(it, []).append(
                            lambda ci=i: qa_step(ci, b=0))
                    for j, s in enumerate(steps):
                        it = max(4, int(nit_A * (0.62 + 0.33 * j / len(steps))))
                        smap.setdefault(it, []).append(s)

                    # a small slice of batch A's out-projection (qi 0-3,
                    # query chunk 0, normalized after group 4) rides the tail
                    # of attend(0) so its out-DMAs start during the DMA lull
                    qmapA = {}

                    def extraA(i):
                        for s in smap.get(i, ()):
                            s()
                        if i in qmapA:
                            out_proj(0, (aux, op), qmapA[i], ("a", "v"))

                    attend(0, (ptp, mp, psS, psC), extra=extraA)
                    for it, fns in sorted(smap.items()):
                        if it >= nit_A:
                            for s in fns:
                                s()
                    done_A = sorted(q for qs in qmapA.values() for q in qs)
                    rest_A = [q for q in range(16) if q not in done_A]
                    # batch B attention carries the rest of batch A's
                    # out-projection, spread across its cadence
                    nb = 8 * KT[1]
                    qsched = {}
                    ns = max(1, nb - 1)
                    nr = len(rest_A)
                    for i in range(ns):
                        # later slots lean on ACT: its exps wind down while
                        # DVE still carries the trailing norms
                        eng = ("v", "v", "a") if i < ns // 2 else ("a", "v")
                        qsched[i] = (rest_A[nr * i // ns: nr * (i + 1) // ns],
                                     eng)

                    def extra(i):
                        if i in qsched:
                            qr, eng = qsched[i]
                            out_proj(0, (aux, op), qr, eng)

                    attend(1, (ptp, mp, psS, psC), extra=extra)
                # final out-projection in its own deep PSUM rotation so the
                # tail runs at the out-DMA rate, not the evacuation rate
                    # first tail tiles run from the still-open aux pool so
                    # PE flows into the tail while psD2's banks hand over
                    out_proj(1, (aux, op), range(0, 2), ("a", "v"))
                with tc.tile_pool(name="psD2", bufs=6, space="PSUM") as psD2:
                    out_proj(1, (psD2, op), range(2, 16), ("a", "v"))
            strm_cm.__exit__(None, None, None)
    nc.compile()
    return nc


def kernel(**inputs):
    global last_results, last_exec_wall_s
    from concourse.bass_utils import run_bass_kernel_spmd

    # BASS_TRACE needs the axon NTFF hook; disable tracing when the hook
    # module is unavailable so a stray env var cannot crash the run.
    if os.environ.get("BASS_TRACE"):
        try:
            from antenv import axon_hooks  # noqa: F401
        except Exception:
            os.environ["BASS_NEVER_TRACE"] = "1"

    q = np.asarray(inputs["queries"], dtype=np.float32)
    kx = np.asarray(inputs["keys"], dtype=np.float32)
    vx = np.asarray(inputs["values"], dtype=np.float32)
    vl = np.asarray(inputs["valid_lens"], dtype=np.int64).reshape(B)
    Wq = np.asarray(inputs["Wq"], dtype=np.float32)
    Wk = np.asarray(inputs["Wk"], dtype=np.float32)
    Wv = np.asarray(inputs["Wv"], dtype=np.float32)
    Wo = np.asarray(inputs["Wo"], dtype=np.float32)
    assert q.shape == (B, SQ, D) and kx.shape == (B, SK, D) and vx.shape == (B, SK, D)

    lens = np.clip(vl, 1, SK)
    KTs = [(int(l) + 127) // 128 for l in lens]
    # batch A = more key tiles, processed first
    bA = 0 if KTs[0] >= KTs[1] else 1
    bB = 1 - bA
    KTA, KTB = KTs[bA], KTs[bB]
    LKA, LKB = KTA * 128, KTB * 128

    LCA = min(LKA, -(-int(lens[bA]) // 8) * 8)
    LCB = min(LKB, -(-int(lens[bB]) // 8) * 8)
    key = (KTA, KTB, LCA, LCB)
    if key not in _NC_CACHE:
        _NC_CACHE[key] = _build(KTA, KTB, LCA, LCB)
    nc = _NC_CACHE[key]

    def m128(b, KT):
        m = np.where(np.arange(KT * 128) < lens[b], 0.0, NEG).astype(np.float32)
        return np.ascontiguousarray(m.reshape(KT, 128).T)

    xqT_full = np.ascontiguousarray(
        np.stack([q[bA].T, q[bB].T]).astype(np.float16))
    in_maps = []
    for c in range(N_CORES):
        cols = slice(DL * c, DL * (c + 1))
        in_maps.append({
            "xqT": xqT_full,
            "xkTA": np.ascontiguousarray(kx[bA, :LKA].T.astype(np.float16)),
            "xvTA": np.ascontiguousarray(vx[bA, :LKA].T.astype(np.float16)),
            "xkTB": np.ascontiguousarray(kx[bB, :LKB].T.astype(np.float16)),
            "xvTB": np.ascontiguousarray(vx[bB, :LKB].T.astype(np.float16)),
            "wqkv": np.ascontiguousarray(np.concatenate(
                [Wk[:, cols], Wq[:, cols], Wv[:, cols]],
                axis=1).astype(np.float16)),
            "wo": np.ascontiguousarray(Wo[cols, :].astype(np.float16)),
            "maskA": m128(bA, KTA),
            "maskB": m128(bB, KTB),
        })

    t0 = time.perf_counter()
    res = run_bass_kernel_spmd(nc, in_maps, core_ids=list(range(N_CORES)))
    last_exec_wall_s = time.perf_counter() - t0
    last_results = res

    outs = [res.results[c]["out"].astype(np.float32) for c in range(N_CORES)]
    acc = outs[0]
    for c in range(1, N_CORES):
        acc = acc + outs[c]
    full = np.empty((B, SQ, D), dtype=np.float32)
    full[bA] = acc[0]
    full[bB] = acc[1]
    return full


# revision 92
# speedup vs baseline: 1.1750x; 1.0025x over previous
"""Multi-head attention (B=2, S=2048, D=1024, H=16) on 8 Trainium2 cores.

Sharding: pure tensor-parallel over heads (Megatron): core c owns heads
{2c, 2c+1} (d_local = 128 columns of Wq/Wk/Wv, 128 rows of Wo) and
processes BOTH batches. Each core emits a [2, 2048, 1024] partial output
(row-parallel Wo); the host sums the 8 partials per batch.

Why: the SPMD program's attention work scales with KT0+KT1 (per 2 heads)
instead of 4*max(KT0,KT1) (per 4 heads) under the old batch x head-group
split, so key-length imbalance between the two batches no longer inflates
the program's critical path.

Key-side truncation: only ceil(valid_len/128) key tiles per batch are
computed; the per-batch mask rides the ScalarE exp as a per-partition
bias. Scores are computed transposed ([key, query]); the softmax
denominator comes free via 64 ones-columns appended to V (ones-trick).

Precision: fp16 streams/weights, fp32 PSUM accumulation (rel err ~8e-4).

Schedule (single instruction stream, deeply interleaved):
- Q streams in column chunks; attention runs per (512-query-chunk, head)
  so the first scores start after only the first Q column pass.
- Batch B's K/V/Q projections run as single-PSUM-bank steps interleaved
  into batch A's attention cadence; batch A's output projection rides
  batch B's attention; batch B's output projection is the only tail.
- PSUM: scores rotation 4x[128,512] + ctx accumulators 2 + shared aux 2
  (projection steps / out-proj) = 8 banks, one configuration end to end.
- PSUM evacuation is split across DVE and ACT (GPSIMD cannot touch PSUM);
  out-DMAs fire per query tile; PE p-state is kept warm by filler
  matmuls during the initial DMA latency.

The program is built at call time from the actual valid_lens (cached by
(KTA, KTB)); batch A is the one with more key tiles and is processed
first so its longer attention phase starts as early as possible.
"""
import sys
if "/opt/trn_rl_repo" not in sys.path:
    sys.path.insert(0, "/opt/trn_rl_repo")
import os
import time
import numpy as np

B, SQ, SK, D, H, HD = 2, 2048, 2048, 1024, 16, 64
NEG = -1.0e6
N_CORES = 8
DL = 128          # d_local: 2 heads * 64
KD = D // 128     # contraction tiles over D

_NC_CACHE = {}
last_results = None
last_exec_wall_s = None


def _build(KTA, KTB, LCA=None, LCB=None):
    import concourse.bass as bass  # noqa: F401
    import concourse.tile as tile
    from concourse import bacc, mybir

    f32 = mybir.dt.float32
    f16 = mybir.dt.float16
    EXP = mybir.ActivationFunctionType.Exp

    LKA, LKB = KTA * 128, KTB * 128
    # K/V stream DMAs only carry the valid columns (rounded up to 8);
    # the SBUF tails are zero-filled so masked tail scores stay exact
    LCA = LKA if LCA is None else LCA
    LCB = LKB if LCB is None else LCB
    # [(k0, nk)] chunk groups for the k/v/weight streams (fewer, larger DMAs)
    kgrp = [(0, 4), (4, 4)]

    nc = bacc.Bacc("TRN2", target_bir_lowering=False, debug=False,
                   num_devices=N_CORES)
    xqT = nc.dram_tensor("xqT", [2, D, SQ], f16, kind="ExternalInput")
    xkTA = nc.dram_tensor("xkTA", [D, LKA], f16, kind="ExternalInput")
    xvTA = nc.dram_tensor("xvTA", [D, LKA], f16, kind="ExternalInput")
    xkTB = nc.dram_tensor("xkTB", [D, LKB], f16, kind="ExternalInput")
    xvTB = nc.dram_tensor("xvTB", [D, LKB], f16, kind="ExternalInput")
    wqkv = nc.dram_tensor("wqkv", [D, 3 * DL], f16, kind="ExternalInput")
    wo = nc.dram_tensor("wo", [DL, D], f16, kind="ExternalInput")
    maskA = nc.dram_tensor("maskA", [128, KTA], f32, kind="ExternalInput")
    maskB = nc.dram_tensor("maskB", [128, KTB], f32, kind="ExternalInput")
    out = nc.dram_tensor("out", [2, SQ, D], f16, kind="ExternalOutput")

    with tile.TileContext(nc) as tc:
        with tc.tile_pool(name="singles", bufs=1) as sg:
            wqkv_sb = sg.tile([128, KD, 3 * DL], f16)
            wo_sb = sg.tile([128, D], f16)
            maskA_sb = sg.tile([128, KTA], f32)
            maskB_sb = sg.tile([128, KTB], f32)
            kt_sb = {0: sg.tile([128, LKA], f16, name="ktA"),
                     1: sg.tile([128, LKB], f16, name="ktB")}
            qt_sb = {0: sg.tile([128, SQ], f16, name="qtA"),
                     1: sg.tile([128, SQ], f16, name="qtB")}
            v_sb = {0: sg.tile([128, KTA, 2, 128], f16, name="vA"),
                    1: sg.tile([128, KTB, 2, 128], f16, name="vB")}
            ctx_sb = {0: sg.tile([128, SQ], f16, name="ctxA"),
                      1: sg.tile([128, SQ], f16, name="ctxB")}
            warm_sb = sg.tile([128, 256], f16)

            KT = {0: KTA, 1: KTB}
            mask_sb = {0: maskA_sb, 1: maskB_sb}
            xkT = {0: xkTA, 1: xkTB}
            xvT = {0: xvTA, 1: xvTB}

            # V'' ones-columns (softmax denominator); dim columns are
            # overwritten by the V-projection evacuations below.
            nc.gpsimd.memset(v_sb[0], 1.0)
            nc.gpsimd.memset(v_sb[1], 1.0)
            nc.vector.memset(warm_sb, 0.0)

            # ---- input DMAs, arrival order = need order ----
            nc.sync.dma_start(
                out=wqkv_sb[:, 0:4, :],
                in_=wqkv[0:512, :].rearrange("(k p) j -> p k j", p=128))
            strm_cm = tc.tile_pool(name="streams", bufs=1)
            strm = strm_cm.__enter__()
            xk = {b: strm.tile([128, KD, 128 * KT[b]], f16, name=f"xk{b}")
                  for b in (0, 1)}
            xq = {b: strm.tile([128, KD, SQ], f16, name=f"xq{b}")
                  for b in (0, 1)}
            xv = {b: strm.tile([128, KD, 128 * KT[b]], f16, name=f"xv{b}")
                  for b in (0, 1)}
            LC = {0: LCA, 1: LCB}

            def dma_kv(b, which):
                src = xkT[b] if which == "k" else xvT[b]
                dst = xk[b] if which == "k" else xv[b]
                if LC[b] < 128 * KT[b]:
                    nc.gpsimd.memset(dst[:, :, LC[b]:], 0.0)
                for k0, nk in kgrp:
                    nc.sync.dma_start(
                        out=dst[:, k0:k0 + nk, 0:LC[b]],
                        in_=src[k0 * 128:(k0 + nk) * 128, 0:LC[b]]
                        .rearrange("(k p) j -> p k j", p=128))

            def dma_q(b):
                for k in range(KD):
                    nc.sync.dma_start(out=xq[b][:, k, :],
                                      in_=xqT[b, k * 128:(k + 1) * 128, :])

            def dma_q_cols(b, first=None):
                # column-chunk order: Q-projection pass ci becomes ready
                # as soon as chunk ci lands (contraction needs all k)
                cis = range(4) if first is None else (
                    range(0, 1) if first else range(1, 4))
                for ci in cis:
                    nc.sync.dma_start(
                        out=xq[b][:, :, ci * 512:(ci + 1) * 512],
                        in_=xqT[b, :, ci * 512:(ci + 1) * 512]
                        .rearrange("(k p) j -> p k j", p=128))

            dma_kv(0, "k")
            nc.sync.dma_start(
                out=wqkv_sb[:, 4:8, :],
                in_=wqkv[512:1024, :].rearrange("(k p) j -> p k j", p=128))
            dma_kv(0, "v")   # xv before xq: ctx never stalls on V''
            dma_q_cols(0, first=1)
            # the tiny mask DMAs hold the descriptor stage for ~625ns each;
            # issued after the attention-gating Q chunk, before the first exp
            nc.sync.dma_start(out=maskA_sb, in_=maskA[:, :])
            nc.sync.dma_start(out=maskB_sb, in_=maskB[:, :])
            dma_q_cols(0, first=0)
            dma_kv(1, "k")
            dma_kv(1, "v")
            dma_q_cols(1)
            nc.sync.dma_start(out=wo_sb, in_=wo[:, :])

            # ---- PE p-state warmup: keep the tensor engine busy during the
            # initial DMA latency so real matmuls start at full clock; the
            # pool stays open so stream-gated phases can emit filler too ----
            psW_cm = tc.tile_pool(name="psW", bufs=1, space="PSUM")
            psW = psW_cm.__enter__()
            wp = psW.tile([128, 256], f32)

            def wfill(n):
                for _ in range(n):
                    nc.tensor.matmul(wp, warm_sb[:, 0:128], warm_sb,
                                     start=True, stop=True)

            wfill(22)

            def copy_eng(eng, dst, src):
                if eng == "v":
                    nc.vector.tensor_copy(dst, src)
                elif eng == "a":
                    nc.scalar.copy(dst, src)
                else:
                    nc.gpsimd.tensor_copy(dst, src)

            def proj_k(b, eng):
                # K^T[b] = Wk^T @ Xk^T : [128, LK_b]
                LK = 128 * KT[b]
                chunks = [(i * 512, min(512, LK - i * 512))
                          for i in range((LK + 511) // 512)]
                with tc.tile_pool(name=f"psA{b}", bufs=1, space="PSUM") as ps:
                    accs = [ps.tile([128, cw], f32, tag=f"kt{ci}",
                                    name=f"kt{b}_{ci}")
                            for ci, (c0, cw) in enumerate(chunks)]
                    for k in range(KD):
                        for ci, (c0, cw) in enumerate(chunks):
                            nc.tensor.matmul(accs[ci],
                                             wqkv_sb[:, k, 0:128],
                                             xk[b][:, k, c0:c0 + cw],
                                             start=(k == 0), stop=(k == KD - 1))
                    for ci, (c0, cw) in enumerate(chunks):
                        copy_eng(eng[ci % len(eng)],
                                 kt_sb[b][:, c0:c0 + cw], accs[ci])

            def proj_q(b, eng, wfill=None):
                # Q^T[b] = Wq^T @ Xq^T : [128, 2048]. wfill emits idle
                # matmuls between DMA-gated k-chunks to hold the PE p-state.
                with tc.tile_pool(name=f"psB{b}", bufs=1, space="PSUM") as ps:
                    accs = [ps.tile([128, 512], f32, tag=f"q{ci}",
                                    name=f"q{b}_{ci}") for ci in range(4)]
                    for k in range(KD):
                        for ci in range(4):
                            nc.tensor.matmul(accs[ci],
                                             wqkv_sb[:, k, 128:256],
                                             xq[b][:, k, ci * 512:(ci + 1) * 512],
                                             start=(k == 0), stop=(k == KD - 1))
                        if wfill is not None and k < KD - 1:
                            wfill(2)
                    for ci in range(4):
                        copy_eng(eng[ci % len(eng)],
                                 qt_sb[b][:, ci * 512:(ci + 1) * 512], accs[ci])

            def proj_v(b, eng):
                # V''[b] : [key, head, dim|ones]; one pool, per-group tags,
                # so a later group never waits an earlier group's evacs
                gs = [list(range(g0, min(g0 + 4, KT[b])))
                      for g0 in range(0, KT[b], 4)]
                if KT[b] > 7:   # bank budget: fall back to serial groups
                    gs = [[t] for t in range(KT[b])]
                with tc.tile_pool(name=f"psV{b}", bufs=1, space="PSUM") as ps:
                    for gi, g in enumerate(gs):
                        tag = f"vg{gi % 4}" if KT[b] > 7 else f"vg{gi}"
                        vacc = ps.tile([128, len(g), 512], f32, tag=tag,
                                       name=f"v{b}_{gi}")
                        for k in range(KD):
                            for vi, t in enumerate(g):
                                nc.tensor.matmul(
                                    vacc[:, vi, 0:DL],
                                    xv[b][:, k, t * 128:(t + 1) * 128],
                                    wqkv_sb[:, k, 256:384],
                                    start=(k == 0), stop=(k == KD - 1),
                                    skip_group_check=True)
                        for vi, t in enumerate(g):
                            for hh in range(2):
                                copy_eng(eng[(2 * vi + hh) % len(eng)],
                                         v_sb[b][:, t, hh, 0:64],
                                         vacc[:, vi, hh * 64:(hh + 1) * 64])

            def attend(b, pools, extra=None):
                # scores^T -> exp -> ctx'' per (head, 512-query-chunk,
                # key-tile), chunk-major so chunk c needs only Q column
                # pass c; ctx accumulates in a [128,512] 1-bank tile,
                # normalized per chunk. extra(i) interleaves foreign work.
                ptp, mp, psS, psC = pools
                nit = 0
                for cq in range(4):
                    for hh in range(2):
                        q0 = cq * 512
                        ctx_ps = psC.tile([128, 512], f32, tag="ctx",
                                          name=f"ctx{b}_{hh}_{cq}")
                        for t in range(KT[b]):
                            s_ps = psS.tile([128, 512], f32, tag="s")
                            nc.tensor.matmul(
                                s_ps,
                                kt_sb[b][hh * 64:hh * 64 + 64,
                                         t * 128:(t + 1) * 128],
                                qt_sb[b][hh * 64:hh * 64 + 64,
                                         q0:q0 + 512],
                                start=True, stop=True)
                            pt = ptp.tile([128, 512], f16, tag="pt")
                            nc.scalar.activation(
                                pt, s_ps, EXP,
                                bias=mask_sb[b][:, t:t + 1], scale=0.125)
                            nc.tensor.matmul(
                                ctx_ps, v_sb[b][:, t, hh, :], pt,
                                start=(t == 0), stop=(t == KT[b] - 1),
                                skip_group_check=True)
                            if extra is not None:
                                extra(nit)
                            nit += 1
                        # rows 64-127 all hold the softmax denominator
                        # (only one TensorTensor input may come from PSUM,
                        # so reciprocal lands in SBUF first)
                        rcb = mp.tile([64, 512], f32, tag="rcb")
                        nc.vector.reciprocal(rcb, ctx_ps[64:128, :])
                        nc.vector.tensor_mul(
                            ctx_sb[b][hh * 64:hh * 64 + 64, q0:q0 + 512],
                            ctx_ps[0:64, :], rcb)

            evac_ct = [0]

            def out_proj(b, pools, qr, engines):
                # partial out[b] rows = ctx''[b]^T @ Wo_local; [128,512]
                # PSUM grain; each query tile DMAs out as soon as its own
                # two evacuations land (finer DMA pipelining than pairs)
                psD, op = pools
                for qi in qr:
                    o_sb = op.tile([128, D], f16, tag=f"o{qi % 3}",
                                   name=f"osb{b}_{qi}")
                    for n in range(2):
                        o_ps = psD.tile([128, 512], f32, tag="x",
                                        name=f"o{b}_{qi}_{n}")
                        nc.tensor.matmul(o_ps,
                                         ctx_sb[b][:, qi * 128:(qi + 1) * 128],
                                         wo_sb[:, n * 512:(n + 1) * 512],
                                         start=True, stop=True)
                        eng = engines[evac_ct[0] % len(engines)]
                        evac_ct[0] += 1
                        copy_eng(eng, o_sb[:, n * 512:(n + 1) * 512], o_ps)
                    nc.sync.dma_start(
                        out=out[b, qi * 128:(qi + 1) * 128, :], in_=o_sb)

            # ---- batch A K/V projections ride the early streams; Q runs
            # as column passes so attention starts while Q still streams ----
            proj_k(0, ("v",))
            proj_v(0, ("v",))
            psW_cm.__exit__(None, None, None)

            # batch-B projections as single-PSUM-bank steps, interleaved
            # into batch-A's ACT-bound attention cadence (all on GPSIMD so
            # nothing queues behind DVE norms)
            def bsteps(psX):
                LKB_ = 128 * KT[1]

                def a1b_step(c0, cw):
                    acc = psX.tile([128, 512], f32, tag="x", name="xa")
                    for k in range(KD):
                        nc.tensor.matmul(acc[:, 0:cw], wqkv_sb[:, k, 0:128],
                                         xk[1][:, k, c0:c0 + cw],
                                         start=(k == 0), stop=(k == KD - 1))
                    copy_eng("v", kt_sb[1][:, c0:c0 + cw], acc[:, 0:cw])

                def vb_step(t):
                    acc = psX.tile([128, 512], f32, tag="x", name="xv")
                    for k in range(KD):
                        nc.tensor.matmul(acc[:, 0:DL],
                                         xv[1][:, k, t * 128:(t + 1) * 128],
                                         wqkv_sb[:, k, 256:384],
                                         start=(k == 0), stop=(k == KD - 1))
                    for hh in range(2):
                        copy_eng("v", v_sb[1][:, t, hh, 0:64],
                                 acc[:, hh * 64:(hh + 1) * 64])

                def qb_step(ci, b=1):
                    acc = psX.tile([128, 512], f32, tag="x", name="xq")
                    for k in range(KD):
                        nc.tensor.matmul(acc, wqkv_sb[:, k, 128:256],
                                         xq[b][:, k, ci * 512:(ci + 1) * 512],
                                         start=(k == 0), stop=(k == KD - 1))
                    copy_eng("v", qt_sb[b][:, ci * 512:(ci + 1) * 512], acc)

                steps = []
                for i in range((LKB_ + 511) // 512):
                    c0 = i * 512
                    steps.append(lambda c0=c0, cw=min(512, LKB_ - c0):
                                 a1b_step(c0, cw))
                steps.extend(lambda t=t: vb_step(t) for t in range(KT[1]))
                steps.extend(lambda ci=ci: qb_step(ci) for ci in range(4))
                return steps, qb_step

            # One continuous PSUM configuration from first attention to last
            # output tile: psS (2 banks, score rotation) + psC (4 banks, ctx
            # accumulator) + aux (2 banks, shared rotation for batch-B
            # projection steps, then both batches' out-projection PSUM).
            with tc.tile_pool(name="pt", bufs=6) as ptp, \
                 tc.tile_pool(name="misc", bufs=2) as mp, \
                 tc.tile_pool(name="ob", bufs=6) as op:
                with tc.tile_pool(name="psS", bufs=4, space="PSUM") as psS, \
                     tc.tile_pool(name="psC", bufs=2, space="PSUM") as psC, \
                     tc.tile_pool(name="aux", bufs=2, space="PSUM") as aux:
                    steps, qa_step = bsteps(aux)
                    # Batch A's Q column pass 0 gates the first group;
                    # passes 1-3 interleave ahead of the chunks that need
                    # them, tracking the xq column-DMA arrivals. Batch-B
                    # projection steps ride attend(0)'s second half.
                    qa_step(0, b=0)
                    nit_A = 8 * KT[0]
                    smap = {}
                    for i, frac in ((1, 0.15), (2, 0.375), (3, 0.55)):
                        it = max(i, int(nit_A * frac))
                        smap.setdefault(it, []).append(
                            lambda ci=i: qa_step(ci, b=0))
                    for j, s in enumerate(steps):
                        it = max(4, int(nit_A * (0.62 + 0.33 * j / len(steps))))
                        smap.setdefault(it, []).append(s)

                    # a small slice of batch A's out-projection (qi 0-3,
                    # query chunk 0, normalized after group 4) rides the tail
                    # of attend(0) so its out-DMAs start during the DMA lull
                    qmapA = {}

                    def extraA(i):
                        for s in smap.get(i, ()):
                            s()
                        if i in qmapA:
                            out_proj(0, (aux, op), qmapA[i], ("a", "v"))

                    attend(0, (ptp, mp, psS, psC), extra=extraA)
                    for it, fns in sorted(smap.items()):
                        if it >= nit_A:
                            for s in fns:
                                s()
                    done_A = sorted(q for qs in qmapA.values() for q in qs)
                    rest_A = [q for q in range(16) if q not in done_A]
                    # batch B attention carries the rest of batch A's
                    # out-projection, spread across its cadence
                    nb = 8 * KT[1]
                    qsched = {}
                    ns = max(1, nb - 1)
                    nr = len(rest_A)
                    for i in range(ns):
                        # later slots lean on ACT: its exps wind down while
                        # DVE still carries the trailing norms
                        eng = ("v", "v", "a") if i < ns // 2 else ("a", "v")
                        qsched[i] = (rest_A[nr * i // ns: nr * (i + 1) // ns],
                                     eng)

                    def extra(i):
                        if i in qsched:
                            qr, eng = qsched[i]
                            out_proj(0, (aux, op), qr, eng)

                    attend(1, (ptp, mp, psS, psC), extra=extra)
                # final out-projection in its own deep PSUM rotation so the
                # tail runs at the out-DMA rate, not the evacuation rate
                    # first tail tiles run from the still-open aux pool so
                    # PE flows into the tail while psD2's banks hand over
                    out_proj(1, (aux, op), range(0, 2), ("a", "v"))
                with tc.tile_pool(name="psD2", bufs=6, space="PSUM") as psD2:
                    out_proj(1, (psD2, op), range(2, 16), ("a", "v"))
            strm_cm.__exit__(None, None, None)
    nc.compile()
    return nc


def kernel(**inputs):
    global last_results, last_exec_wall_s
    from concourse.bass_utils import run_bass_kernel_spmd

    # BASS_TRACE needs the axon NTFF hook; disable tracing when the hook
    # module is unavailable so a stray env var cannot crash the run.
    if os.environ.get("BASS_TRACE"):
        try:
            from antenv import axon_hooks  # noqa: F401
        except Exception:
            os.environ["BASS_NEVER_TRACE"] = "1"

    q = np.asarray(inputs["queries"], dtype=np.float32)
    kx = np.asarray(inputs["keys"], dtype=np.float32)
    vx = np.asarray(inputs["values"], dtype=np.float32)
    vl = np.asarray(inputs["valid_lens"], dtype=np.int64).reshape(B)
    Wq = np.asarray(inputs["Wq"], dtype=np.float32)
    Wk = np.asarray(inputs["Wk"], dtype=np.float32)
    Wv = np.asarray(inputs["Wv"], dtype=np.float32)
    Wo = np.asarray(inputs["Wo"], dtype=np.float32)
    assert q.shape == (B, SQ, D) and kx.shape == (B, SK, D) and vx.shape == (B, SK, D)

    lens = np.clip(vl, 1, SK)
    KTs = [(int(l) + 127) // 128 for l in lens]
    # batch A = more key tiles, processed first
    bA = 0 if KTs[0] >= KTs[1] else 1
    bB = 1 - bA
    KTA, KTB = KTs[bA], KTs[bB]
    LKA, LKB = KTA * 128, KTB * 128

    LCA = min(LKA, -(-int(lens[bA]) // 8) * 8)
    LCB = min(LKB, -(-int(lens[bB]) // 8) * 8)
    key = (KTA, KTB, LCA, LCB)
    if key not in _NC_CACHE:
        _NC_CACHE[key] = _build(KTA, KTB, LCA, LCB)
    nc = _NC_CACHE[key]

    def m128(b, KT):
        m = np.where(np.arange(KT * 128) < lens[b], 0.0, NEG).astype(np.float32)
        return np.ascontiguousarray(m.reshape(KT, 128).T)

    xqT_full = np.ascontiguousarray(
        np.stack([q[bA].T, q[bB].T]).astype(np.float16))
    in_maps = []
    for c in range(N_CORES):
        cols = slice(DL * c, DL * (c + 1))
        in_maps.append({
            "xqT": xqT_full,
            "xkTA": np.ascontiguousarray(kx[bA, :LKA].T.astype(np.float16)),
            "xvTA": np.ascontiguousarray(vx[bA, :LKA].T.astype(np.float16)),
            "xkTB": np.ascontiguousarray(kx[bB, :LKB].T.astype(np.float16)),
            "xvTB": np.ascontiguousarray(vx[bB, :LKB].T.astype(np.float16)),
            "wqkv": np.ascontiguousarray(np.concatenate(
                [Wk[:, cols], Wq[:, cols], Wv[:, cols]],
                axis=1).astype(np.float16)),
            "wo": np.ascontiguousarray(Wo[cols, :].astype(np.float16)),
            "maskA": m128(bA, KTA),
            "maskB": m128(bB, KTB),
        })

    t0 = time.perf_counter()
    res = run_bass_kernel_spmd(nc, in_maps, core_ids=list(range(N_CORES)))
    last_exec_wall_s = time.perf_counter() - t0
    last_results = res

    outs = [res.results[c]["out"].astype(np.float32) for c in range(N_CORES)]
    acc = outs[0]
    for c in range(1, N_CORES):
        acc = acc + outs[c]
    full = np.empty((B, SQ, D), dtype=np.float32)
    full[bA] = acc[0]
    full[bB] = acc[1]
    return full
